# revision 12
# baseline (speedup 1.0000x reference)
"""Trainium2 Bass kernel for the STU (spectral transform unit) block. v2.

Strategy
--------
Time-shard the sequence across 8 cores (256 output steps each, halos for
causal history). Each core runs an identical SPMD program:

  rmsnorm -> causal filter-bank convolution as block-Toeplitz matmuls
  (per-filter lag truncation, filters sr-weighted) -> (k,d)->o contraction
  + AR-on-inputs taps -> output AR scan as a truncated matrix-tap
  convolution -> SwiGLU MLP -> residuals.

v2: all matmuls in bf16 (enables the compiler's fast-weight-load path;
fp32r disables it), filter truncation capped at 4 lag blocks, scan taps
truncated at J=12, the scan-halo block narrowed from 128 to 32 steps
(its filter columns are a strided slice of the main Toeplitz bank), a
single +-bank shared by conv(u) and conv(alt*u), and rms_w folded into
the contraction weights. PSUM->SBUF casts round-robin across the
vector/scalar/gpsimd engines.
"""

import contextlib
import numpy as np

# ---------------- problem constants (hardcoded shapes) ----------------
B, T, D, K, KU, KY, H = 4, 2048, 256, 24, 3, 2, 1024
NCORES = 8
TB = T // NCORES          # 256 output timesteps per core
C = 128                   # conv / tile block

# per-filter truncation: number of 128-lag blocks kept for each k (0..23)
NB = [1, 1, 1, 1, 1, 1, 1, 1, 1, 2, 3, 3, 3, 4, 4, 4, 4, 3, 2, 2, 2, 1, 1, 1]
J = 12                    # scan taps
GS = 4                    # filters per conv group
HW = 32                   # scan-halo width (needs >= J-1)
PRE = 4                   # history blocks before the core's 2-block window
NXB = PRE + 2             # u/x window blocks per core
NIB = 3                   # output regions per core: halo(32) + 2 full blocks
YW = HW + 2 * C           # y window width (288)

_ORDER = sorted(range(K), key=lambda k: -NB[k])
_GROUPS = [_ORDER[i * GS:(i + 1) * GS] for i in range(K // GS)]
# remainder lag-block only for nb=1 filters (concentrated); long filters'
# parallelogram truncation error matches their tail plateau anyway
_GNB = [max((NB[k] + 1 if NB[k] == 1 else NB[k]) for k in g) for g in _GROUPS]
NG = len(_GROUPS)
NBT = sum(_GNB)           # total (g, m) lag blocks in the bank (15)
_GOFF = [sum(_GNB[:i]) for i in range(NG)]

_BUILT = {}


def _build_program():
    import concourse.bacc as bacc
    import concourse.tile as tile
    import concourse.mybir as mybir

    f32 = mybir.dt.float32
    bf16 = mybir.dt.bfloat16
    AF = mybir.ActivationFunctionType

    nc = bacc.Bacc("TRN2", target_bir_lowering=False, debug=False,
                   num_devices=NCORES)

    # ---------------- DRAM tensors ----------------
    xw_ap = nc.dram_tensor("xw", [B, 2 * C, D], f32, kind="ExternalInput").ap()
    xb_ap = nc.dram_tensor("xb", [B, NXB * C, D], bf16, kind="ExternalInput").ap()
    bb_ap = nc.dram_tensor("bb", [C, NBT * GS * C], bf16, kind="ExternalInput").ap()
    mm_ap = nc.dram_tensor("mm", [C, NG * GS * 2 * 2 * D], bf16, kind="ExternalInput").ap()
    mu_ap = nc.dram_tensor("mu", [C, KU * 2 * D], bf16, kind="ExternalInput").ap()
    tp_ap = nc.dram_tensor("tp", [C, J * 2 * D], bf16, kind="ExternalInput").ap()
    w1_ap = nc.dram_tensor("w1", [C, 2 * H], bf16, kind="ExternalInput").ap()
    vv_ap = nc.dram_tensor("vv", [C, 2 * H], bf16, kind="ExternalInput").ap()
    w2_ap = nc.dram_tensor("w2", [C, 8 * D], bf16, kind="ExternalInput").ap()
    al_ap = nc.dram_tensor("al", [C], f32, kind="ExternalInput").ap()
    alh_ap = nc.dram_tensor("alh", [GS * HW], bf16, kind="ExternalInput").ap()
    bh_ap = nc.dram_tensor("bh", [C, NBT * 2 * GS * HW], bf16, kind="ExternalInput").ap()
    ey_ap = nc.dram_tensor("ey", [C, C], f32, kind="ExternalInput").ap()
    out_ap = nc.dram_tensor("out", [B, TB, D], f32, kind="ExternalOutput").ap()

    import concourse.bass as bass

    with tile.TileContext(nc) as tc:
        ctx = contextlib.ExitStack()
        with ctx:
            p0 = ctx.enter_context(tc.tile_pool(name="p0", bufs=1))
            pc = ctx.enter_context(tc.tile_pool(name="pc", bufs=1))
            small = ctx.enter_context(tc.tile_pool(name="small", bufs=4))
            ppc = ctx.enter_context(tc.tile_pool(name="ppc", bufs=1, space="PSUM"))
            ppt = ctx.enter_context(tc.tile_pool(name="ppt", bufs=2, space="PSUM"))
            ppm = ctx.enter_context(tc.tile_pool(name="ppm", bufs=1, space="PSUM"))

            # ---------------- constants ----------------
            eye = p0.tile([C, C], f32)
            nc.sync.dma_start(out=eye[:], in_=ey_ap)
            eyeb = p0.tile([C, C], bf16)
            nc.vector.tensor_copy(out=eyeb[:], in_=eye[:])
            altc = p0.tile([C, 1], f32)
            nc.sync.dma_start(
                out=altc[:],
                in_=bass.AP(tensor=al_ap.tensor, offset=al_ap.offset,
                            ap=[[1, C], [0, 1]]))
            epst = p0.tile([C, 1], f32)
            nc.vector.memset(epst[:], 1e-6)
            ones = p0.tile([C, D], f32)
            nc.vector.memset(ones[:], 1.0)
            altB = p0.tile([C, D], bf16)
            nc.scalar.activation(out=altB[:], in_=ones[:], func=AF.Copy,
                                 scale=altc[:])
            alth = p0.tile([C, GS * HW], bf16)
            nc.sync.dma_start(
                out=alth[:],
                in_=bass.AP(tensor=alh_ap.tensor, offset=alh_ap.offset,
                            ap=[[0, C], [1, GS * HW]]))

            # ---------------- weights (DMA'd in order of first use) ----------------
            bbt = p0.tile([C, NBT, GS, C], bf16)
            bht = p0.tile([C, NBT, 2, GS, HW], bf16)
            mmt = p0.tile([C, NG, GS * 2, 2, D], bf16)
            mut = p0.tile([C, KU, 2, D], bf16)
            tpt = p0.tile([C, J, 2, D], bf16)

            def dma_bb(g):
                nc.gpsimd.dma_start(
                    out=bbt[:, _GOFF[g]:_GOFF[g] + _GNB[g], :, :].rearrange(
                        "p a b c -> p (a b c)"),
                    in_=bb_ap[:, _GOFF[g] * GS * C:(_GOFF[g] + _GNB[g]) * GS * C])

            def dma_mm(g):
                nc.gpsimd.dma_start(
                    out=mmt[:, g].rearrange("p a b c -> p (a b c)"),
                    in_=mm_ap[:, g * GS * 2 * 2 * D:(g + 1) * GS * 2 * 2 * D])

            nc.gpsimd.dma_start(
                out=bht[:].rearrange("p a b c d -> p (a b c d)"), in_=bh_ap)
            dma_bb(0)
            nc.gpsimd.dma_start(out=mut[:].rearrange("p a b c -> p (a b c)"), in_=mu_ap)
            dma_mm(0)
            for g in range(1, NG):
                dma_bb(g)
                dma_mm(g)
            nc.gpsimd.dma_start(out=tpt[:].rearrange("p a b c -> p (a b c)"), in_=tp_ap)
            w1t = p0.tile([C, 2, H], bf16)
            nc.gpsimd.dma_start(out=w1t[:].rearrange("p a b -> p (a b)"), in_=w1_ap)
            vvt = p0.tile([C, 2, H], bf16)
            nc.gpsimd.dma_start(out=vvt[:].rearrange("p a b -> p (a b)"), in_=vv_ap)
            w2t = p0.tile([C, 8, D], bf16)
            nc.gpsimd.dma_start(out=w2t[:].rearrange("p a b -> p (a b)"), in_=w2_ap)

            # persistent activation stores
            y_st = pc.tile([C, 2, B, YW], bf16)   # spectral+ar accum (o x (b,t))
            h_st = pc.tile([C, 2, B, TB], bf16)
            xr = pc.tile([C, 2, B, D], f32)
            for w in range(2):
                for b in range(B):
                    nc.sync.dma_start(out=xr[:, w, b, :],
                                      in_=xw_ap[b, w * C:(w + 1) * C, :])

            with tc.tile_pool(name="pa", bufs=1) as pa, \
                 tc.tile_pool(name="pb", bufs=1) as pb:
                # ---------------- phase A: rmsnorm (+ alt copy) ----------------
                u_all = pa.tile([C, NXB, B, D], bf16)
                v_all = pa.tile([C, NXB, B, D], bf16)
                for blk in range(NXB):
                    for b in range(B):
                        xt = pb.tile([C, D], bf16, tag="xt", bufs=3)
                        nc.sync.dma_start(out=xt[:], in_=xb_ap[b, blk * C:(blk + 1) * C, :])
                        sq = pb.tile([C, D], bf16, tag="sq", bufs=1)
                        ssum = small.tile([C, 1], f32, tag="ssum")
                        nc.scalar.activation(out=sq[:], in_=xt[:], func=AF.Square,
                                             accum_out=ssum[:])
                        nc.scalar.activation(out=ssum[:], in_=ssum[:], func=AF.Sqrt,
                                             bias=epst[:], scale=1.0 / D)
                        nc.vector.reciprocal(out=ssum[:], in_=ssum[:])
                        nc.scalar.activation(out=u_all[:, blk, b, :], in_=xt[:],
                                             func=AF.Copy, scale=ssum[:])
                        nc.vector.tensor_mul(out=v_all[:, blk, b, :],
                                             in0=u_all[:, blk, b, :], in1=altB[:])

                # u^T for AR-on-inputs taps: blocks PRE-2 .. PRE+1
                uT = pa.tile([C, 2, B, 4 * C], bf16)

                def do_uT():
                    cnt = 0
                    for w in range(4):
                        blk = PRE - 2 + w
                        for b in range(B):
                            for dh in range(2):
                                tps = ppm.tile([C, C], bf16, tag="tr", bufs=2)
                                nc.tensor.transpose(
                                    tps[:], u_all[:, blk, b, dh * C:(dh + 1) * C], eyeb[:])
                                if cnt % 2 == 0:
                                    nc.scalar.activation(
                                        out=uT[:, dh, b, w * C:(w + 1) * C], in_=tps[:],
                                        func=AF.Copy)
                                else:
                                    nc.vector.tensor_copy(
                                        out=uT[:, dh, b, w * C:(w + 1) * C], in_=tps[:])
                                cnt += 1

                # ---------------- phase B: conv + contraction ----------------
                # order: conv(g0,i0) fills the PE while phase A drains, then
                # uT/AR, then the remaining conv blocks.
                cast_rr = 0

                def do_ar(i):
                    wdt = HW if i == 0 else C
                    col0 = 0 if i == 0 else HW + (i - 1) * C
                    base = (2 * C - HW) if i == 0 else ((1 + i) * C)
                    for ot in range(2):
                        ctp = ppt.tile([C, 512], f32, tag="ct", bufs=2)
                        step, last = 0, KU * 2 - 1
                        for j in range(KU):
                            off = base - j
                            for dh in range(2):
                                nc.tensor.matmul(
                                    ctp[:, :B * wdt],
                                    mut[:, j, dh, ot * C:(ot + 1) * C],
                                    uT[:, dh, :, off:off + wdt],
                                    start=(step == 0), stop=(step == last))
                                step += 1
                        dst = y_st[:, ot, :, col0:col0 + wdt]
                        srcv = ctp[:, :B * wdt].rearrange("p (b c) -> p b c", b=B)
                        if i == 0:
                            nc.vector.tensor_add(out=dst, in0=dst, in1=srcv)
                        else:
                            nc.vector.tensor_copy(out=dst, in_=srcv)

                def do_conv(g, i, first):
                    nonlocal cast_rr
                    nbg = _GNB[g]
                    goff = _GOFF[g]
                    if True:
                        wdt = HW if i == 0 else C
                        gsw = GS * wdt
                        col0 = 0 if i == 0 else HW + (i - 1) * C
                        up = pb.tile([C, 2, 2, GS, B, C], bf16, tag="up", bufs=2)
                        if i == 0:
                            # s-packed halo conv: moving = [plus | modulated-minus]
                            for b in range(B):
                                cps = [ppc.tile([C, 512], f32, tag=f"cv0{dh}",
                                                name=f"cv0{dh}", bufs=1)
                                       for dh in range(2)]
                                for m in range(nbg):
                                    blk = PRE - 1 - m
                                    for dh in range(2):
                                        nc.tensor.matmul(
                                            cps[dh][:, :2 * gsw],
                                            u_all[:, blk, b, dh * C:(dh + 1) * C],
                                            bht[:, goff + m],
                                            start=(m == 0), stop=(m == nbg - 1))
                                for dh in range(2):
                                    srcv = cps[dh][:, :2 * gsw].rearrange(
                                        "p (s k c) -> p s k c", s=2, k=GS)
                                    nc.scalar.activation(
                                        out=up[:, 0, dh, :, b, :wdt],
                                        in_=srcv[:, 0], func=AF.Copy)
                                    nc.vector.tensor_mul(
                                        out=up[:, 1, dh, :, b, :wdt],
                                        in0=srcv[:, 1],
                                        in1=alth[:].rearrange("p (k c) -> p k c", k=GS))
                        else:
                            for b in range(B):
                                cps = [[ppc.tile([C, 512], f32, tag=f"cv{s}{dh}",
                                                 name=f"cv{s}{dh}", bufs=1)
                                        for dh in range(2)] for s in range(2)]
                                for m in range(nbg):
                                    blk = PRE - 1 + i - m
                                    mov = bbt[:, goff + m, :, :]
                                    for s in range(2):
                                        src = u_all if s == 0 else v_all
                                        for dh in range(2):
                                            nc.tensor.matmul(
                                                cps[s][dh][:, :gsw],
                                                src[:, blk, b, dh * C:(dh + 1) * C],
                                                mov,
                                                start=(m == 0), stop=(m == nbg - 1))
                                for s in range(2):
                                    for dh in range(2):
                                        dst = up[:, s, dh, :, b, :wdt]
                                        srcv = cps[s][dh][:, :gsw].rearrange(
                                            "p (k c) -> p k c", k=GS)
                                        if cast_rr % 2 == 0:
                                            nc.vector.tensor_copy(out=dst, in_=srcv)
                                        else:
                                            nc.scalar.activation(out=dst, in_=srcv,
                                                                 func=AF.Copy)
                                        cast_rr += 1
                        for ot in range(2):
                            ctp = ppt.tile([C, 512], f32, tag="ct", bufs=2)
                            step, last = 0, GS * 2 * 2 - 1
                            for kl in range(GS):
                                for s in range(2):
                                    for dh in range(2):
                                        nc.tensor.matmul(
                                            ctp[:, :B * wdt],
                                            mmt[:, g, kl * 2 + s, dh, ot * C:(ot + 1) * C],
                                            up[:, s, dh, kl, :, :wdt],
                                            start=(step == 0), stop=(step == last))
                                        step += 1
                            dst = y_st[:, ot, :, col0:col0 + wdt]
                            srcv = ctp[:, :B * wdt].rearrange("p (b c) -> p b c", b=B)
                            if first:
                                nc.vector.tensor_copy(out=dst, in_=srcv)
                            else:
                                nc.vector.tensor_add(out=dst, in0=dst, in1=srcv)

                do_conv(0, 0, True)
                do_uT()
                for i in range(NIB):
                    do_ar(i)
                for g in range(NG):
                    for i in range(NIB):
                        if g == 0 and i == 0:
                            continue
                        do_conv(g, i, False)

            # ---------------- phase C: AR-scan as tap conv ----------------
            with tc.tile_pool(name="pd", bufs=1) as pd:
                for ch in range(2):
                    for ot in range(2):
                        tg = ("cv00", "cv01")[(2 * ch + ot) % 2]
                        yps = ppc.tile([C, 512], f32, tag=tg, bufs=1)
                        step, last = 0, J * 2 - 1
                        for j in range(J):
                            for dh in range(2):
                                rhs = y_st[:, dh, 2 * ch:2 * ch + 2, HW - j:HW - j + TB]
                                nc.tensor.matmul(
                                    yps[:], tpt[:, j, dh, ot * C:(ot + 1) * C], rhs,
                                    start=(step == 0), stop=(step == last))
                                step += 1
                        nc.scalar.activation(
                            out=h_st[:, ot, 2 * ch:2 * ch + 2, :],
                            in_=yps[:].rearrange("p (b c) -> p b c", b=2),
                            func=AF.Copy)

                # ---------------- phase D: SwiGLU MLP + residuals ----------------
                g_st = pd.tile([C, 8, 2, 512], bf16)
                for ch in range(2):
                    for hs in range(4):
                        for mtl in range(2):
                            apx = ppc.tile([C, 512], f32, tag="cv01", bufs=1)
                            gpx = ppc.tile([C, 512], f32, tag="cv10", bufs=1)
                            hcol = hs * 256 + mtl * C
                            for dh in range(2):
                                nc.tensor.matmul(
                                    apx[:], w1t[:, dh, hcol:hcol + C],
                                    h_st[:, dh, 2 * ch:2 * ch + 2, :],
                                    start=(dh == 0), stop=(dh == 1))
                            for dh in range(2):
                                nc.tensor.matmul(
                                    gpx[:], vvt[:, dh, hcol:hcol + C],
                                    h_st[:, dh, 2 * ch:2 * ch + 2, :],
                                    start=(dh == 0), stop=(dh == 1))
                            sil = pd.tile([C, 512], f32, tag="sil", bufs=2)
                            nc.scalar.activation(out=sil[:], in_=apx[:], func=AF.Sigmoid)
                            nc.vector.tensor_mul(out=sil[:], in0=sil[:], in1=apx[:])
                            nc.vector.tensor_mul(
                                out=g_st[:, hs * 2 + mtl, ch, :],
                                in0=sil[:], in1=gpx[:])

                    tmps = []
                    for ot in range(2):
                        ops = ppt.tile([C, 512], f32, tag="ct", bufs=2)
                        for hh in range(8):
                            nc.tensor.matmul(ops[:], w2t[:, hh, ot * C:(ot + 1) * C],
                                             g_st[:, hh, ch, :],
                                             start=(hh == 0), stop=(hh == 7))
                        tmp = pd.tile([C, 512], bf16, tag=f"tmp{ot}", bufs=1)
                        nc.vector.tensor_add(
                            out=tmp[:], in0=ops[:],
                            in1=h_st[:, ot, 2 * ch:2 * ch + 2, :])
                        tmps.append(tmp)
                    for bb_ in range(2):
                        b = 2 * ch + bb_
                        for tt in range(2):
                            osb = pd.tile([C, D], f32, tag="osb", bufs=3)
                            for ot in range(2):
                                tps = ppm.tile([C, C], bf16, tag="tr", bufs=2)
                                nc.tensor.transpose(
                                    tps[:],
                                    tmps[ot][:, bb_ * 256 + tt * C:bb_ * 256 + (tt + 1) * C],
                                    eyeb[:])
                                nc.vector.tensor_add(
                                    out=osb[:, ot * C:(ot + 1) * C], in0=tps[:],
                                    in1=xr[:, tt, b, ot * C:(ot + 1) * C])
                            nc.sync.dma_start(
                                out=out_ap[b, tt * C:(tt + 1) * C, :], in_=osb[:])

    nc.compile()
    return nc


def _to_bf16(x):
    import ml_dtypes
    u = np.ascontiguousarray(np.asarray(x, np.float32)).view(np.uint32)
    r = (u + 0x7FFF + ((u >> 16) & 1)) & 0xFFFF0000
    return np.ascontiguousarray((r >> 16).astype(np.uint16)).view(ml_dtypes.bfloat16)


def _host_prep(inputs):
    x = np.ascontiguousarray(np.asarray(inputs["x"], np.float32))
    sigma = np.asarray(inputs["sigma"], np.float64)
    phi = np.asarray(inputs["phi"], np.float64)
    rms_w = np.asarray(inputs["rms_w"], np.float64)
    M_u = np.asarray(inputs["M_u"], np.float64)
    Mp = np.asarray(inputs["M_phi_plus"], np.float64)
    Mm = np.asarray(inputs["M_phi_minus"], np.float64)
    m_y = np.asarray(inputs["m_y"], np.float64)
    w1 = np.ascontiguousarray(np.asarray(inputs["w1"], np.float32))
    v = np.ascontiguousarray(np.asarray(inputs["v"], np.float32))
    w2 = np.ascontiguousarray(np.asarray(inputs["w2"], np.float32))

    sr = np.clip(sigma, 1e-12, None) ** 0.25
    g_plus = (phi * sr[None, :]).astype(np.float32)

    # Toeplitz filter bank (plus only; minus shares it via v = alt*u),
    # partition-first: bb[tau_p, ((g,m), kl*C + tau)]
    bb = np.zeros((C, NBT, GS * C), np.float32)
    tau = np.arange(C)
    idx = tau[None, :] - tau[:, None]           # tau - tau_p
    for gi, grp in enumerate(_GROUPS):
        for kl, k in enumerate(grp):
            for m in range(min(NB[k] + 1, _GNB[gi])):
                sidx = m * C + idx
                valid = (sidx >= 0) & (sidx < NB[k] * C)
                si = np.clip(sidx, 0, T - 1)
                bb[:, _GOFF[gi] + m, kl * C:(kl + 1) * C] = np.where(
                    valid, g_plus[si, k], 0.0)
    bb = _to_bf16(bb.reshape(C, NBT * GS * C))

    # projection matrices, transposed to (d, o), rms_w folded into d rows;
    # partition-first [p, g, ks, dh, o]
    mm = np.zeros((NG, C, GS * 2, 2, D), np.float64)
    for gi, grp in enumerate(_GROUPS):
        for kl, k in enumerate(grp):
            for dh in range(2):
                wrow = rms_w[dh * C:(dh + 1) * C, None]
                mm[gi, :, kl * 2 + 0, dh, :] = Mp[k].T[dh * C:(dh + 1) * C, :] * wrow
                mm[gi, :, kl * 2 + 1, dh, :] = Mm[k].T[dh * C:(dh + 1) * C, :] * wrow
    mm = _to_bf16(mm.transpose(1, 0, 2, 3, 4).reshape(C, NG * GS * 2 * 2 * D))

    mu = np.zeros((C, KU, 2, D), np.float64)
    for j in range(KU):
        for dh in range(2):
            mu[:, j, dh, :] = M_u[j].T[dh * C:(dh + 1) * C, :] * rms_w[dh * C:(dh + 1) * C, None]
    mu = _to_bf16(mu.reshape(C, KU * 2 * D))

    # scan taps P_j (transposed), fp64 recurrence on host
    A1, A2 = m_y[0], m_y[1]
    P = [np.eye(D), A1.copy()]
    for j in range(2, J):
        P.append(A1 @ P[-1] + A2 @ P[-2])
    tp = np.zeros((C, J, 2, D), np.float64)
    for j in range(J):
        pjt = P[j].T
        tp[:, j, 0, :] = pjt[:C, :]
        tp[:, j, 1, :] = pjt[C:, :]
    tp = _to_bf16(tp.reshape(C, J * 2 * D))
    w1b = _to_bf16(w1.reshape(2, C, H).transpose(1, 0, 2).reshape(C, 2 * H))
    vb = _to_bf16(v.reshape(2, C, H).transpose(1, 0, 2).reshape(C, 2 * H))
    w2b = _to_bf16(w2.reshape(8, C, D).transpose(1, 0, 2).reshape(C, 8 * D))

    al = np.where(np.arange(C) % 2 == 0, 1.0, -1.0).astype(np.float32)
    alh = _to_bf16(np.tile(al[96:], GS))
    ey = np.eye(C, dtype=np.float32)

    # s-packed halo bank: bh[tau_p, (g,m), s, kl, t32] = g_s[m*C + 96 + t32 - tau_p]
    alt_t = np.where(np.arange(T) % 2 == 0, 1.0, -1.0)
    g_minus = (phi * alt_t[:, None] * sr[None, :]).astype(np.float32)
    bh = np.zeros((C, NBT, 2, GS, HW), np.float32)
    th = np.arange(96, C)
    idxh = th[None, :] - tau[:, None]
    for gi, grp in enumerate(_GROUPS):
        for kl, k in enumerate(grp):
            for m in range(min(NB[k] + 1, _GNB[gi])):
                sidx = m * C + idxh
                valid = (sidx >= 0) & (sidx < NB[k] * C)
                si = np.clip(sidx, 0, T - 1)
                bh[:, _GOFF[gi] + m, 0, kl, :] = np.where(valid, g_plus[si, k], 0.0)
                bh[:, _GOFF[gi] + m, 1, kl, :] = np.where(valid, g_minus[si, k], 0.0)
    bh = _to_bf16(bh.reshape(C, NBT * 2 * GS * HW))

    common = dict(bb=bb, mm=mm, mu=mu, tp=tp, w1=w1b, vv=vb, w2=w2b,
                  al=al, alh=alh, bh=bh, ey=ey)
    in_maps = []
    for c in range(NCORES):
        t0 = c * TB - PRE * C
        xwin = np.zeros((B, NXB * C, D), np.float32)
        lo = max(t0, 0)
        hi = min(t0 + NXB * C, T)
        if hi > lo:
            xwin[:, lo - t0:hi - t0, :] = x[:, lo:hi, :]
        m = dict(common)
        m["xb"] = _to_bf16(xwin)
        m["xw"] = np.ascontiguousarray(x[:, c * TB:(c + 1) * TB, :])
        in_maps.append(m)
    return in_maps


def kernel(**inputs):
    from concourse.bass_utils import run_bass_kernel_spmd
    if "nc" not in _BUILT:
        _BUILT["nc"] = _build_program()
    nc = _BUILT["nc"]
    in_maps = _host_prep(inputs)
    res = run_bass_kernel_spmd(nc, in_maps, core_ids=list(range(NCORES)))
    out = np.concatenate([res.results[c]["out"] for c in range(NCORES)], axis=1)
    return np.ascontiguousarray(out.astype(np.float32))


# revision 16
# speedup vs baseline: 1.0193x; 1.0193x over previous
"""Trainium2 Bass kernel for the STU (spectral transform unit) block. v2.

Strategy
--------
Time-shard the sequence across 8 cores (256 output steps each, halos for
causal history). Each core runs an identical SPMD program:

  rmsnorm -> causal filter-bank convolution as block-Toeplitz matmuls
  (per-filter lag truncation, filters sr-weighted) -> (k,d)->o contraction
  + AR-on-inputs taps -> output AR scan as a truncated matrix-tap
  convolution -> SwiGLU MLP -> residuals.

v2: all matmuls in bf16 (enables the compiler's fast-weight-load path;
fp32r disables it), filter truncation capped at 4 lag blocks, scan taps
truncated at J=12, the scan-halo block narrowed from 128 to 32 steps
(its filter columns are a strided slice of the main Toeplitz bank), a
single +-bank shared by conv(u) and conv(alt*u), and rms_w folded into
the contraction weights. PSUM->SBUF casts round-robin across the
vector/scalar/gpsimd engines.
"""

import contextlib
import numpy as np

# ---------------- problem constants (hardcoded shapes) ----------------
B, T, D, K, KU, KY, H = 4, 2048, 256, 24, 3, 2, 1024
NCORES = 8
TB = T // NCORES          # 256 output timesteps per core
C = 128                   # conv / tile block

# per-filter truncation: number of 128-lag blocks kept for each k (0..23)
NB = [1, 1, 1, 1, 1, 1, 1, 1, 1, 2, 3, 3, 3, 4, 4, 4, 4, 3, 2, 2, 2, 1, 1, 1]
J = 12                    # scan taps
GS = 4                    # filters per conv group
HW = 32                   # scan-halo width (needs >= J-1)
PRE = 4                   # history blocks before the core's 2-block window
NXB = PRE + 2             # u/x window blocks per core
NIB = 3                   # output regions per core: halo(32) + 2 full blocks
YW = HW + 2 * C           # y window width (288)

_ORDER = sorted(range(K), key=lambda k: -NB[k])
_GROUPS = [_ORDER[i * GS:(i + 1) * GS] for i in range(K // GS)]
# remainder lag-block only for nb=1 filters (concentrated); long filters'
# parallelogram truncation error matches their tail plateau anyway
_GNB = [max((NB[k] + 1 if NB[k] == 1 else NB[k]) for k in g) for g in _GROUPS]
NG = len(_GROUPS)
NBT = sum(_GNB)           # total (g, m) lag blocks in the bank (15)
_GOFF = [sum(_GNB[:i]) for i in range(NG)]

_BUILT = {}


def _build_program():
    import concourse.bacc as bacc
    import concourse.tile as tile
    import concourse.mybir as mybir

    f32 = mybir.dt.float32
    bf16 = mybir.dt.bfloat16
    AF = mybir.ActivationFunctionType
    ALU = mybir.AluOpType

    nc = bacc.Bacc("TRN2", target_bir_lowering=False, debug=False,
                   num_devices=NCORES)

    # ---------------- DRAM tensors ----------------
    xw_ap = nc.dram_tensor("xw", [B, 2 * C, D], f32, kind="ExternalInput").ap()
    xb_ap = nc.dram_tensor("xb", [B, NXB * C, D], bf16, kind="ExternalInput").ap()
    bb_ap = nc.dram_tensor("bb", [C, NBT * GS * C], bf16, kind="ExternalInput").ap()
    mm_ap = nc.dram_tensor("mm", [C, NG * GS * 2 * 2 * D], bf16, kind="ExternalInput").ap()
    mu_ap = nc.dram_tensor("mu", [C, KU * 2 * D], bf16, kind="ExternalInput").ap()
    tp_ap = nc.dram_tensor("tp", [C, J * 2 * D], bf16, kind="ExternalInput").ap()
    w1_ap = nc.dram_tensor("w1", [C, 2 * H], bf16, kind="ExternalInput").ap()
    vv_ap = nc.dram_tensor("vv", [C, 2 * H], bf16, kind="ExternalInput").ap()
    w2_ap = nc.dram_tensor("w2", [C, 8 * D], bf16, kind="ExternalInput").ap()
    al_ap = nc.dram_tensor("al", [C], f32, kind="ExternalInput").ap()
    alh_ap = nc.dram_tensor("alh", [GS * HW], bf16, kind="ExternalInput").ap()
    bh_ap = nc.dram_tensor("bh", [C, NBT * 2 * GS * HW], bf16, kind="ExternalInput").ap()
    ey_ap = nc.dram_tensor("ey", [C, C], f32, kind="ExternalInput").ap()
    out_ap = nc.dram_tensor("out", [B, TB, D], f32, kind="ExternalOutput").ap()

    import concourse.bass as bass

    with tile.TileContext(nc) as tc:
        ctx = contextlib.ExitStack()
        with ctx:
            p0 = ctx.enter_context(tc.tile_pool(name="p0", bufs=1))
            pc = ctx.enter_context(tc.tile_pool(name="pc", bufs=1))
            small = ctx.enter_context(tc.tile_pool(name="small", bufs=4))
            ppc = ctx.enter_context(tc.tile_pool(name="ppc", bufs=1, space="PSUM"))
            ppt = ctx.enter_context(tc.tile_pool(name="ppt", bufs=2, space="PSUM"))
            ppm = ctx.enter_context(tc.tile_pool(name="ppm", bufs=1, space="PSUM"))

            # ---------------- input window first (phase A gates everything) ----
            xta = p0.tile([C, NXB, B, D], bf16)
            for blk in range(NXB):
                for b in range(B):
                    nc.sync.dma_start(out=xta[:, blk, b, :],
                                      in_=xb_ap[b, blk * C:(blk + 1) * C, :])

            # ---------------- constants ----------------
            eye = p0.tile([C, C], f32)
            nc.sync.dma_start(out=eye[:], in_=ey_ap)
            eyeb = p0.tile([C, C], bf16)
            nc.vector.tensor_copy(out=eyeb[:], in_=eye[:])
            altc = p0.tile([C, 1], f32)
            nc.sync.dma_start(
                out=altc[:],
                in_=bass.AP(tensor=al_ap.tensor, offset=al_ap.offset,
                            ap=[[1, C], [0, 1]]))
            epst = p0.tile([C, 1], f32)
            nc.vector.memset(epst[:], 1e-6)
            ones = p0.tile([C, D], f32)
            nc.vector.memset(ones[:], 1.0)
            altB = p0.tile([C, D], bf16)
            nc.scalar.activation(out=altB[:], in_=ones[:], func=AF.Copy,
                                 scale=altc[:])
            alth = p0.tile([C, GS * HW], bf16)
            nc.sync.dma_start(
                out=alth[:],
                in_=bass.AP(tensor=alh_ap.tensor, offset=alh_ap.offset,
                            ap=[[0, C], [1, GS * HW]]))

            # ---------------- weights (DMA'd in order of first use) ----------------
            bbt = p0.tile([C, NBT, GS, C], bf16)
            bht = p0.tile([C, NBT, 2, GS, HW], bf16)
            mmt = p0.tile([C, NG, GS * 2, 2, D], bf16)
            mut = p0.tile([C, KU, 2, D], bf16)
            tpt = p0.tile([C, J, 2, D], bf16)

            def dma_bb(g):
                nc.gpsimd.dma_start(
                    out=bbt[:, _GOFF[g]:_GOFF[g] + _GNB[g], :, :].rearrange(
                        "p a b c -> p (a b c)"),
                    in_=bb_ap[:, _GOFF[g] * GS * C:(_GOFF[g] + _GNB[g]) * GS * C])

            def dma_mm(g):
                nc.gpsimd.dma_start(
                    out=mmt[:, g].rearrange("p a b c -> p (a b c)"),
                    in_=mm_ap[:, g * GS * 2 * 2 * D:(g + 1) * GS * 2 * 2 * D])

            nc.gpsimd.dma_start(
                out=bht[:].rearrange("p a b c d -> p (a b c d)"), in_=bh_ap)
            dma_bb(0)
            nc.gpsimd.dma_start(out=mut[:].rearrange("p a b c -> p (a b c)"), in_=mu_ap)
            dma_mm(0)
            for g in range(1, NG):
                dma_bb(g)
                dma_mm(g)
            nc.gpsimd.dma_start(out=tpt[:].rearrange("p a b c -> p (a b c)"), in_=tp_ap)
            w1t = p0.tile([C, 2, H], bf16)
            nc.gpsimd.dma_start(out=w1t[:].rearrange("p a b -> p (a b)"), in_=w1_ap)
            vvt = p0.tile([C, 2, H], bf16)
            nc.gpsimd.dma_start(out=vvt[:].rearrange("p a b -> p (a b)"), in_=vv_ap)
            w2t = p0.tile([C, 8, D], bf16)
            nc.gpsimd.dma_start(out=w2t[:].rearrange("p a b -> p (a b)"), in_=w2_ap)

            # persistent activation stores
            y_st = pc.tile([C, 2, B, YW], bf16)   # spectral+ar accum (o x (b,t))
            h_st = pc.tile([C, 2, B, TB], bf16)
            xr = pc.tile([C, 2, B, D], f32)
            for w in range(2):
                for b in range(B):
                    nc.gpsimd.dma_start(out=xr[:, w, b, :],
                                        in_=xw_ap[b, w * C:(w + 1) * C, :])

            with tc.tile_pool(name="pa", bufs=1) as pa, \
                 tc.tile_pool(name="pb", bufs=1) as pb:
                # ---------------- phase A: rmsnorm (+ alt copy) ----------------
                u_all = pa.tile([C, NXB, B, D], bf16)
                v_all = pa.tile([C, NXB, B, D], bf16)
                for blk in range(NXB):
                    for b in range(B):
                        xt = xta[:, blk, b, :]
                        sq = pb.tile([C, D], bf16, tag="sq", bufs=2)
                        ssum = small.tile([C, 1], f32, tag="ssum")
                        nc.scalar.activation(out=sq[:], in_=xt, func=AF.Square,
                                             accum_out=ssum[:])
                        nc.scalar.activation(out=ssum[:], in_=ssum[:], func=AF.Sqrt,
                                             bias=epst[:], scale=1.0 / D)
                        nc.vector.reciprocal(out=ssum[:], in_=ssum[:])
                        nc.scalar.activation(out=u_all[:, blk, b, :], in_=xt,
                                             func=AF.Copy, scale=ssum[:])
                        nc.vector.tensor_mul(out=v_all[:, blk, b, :],
                                             in0=u_all[:, blk, b, :], in1=altB[:])

                # u^T for AR-on-inputs taps: blocks PRE-2 .. PRE+1
                uT = pa.tile([C, 2, B, 4 * C], bf16)

                def do_uT():
                    cnt = 0
                    for w in range(4):
                        blk = PRE - 2 + w
                        for b in range(B):
                            for dh in range(2):
                                tps = ppm.tile([C, C], bf16, tag="tr", bufs=2)
                                nc.tensor.transpose(
                                    tps[:], u_all[:, blk, b, dh * C:(dh + 1) * C], eyeb[:])
                                if cnt % 2 == 0:
                                    nc.scalar.activation(
                                        out=uT[:, dh, b, w * C:(w + 1) * C], in_=tps[:],
                                        func=AF.Copy)
                                else:
                                    nc.vector.tensor_copy(
                                        out=uT[:, dh, b, w * C:(w + 1) * C], in_=tps[:])
                                cnt += 1

                # ---------------- phase B: conv + contraction ----------------
                # order: conv(g0,i0) fills the PE while phase A drains, then
                # uT/AR, then the remaining conv blocks.
                cast_rr = 0

                def do_ar(i):
                    wdt = HW if i == 0 else C
                    col0 = 0 if i == 0 else HW + (i - 1) * C
                    base = (2 * C - HW) if i == 0 else ((1 + i) * C)
                    for ot in range(2):
                        ctp = ppt.tile([C, 512], f32, tag="ct", bufs=2)
                        step, last = 0, KU * 2 - 1
                        for j in range(KU):
                            off = base - j
                            for dh in range(2):
                                nc.tensor.matmul(
                                    ctp[:, :B * wdt],
                                    mut[:, j, dh, ot * C:(ot + 1) * C],
                                    uT[:, dh, :, off:off + wdt],
                                    start=(step == 0), stop=(step == last))
                                step += 1
                        dst = y_st[:, ot, :, col0:col0 + wdt]
                        srcv = ctp[:, :B * wdt].rearrange("p (b c) -> p b c", b=B)
                        if i == 0:
                            nc.vector.tensor_add(out=dst, in0=dst, in1=srcv)
                        else:
                            nc.vector.tensor_copy(out=dst, in_=srcv)

                def do_conv(g, i, first):
                    nonlocal cast_rr
                    nbg = _GNB[g]
                    goff = _GOFF[g]
                    if True:
                        wdt = HW if i == 0 else C
                        gsw = GS * wdt
                        col0 = 0 if i == 0 else HW + (i - 1) * C
                        up = pb.tile([C, 2, 2, GS, B, C], bf16, tag="up", bufs=2)
                        if i == 0:
                            # s-packed halo conv: moving = [plus | modulated-minus]
                            for b in range(B):
                                cps = [ppc.tile([C, 512], f32, tag=f"cv0{dh}",
                                                name=f"cv0{dh}", bufs=1)
                                       for dh in range(2)]
                                for m in range(nbg):
                                    blk = PRE - 1 - m
                                    for dh in range(2):
                                        nc.tensor.matmul(
                                            cps[dh][:, :2 * gsw],
                                            u_all[:, blk, b, dh * C:(dh + 1) * C],
                                            bht[:, goff + m],
                                            start=(m == 0), stop=(m == nbg - 1))
                                for dh in range(2):
                                    srcv = cps[dh][:, :2 * gsw].rearrange(
                                        "p (s k c) -> p s k c", s=2, k=GS)
                                    nc.scalar.activation(
                                        out=up[:, 0, dh, :, b, :wdt],
                                        in_=srcv[:, 0], func=AF.Copy)
                                    nc.vector.tensor_mul(
                                        out=up[:, 1, dh, :, b, :wdt],
                                        in0=srcv[:, 1],
                                        in1=alth[:].rearrange("p (k c) -> p k c", k=GS))
                        else:
                            for b in range(B):
                                cps = [[ppc.tile([C, 512], f32, tag=f"cv{s}{dh}",
                                                 name=f"cv{s}{dh}", bufs=1)
                                        for dh in range(2)] for s in range(2)]
                                for m in range(nbg):
                                    blk = PRE - 1 + i - m
                                    mov = bbt[:, goff + m, :, :]
                                    for s in range(2):
                                        src = u_all if s == 0 else v_all
                                        for dh in range(2):
                                            nc.tensor.matmul(
                                                cps[s][dh][:, :gsw],
                                                src[:, blk, b, dh * C:(dh + 1) * C],
                                                mov,
                                                start=(m == 0), stop=(m == nbg - 1))
                                for s in range(2):
                                    for dh in range(2):
                                        dst = up[:, s, dh, :, b, :wdt]
                                        srcv = cps[s][dh][:, :gsw].rearrange(
                                            "p (k c) -> p k c", k=GS)
                                        if cast_rr % 2 == 0:
                                            nc.vector.tensor_copy(out=dst, in_=srcv)
                                        else:
                                            nc.scalar.activation(out=dst, in_=srcv,
                                                                 func=AF.Copy)
                                        cast_rr += 1
                        for ot in range(2):
                            ctp = ppt.tile([C, 512], f32, tag="ct", bufs=2)
                            step, last = 0, GS * 2 * 2 - 1
                            for kl in range(GS):
                                for s in range(2):
                                    for dh in range(2):
                                        nc.tensor.matmul(
                                            ctp[:, :B * wdt],
                                            mmt[:, g, kl * 2 + s, dh, ot * C:(ot + 1) * C],
                                            up[:, s, dh, kl, :, :wdt],
                                            start=(step == 0), stop=(step == last))
                                        step += 1
                            dst = y_st[:, ot, :, col0:col0 + wdt]
                            srcv = ctp[:, :B * wdt].rearrange("p (b c) -> p b c", b=B)
                            if first:
                                nc.vector.tensor_copy(out=dst, in_=srcv)
                            else:
                                nc.vector.tensor_add(out=dst, in0=dst, in1=srcv)

                do_conv(0, 0, True)
                do_uT()
                for i in range(NIB):
                    do_ar(i)
                for g in range(NG):
                    for i in range(NIB):
                        if g == 0 and i == 0:
                            continue
                        do_conv(g, i, False)

            # ---------------- phase C: AR-scan as tap conv ----------------
            with tc.tile_pool(name="pd", bufs=1) as pd:
                for ch in range(2):
                    for ot in range(2):
                        tg = ("cv00", "cv01")[(2 * ch + ot) % 2]
                        yps = ppc.tile([C, 512], f32, tag=tg, bufs=1)
                        step, last = 0, J * 2 - 1
                        for j in range(J):
                            for dh in range(2):
                                rhs = y_st[:, dh, 2 * ch:2 * ch + 2, HW - j:HW - j + TB]
                                nc.tensor.matmul(
                                    yps[:], tpt[:, j, dh, ot * C:(ot + 1) * C], rhs,
                                    start=(step == 0), stop=(step == last))
                                step += 1
                        nc.scalar.activation(
                            out=h_st[:, ot, 2 * ch:2 * ch + 2, :],
                            in_=yps[:].rearrange("p (b c) -> p b c", b=2),
                            func=AF.Copy)

                # ---------------- phase D: SwiGLU MLP + residuals ----------------
                g_st = pd.tile([C, 8, 2, 512], bf16)
                for ch in range(2):
                    for hs in range(4):
                        for mtl in range(2):
                            apx = ppc.tile([C, 512], f32, tag="cv01", bufs=1)
                            gpx = ppc.tile([C, 512], f32, tag="cv10", bufs=1)
                            hcol = hs * 256 + mtl * C
                            for dh in range(2):
                                nc.tensor.matmul(
                                    apx[:], w1t[:, dh, hcol:hcol + C],
                                    h_st[:, dh, 2 * ch:2 * ch + 2, :],
                                    start=(dh == 0), stop=(dh == 1))
                            for dh in range(2):
                                nc.tensor.matmul(
                                    gpx[:], vvt[:, dh, hcol:hcol + C],
                                    h_st[:, dh, 2 * ch:2 * ch + 2, :],
                                    start=(dh == 0), stop=(dh == 1))
                            sil = pd.tile([C, 512], f32, tag="sil", bufs=2)
                            nc.scalar.activation(out=sil[:], in_=apx[:], func=AF.Sigmoid)
                            nc.vector.tensor_mul(out=sil[:], in0=sil[:], in1=apx[:])
                            nc.vector.tensor_mul(
                                out=g_st[:, hs * 2 + mtl, ch, :],
                                in0=sil[:], in1=gpx[:])

                    tmps = []
                    for ot in range(2):
                        ops = ppt.tile([C, 512], f32, tag="ct", bufs=2)
                        for hh in range(8):
                            nc.tensor.matmul(ops[:], w2t[:, hh, ot * C:(ot + 1) * C],
                                             g_st[:, hh, ch, :],
                                             start=(hh == 0), stop=(hh == 7))
                        tmp = pd.tile([C, 512], bf16, tag=f"tmp{ot}", bufs=1)
                        nc.vector.tensor_add(
                            out=tmp[:], in0=ops[:],
                            in1=h_st[:, ot, 2 * ch:2 * ch + 2, :])
                        tmps.append(tmp)
                    for bb_ in range(2):
                        b = 2 * ch + bb_
                        for tt in range(2):
                            osb = pd.tile([C, D], f32, tag="osb", bufs=3)
                            for ot in range(2):
                                tps = ppm.tile([C, C], bf16, tag="tr", bufs=2)
                                nc.tensor.transpose(
                                    tps[:],
                                    tmps[ot][:, bb_ * 256 + tt * C:bb_ * 256 + (tt + 1) * C],
                                    eyeb[:])
                                nc.vector.tensor_add(
                                    out=osb[:, ot * C:(ot + 1) * C], in0=tps[:],
                                    in1=xr[:, tt, b, ot * C:(ot + 1) * C])
                            nc.sync.dma_start(
                                out=out_ap[b, tt * C:(tt + 1) * C, :], in_=osb[:])

    nc.compile()
    return nc


def _to_bf16(x):
    import ml_dtypes
    u = np.ascontiguousarray(np.asarray(x, np.float32)).view(np.uint32)
    r = (u + 0x7FFF + ((u >> 16) & 1)) & 0xFFFF0000
    return np.ascontiguousarray((r >> 16).astype(np.uint16)).view(ml_dtypes.bfloat16)


def _host_prep(inputs):
    x = np.ascontiguousarray(np.asarray(inputs["x"], np.float32))
    sigma = np.asarray(inputs["sigma"], np.float64)
    phi = np.asarray(inputs["phi"], np.float64)
    rms_w = np.asarray(inputs["rms_w"], np.float64)
    M_u = np.asarray(inputs["M_u"], np.float64)
    Mp = np.asarray(inputs["M_phi_plus"], np.float64)
    Mm = np.asarray(inputs["M_phi_minus"], np.float64)
    m_y = np.asarray(inputs["m_y"], np.float64)
    w1 = np.ascontiguousarray(np.asarray(inputs["w1"], np.float32))
    v = np.ascontiguousarray(np.asarray(inputs["v"], np.float32))
    w2 = np.ascontiguousarray(np.asarray(inputs["w2"], np.float32))

    sr = np.clip(sigma, 1e-12, None) ** 0.25
    g_plus = (phi * sr[None, :]).astype(np.float32)

    # Toeplitz filter bank (plus only; minus shares it via v = alt*u),
    # partition-first: bb[tau_p, ((g,m), kl*C + tau)]
    bb = np.zeros((C, NBT, GS * C), np.float32)
    tau = np.arange(C)
    idx = tau[None, :] - tau[:, None]           # tau - tau_p
    for gi, grp in enumerate(_GROUPS):
        for kl, k in enumerate(grp):
            for m in range(min(NB[k] + 1, _GNB[gi])):
                sidx = m * C + idx
                valid = (sidx >= 0) & (sidx < NB[k] * C)
                si = np.clip(sidx, 0, T - 1)
                bb[:, _GOFF[gi] + m, kl * C:(kl + 1) * C] = np.where(
                    valid, g_plus[si, k], 0.0)
    bb = _to_bf16(bb.reshape(C, NBT * GS * C))

    # projection matrices, transposed to (d, o), rms_w folded into d rows;
    # partition-first [p, g, ks, dh, o]
    mm = np.zeros((NG, C, GS * 2, 2, D), np.float64)
    for gi, grp in enumerate(_GROUPS):
        for kl, k in enumerate(grp):
            for dh in range(2):
                wrow = rms_w[dh * C:(dh + 1) * C, None]
                mm[gi, :, kl * 2 + 0, dh, :] = Mp[k].T[dh * C:(dh + 1) * C, :] * wrow
                mm[gi, :, kl * 2 + 1, dh, :] = Mm[k].T[dh * C:(dh + 1) * C, :] * wrow
    mm = _to_bf16(mm.transpose(1, 0, 2, 3, 4).reshape(C, NG * GS * 2 * 2 * D))

    mu = np.zeros((C, KU, 2, D), np.float64)
    for j in range(KU):
        for dh in range(2):
            mu[:, j, dh, :] = M_u[j].T[dh * C:(dh + 1) * C, :] * rms_w[dh * C:(dh + 1) * C, None]
    mu = _to_bf16(mu.reshape(C, KU * 2 * D))

    # scan taps P_j (transposed), fp64 recurrence on host
    A1, A2 = m_y[0], m_y[1]
    P = [np.eye(D), A1.copy()]
    for j in range(2, J):
        P.append(A1 @ P[-1] + A2 @ P[-2])
    tp = np.zeros((C, J, 2, D), np.float64)
    for j in range(J):
        pjt = P[j].T
        tp[:, j, 0, :] = pjt[:C, :]
        tp[:, j, 1, :] = pjt[C:, :]
    tp = _to_bf16(tp.reshape(C, J * 2 * D))
    w1b = _to_bf16(w1.reshape(2, C, H).transpose(1, 0, 2).reshape(C, 2 * H))
    vb = _to_bf16(v.reshape(2, C, H).transpose(1, 0, 2).reshape(C, 2 * H))
    w2b = _to_bf16(w2.reshape(8, C, D).transpose(1, 0, 2).reshape(C, 8 * D))

    al = np.where(np.arange(C) % 2 == 0, 1.0, -1.0).astype(np.float32)
    alh = _to_bf16(np.tile(al[96:], GS))
    ey = np.eye(C, dtype=np.float32)

    # s-packed halo bank: bh[tau_p, (g,m), s, kl, t32] = g_s[m*C + 96 + t32 - tau_p]
    alt_t = np.where(np.arange(T) % 2 == 0, 1.0, -1.0)
    g_minus = (phi * alt_t[:, None] * sr[None, :]).astype(np.float32)
    bh = np.zeros((C, NBT, 2, GS, HW), np.float32)
    th = np.arange(96, C)
    idxh = th[None, :] - tau[:, None]
    for gi, grp in enumerate(_GROUPS):
        for kl, k in enumerate(grp):
            for m in range(min(NB[k] + 1, _GNB[gi])):
                sidx = m * C + idxh
                valid = (sidx >= 0) & (sidx < NB[k] * C)
                si = np.clip(sidx, 0, T - 1)
                bh[:, _GOFF[gi] + m, 0, kl, :] = np.where(valid, g_plus[si, k], 0.0)
                bh[:, _GOFF[gi] + m, 1, kl, :] = np.where(valid, g_minus[si, k], 0.0)
    bh = _to_bf16(bh.reshape(C, NBT * 2 * GS * HW))

    common = dict(bb=bb, mm=mm, mu=mu, tp=tp, w1=w1b, vv=vb, w2=w2b,
                  al=al, alh=alh, bh=bh, ey=ey)
    in_maps = []
    for c in range(NCORES):
        t0 = c * TB - PRE * C
        xwin = np.zeros((B, NXB * C, D), np.float32)
        lo = max(t0, 0)
        hi = min(t0 + NXB * C, T)
        if hi > lo:
            xwin[:, lo - t0:hi - t0, :] = x[:, lo:hi, :]
        m = dict(common)
        m["xb"] = _to_bf16(xwin)
        m["xw"] = np.ascontiguousarray(x[:, c * TB:(c + 1) * TB, :])
        in_maps.append(m)
    return in_maps


def kernel(**inputs):
    from concourse.bass_utils import run_bass_kernel_spmd
    if "nc" not in _BUILT:
        _BUILT["nc"] = _build_program()
    nc = _BUILT["nc"]
    in_maps = _host_prep(inputs)
    res = run_bass_kernel_spmd(nc, in_maps, core_ids=list(range(NCORES)))
    out = np.concatenate([res.results[c]["out"] for c in range(NCORES)], axis=1)
    return np.ascontiguousarray(out.astype(np.float32))


# revision 19
# speedup vs baseline: 1.0841x; 1.0636x over previous
"""Trainium2 Bass kernel for the STU (spectral transform unit) block. v2.

Strategy
--------
Time-shard the sequence across 8 cores (256 output steps each, halos for
causal history). Each core runs an identical SPMD program:

  rmsnorm -> causal filter-bank convolution as block-Toeplitz matmuls
  (per-filter lag truncation, filters sr-weighted) -> (k,d)->o contraction
  + AR-on-inputs taps -> output AR scan as a truncated matrix-tap
  convolution -> SwiGLU MLP -> residuals.

v2: all matmuls in bf16 (enables the compiler's fast-weight-load path;
fp32r disables it), filter truncation capped at 4 lag blocks, scan taps
truncated at J=12, the scan-halo block narrowed from 128 to 32 steps
(its filter columns are a strided slice of the main Toeplitz bank), a
single +-bank shared by conv(u) and conv(alt*u), and rms_w folded into
the contraction weights. PSUM->SBUF casts round-robin across the
vector/scalar/gpsimd engines.
"""

import contextlib
import numpy as np

# ---------------- problem constants (hardcoded shapes) ----------------
B, T, D, K, KU, KY, H = 4, 2048, 256, 24, 3, 2, 1024
NCORES = 8
TB = T // NCORES          # 256 output timesteps per core
C = 128                   # conv / tile block

# per-filter truncation: number of 128-lag blocks kept for each k (0..23)
NB = [1, 1, 1, 1, 1, 1, 1, 1, 1, 2, 3, 3, 3, 4, 4, 4, 4, 3, 2, 2, 2, 1, 1, 1]
J = 12                    # scan taps
GS = 4                    # filters per conv group
HW = 32                   # scan-halo width (needs >= J-1)
PRE = 4                   # history blocks before the core's 2-block window
NXB = PRE + 2             # u/x window blocks per core
NIB = 3                   # output regions per core: halo(32) + 2 full blocks
YW = HW + 2 * C           # y window width (288)

_ORDER = sorted(range(K), key=lambda k: -NB[k])
_GROUPS = [_ORDER[i * GS:(i + 1) * GS] for i in range(K // GS)]
# remainder lag-block only for nb=1 filters (concentrated); long filters'
# parallelogram truncation error matches their tail plateau anyway
_GNB = [max((NB[k] + 1 if NB[k] == 1 else NB[k]) for k in g) for g in _GROUPS]
NG = len(_GROUPS)
NBT = sum(_GNB)           # total (g, m) lag blocks in the bank (15)
_GOFF = [sum(_GNB[:i]) for i in range(NG)]

_BUILT = {}


def _build_program():
    import concourse.bacc as bacc
    import concourse.tile as tile
    import concourse.mybir as mybir

    f32 = mybir.dt.float32
    bf16 = mybir.dt.bfloat16
    AF = mybir.ActivationFunctionType
    ALU = mybir.AluOpType

    nc = bacc.Bacc("TRN2", target_bir_lowering=False, debug=False,
                   num_devices=NCORES)

    # ---------------- DRAM tensors ----------------
    xw_ap = nc.dram_tensor("xw", [B, 2 * C, D], f32, kind="ExternalInput").ap()
    xb_ap = nc.dram_tensor("xb", [B, NXB * C, D], bf16, kind="ExternalInput").ap()
    bb_ap = nc.dram_tensor("bb", [C, NBT * GS * C], bf16, kind="ExternalInput").ap()
    mm_ap = nc.dram_tensor("mm", [C, NG * GS * 2 * 2 * D], bf16, kind="ExternalInput").ap()
    mu_ap = nc.dram_tensor("mu", [C, KU * 2 * D], bf16, kind="ExternalInput").ap()
    tp_ap = nc.dram_tensor("tp", [C, J * 2 * D], bf16, kind="ExternalInput").ap()
    w1_ap = nc.dram_tensor("w1", [C, 2 * H], bf16, kind="ExternalInput").ap()
    vv_ap = nc.dram_tensor("vv", [C, 2 * H], bf16, kind="ExternalInput").ap()
    w2_ap = nc.dram_tensor("w2", [C, 8 * D], bf16, kind="ExternalInput").ap()
    al_ap = nc.dram_tensor("al", [C], f32, kind="ExternalInput").ap()
    alh_ap = nc.dram_tensor("alh", [GS * HW], bf16, kind="ExternalInput").ap()
    bh_ap = nc.dram_tensor("bh", [C, NBT * 2 * GS * HW], bf16, kind="ExternalInput").ap()
    ey_ap = nc.dram_tensor("ey", [C, C], f32, kind="ExternalInput").ap()
    out_ap = nc.dram_tensor("out", [B, TB, D], f32, kind="ExternalOutput").ap()

    import concourse.bass as bass

    with tile.TileContext(nc) as tc:
        ctx = contextlib.ExitStack()
        with ctx:
            p0 = ctx.enter_context(tc.tile_pool(name="p0", bufs=1))
            pc = ctx.enter_context(tc.tile_pool(name="pc", bufs=1))
            small = ctx.enter_context(tc.tile_pool(name="small", bufs=4))
            ppc = ctx.enter_context(tc.tile_pool(name="ppc", bufs=1, space="PSUM"))
            ppt = ctx.enter_context(tc.tile_pool(name="ppt", bufs=2, space="PSUM"))
            ppm = ctx.enter_context(tc.tile_pool(name="ppm", bufs=1, space="PSUM"))

            # ---------------- input window first (phase A gates everything) ----
            xta = p0.tile([C, NXB, B, D], bf16)
            for b in range(B):
                for blk in range(NXB):
                    nc.sync.dma_start(out=xta[:, blk, b, :],
                                      in_=xb_ap[b, blk * C:(blk + 1) * C, :])

            # ---------------- constants ----------------
            eye = p0.tile([C, C], f32)
            nc.sync.dma_start(out=eye[:], in_=ey_ap)
            eyeb = p0.tile([C, C], bf16)
            nc.vector.tensor_copy(out=eyeb[:], in_=eye[:])
            altc = p0.tile([C, 1], f32)
            nc.sync.dma_start(
                out=altc[:],
                in_=bass.AP(tensor=al_ap.tensor, offset=al_ap.offset,
                            ap=[[1, C], [0, 1]]))
            epst = p0.tile([C, 1], f32)
            nc.vector.memset(epst[:], 1e-6)
            ones = p0.tile([C, D], f32)
            nc.vector.memset(ones[:], 1.0)
            altB = p0.tile([C, D], bf16)
            nc.scalar.activation(out=altB[:], in_=ones[:], func=AF.Copy,
                                 scale=altc[:])
            alth = p0.tile([C, GS * HW], bf16)
            nc.sync.dma_start(
                out=alth[:],
                in_=bass.AP(tensor=alh_ap.tensor, offset=alh_ap.offset,
                            ap=[[0, C], [1, GS * HW]]))

            # ---------------- weights (DMA'd in order of first use) ----------------
            bbt = p0.tile([C, NBT, GS, C], bf16)
            bht = p0.tile([C, NBT, 2, GS, HW], bf16)
            mmt = p0.tile([C, NG, GS * 2, 2, D], bf16)
            mut = p0.tile([C, KU, 2, D], bf16)
            tpt = p0.tile([C, J, 2, D], bf16)

            def dma_bb(g):
                nc.gpsimd.dma_start(
                    out=bbt[:, _GOFF[g]:_GOFF[g] + _GNB[g], :, :].rearrange(
                        "p a b c -> p (a b c)"),
                    in_=bb_ap[:, _GOFF[g] * GS * C:(_GOFF[g] + _GNB[g]) * GS * C])

            def dma_mm(g):
                nc.gpsimd.dma_start(
                    out=mmt[:, g].rearrange("p a b c -> p (a b c)"),
                    in_=mm_ap[:, g * GS * 2 * 2 * D:(g + 1) * GS * 2 * 2 * D])

            nc.gpsimd.dma_start(
                out=bht[:].rearrange("p a b c d -> p (a b c d)"), in_=bh_ap)
            dma_bb(0)
            nc.gpsimd.dma_start(out=mut[:].rearrange("p a b c -> p (a b c)"), in_=mu_ap)
            dma_mm(0)
            for g in range(1, NG):
                dma_bb(g)
                dma_mm(g)
            nc.gpsimd.dma_start(out=tpt[:].rearrange("p a b c -> p (a b c)"), in_=tp_ap)
            w1t = p0.tile([C, 2, H], bf16)
            nc.gpsimd.dma_start(out=w1t[:].rearrange("p a b -> p (a b)"), in_=w1_ap)
            vvt = p0.tile([C, 2, H], bf16)
            nc.gpsimd.dma_start(out=vvt[:].rearrange("p a b -> p (a b)"), in_=vv_ap)
            w2t = p0.tile([C, 8, D], bf16)
            nc.gpsimd.dma_start(out=w2t[:].rearrange("p a b -> p (a b)"), in_=w2_ap)

            # persistent activation stores
            y_st = pc.tile([C, 2, B, YW], bf16)   # spectral+ar accum (o x (b,t))
            h_st = pc.tile([C, 2, B, TB], bf16)
            xr = pc.tile([C, 2, B, D], f32)
            for w in range(2):
                for b in range(B):
                    nc.gpsimd.dma_start(out=xr[:, w, b, :],
                                        in_=xw_ap[b, w * C:(w + 1) * C, :])

            with tc.tile_pool(name="pa", bufs=1) as pa, \
                 tc.tile_pool(name="pb", bufs=1) as pb:
                # ---------------- phase A: rmsnorm (+ alt copy) ----------------
                u_all = pa.tile([C, NXB, B, D], bf16)
                v_all = pa.tile([C, NXB, B, D], bf16)
                tix = 0
                for b in range(B):
                    for blk in range(NXB):
                        xt = xta[:, blk, b, :]
                        sq = pb.tile([C, D], bf16, tag="sq", bufs=2)
                        ssum = small.tile([C, 1], f32, tag="ssum")
                        if tix % 2 == 0:
                            nc.scalar.activation(out=sq[:], in_=xt, func=AF.Square,
                                                 accum_out=ssum[:])
                        else:
                            nc.vector.tensor_mul(out=sq[:], in0=xt, in1=xt)
                            nc.vector.tensor_reduce(out=ssum[:], in_=sq[:],
                                                    axis=mybir.AxisListType.X,
                                                    op=ALU.add)
                        tix += 1
                        nc.scalar.activation(out=ssum[:], in_=ssum[:], func=AF.Sqrt,
                                             bias=epst[:], scale=1.0 / D)
                        nc.vector.reciprocal(out=ssum[:], in_=ssum[:])
                        nc.scalar.activation(out=u_all[:, blk, b, :], in_=xt,
                                             func=AF.Copy, scale=ssum[:])
                        nc.vector.tensor_mul(out=v_all[:, blk, b, :],
                                             in0=u_all[:, blk, b, :], in1=altB[:])

                # u^T for AR-on-inputs taps: blocks PRE-2 .. PRE+1
                uT = pa.tile([C, 2, B, 4 * C], bf16)

                def do_uT():
                    cnt = 0
                    for w in range(4):
                        blk = PRE - 2 + w
                        for b in range(B):
                            for dh in range(2):
                                tps = ppm.tile([C, C], bf16, tag="tr", bufs=2)
                                nc.tensor.transpose(
                                    tps[:], u_all[:, blk, b, dh * C:(dh + 1) * C], eyeb[:])
                                if cnt % 2 == 0:
                                    nc.scalar.activation(
                                        out=uT[:, dh, b, w * C:(w + 1) * C], in_=tps[:],
                                        func=AF.Copy)
                                else:
                                    nc.vector.tensor_copy(
                                        out=uT[:, dh, b, w * C:(w + 1) * C], in_=tps[:])
                                cnt += 1

                # ---------------- phase B: conv + contraction ----------------
                # order: conv(g0,i0) fills the PE while phase A drains, then
                # uT/AR, then the remaining conv blocks.
                cast_rr = 0

                def do_ar(i):
                    wdt = HW if i == 0 else C
                    col0 = 0 if i == 0 else HW + (i - 1) * C
                    base = (2 * C - HW) if i == 0 else ((1 + i) * C)
                    for ot in range(2):
                        ctp = ppt.tile([C, 512], f32, tag="ct", bufs=2)
                        step, last = 0, KU * 2 - 1
                        for j in range(KU):
                            off = base - j
                            for dh in range(2):
                                nc.tensor.matmul(
                                    ctp[:, :B * wdt],
                                    mut[:, j, dh, ot * C:(ot + 1) * C],
                                    uT[:, dh, :, off:off + wdt],
                                    start=(step == 0), stop=(step == last))
                                step += 1
                        dst = y_st[:, ot, :, col0:col0 + wdt]
                        srcv = ctp[:, :B * wdt].rearrange("p (b c) -> p b c", b=B)
                        nc.vector.tensor_add(out=dst, in0=dst, in1=srcv)

                def do_conv(g, first):
                    nonlocal cast_rr
                    nbg = _GNB[g]
                    goff = _GOFF[g]
                    ups = [pb.tile([C, 2, 2, GS, B, HW if i == 0 else C], bf16,
                                    tag=f"up{i}", name=f"up{i}", bufs=1)
                           for i in range(NIB)]
                    for b in range(B):
                        for i in range(NIB):
                            wdt = HW if i == 0 else C
                            gsw = GS * wdt
                            up = ups[i]
                            if i == 0:
                                cps = [ppc.tile([C, 512], f32, tag=f"cv0{dh}",
                                                name=f"cv0{dh}", bufs=1)
                                       for dh in range(2)]
                                for m in range(nbg):
                                    blk = PRE - 1 - m
                                    for dh in range(2):
                                        nc.tensor.matmul(
                                            cps[dh][:, :2 * gsw],
                                            u_all[:, blk, b, dh * C:(dh + 1) * C],
                                            bht[:, goff + m],
                                            start=(m == 0), stop=(m == nbg - 1))
                                for dh in range(2):
                                    srcv = cps[dh][:, :2 * gsw].rearrange(
                                        "p (s k c) -> p s k c", s=2, k=GS)
                                    nc.scalar.activation(
                                        out=up[:, 0, dh, :, b, :wdt],
                                        in_=srcv[:, 0], func=AF.Copy)
                                    nc.vector.tensor_mul(
                                        out=up[:, 1, dh, :, b, :wdt],
                                        in0=srcv[:, 1],
                                        in1=alth[:].rearrange("p (k c) -> p k c", k=GS))
                            else:
                                cps = [[ppc.tile([C, 512], f32, tag=f"cv{s}{dh}",
                                                 name=f"cv{s}{dh}", bufs=1)
                                        for dh in range(2)] for s in range(2)]
                                for m in range(nbg):
                                    blk = PRE - 1 + i - m
                                    mov = bbt[:, goff + m, :, :]
                                    for s in range(2):
                                        src = u_all if s == 0 else v_all
                                        for dh in range(2):
                                            nc.tensor.matmul(
                                                cps[s][dh][:, :gsw],
                                                src[:, blk, b, dh * C:(dh + 1) * C],
                                                mov,
                                                start=(m == 0), stop=(m == nbg - 1))
                                for s in range(2):
                                    for dh in range(2):
                                        dst = up[:, s, dh, :, b, :wdt]
                                        srcv = cps[s][dh][:, :gsw].rearrange(
                                            "p (k c) -> p k c", k=GS)
                                        if cast_rr % 2 == 0:
                                            nc.vector.tensor_copy(out=dst, in_=srcv)
                                        else:
                                            nc.scalar.activation(out=dst, in_=srcv,
                                                                 func=AF.Copy)
                                        cast_rr += 1
                    for i in range(NIB):
                        wdt = HW if i == 0 else C
                        col0 = 0 if i == 0 else HW + (i - 1) * C
                        for ot in range(2):
                            ctp = ppt.tile([C, 512], f32, tag="ct", bufs=2)
                            step, last = 0, GS * 2 * 2 - 1
                            for kl in range(GS):
                                for s in range(2):
                                    for dh in range(2):
                                        nc.tensor.matmul(
                                            ctp[:, :B * wdt],
                                            mmt[:, g, kl * 2 + s, dh, ot * C:(ot + 1) * C],
                                            ups[i][:, s, dh, kl, :, :wdt],
                                            start=(step == 0), stop=(step == last))
                                        step += 1
                            dst = y_st[:, ot, :, col0:col0 + wdt]
                            srcv = ctp[:, :B * wdt].rearrange("p (b c) -> p b c", b=B)
                            if first:
                                nc.vector.tensor_copy(out=dst, in_=srcv)
                            else:
                                nc.vector.tensor_add(out=dst, in0=dst, in1=srcv)

                do_conv(0, True)
                do_uT()
                for i in range(NIB):
                    do_ar(i)
                for g in range(1, NG):
                    do_conv(g, False)

            # ---------------- phase C: AR-scan as tap conv ----------------
            with tc.tile_pool(name="pd", bufs=1) as pd:
                for ch in range(2):
                    for ot in range(2):
                        tg = ("cv00", "cv01")[(2 * ch + ot) % 2]
                        yps = ppc.tile([C, 512], f32, tag=tg, bufs=1)
                        step, last = 0, J * 2 - 1
                        for j in range(J):
                            for dh in range(2):
                                rhs = y_st[:, dh, 2 * ch:2 * ch + 2, HW - j:HW - j + TB]
                                nc.tensor.matmul(
                                    yps[:], tpt[:, j, dh, ot * C:(ot + 1) * C], rhs,
                                    start=(step == 0), stop=(step == last))
                                step += 1
                        nc.scalar.activation(
                            out=h_st[:, ot, 2 * ch:2 * ch + 2, :],
                            in_=yps[:].rearrange("p (b c) -> p b c", b=2),
                            func=AF.Copy)

                # ---------------- phase D: SwiGLU MLP + residuals ----------------
                g_st = pd.tile([C, 8, 2, 512], bf16)
                for ch in range(2):
                    for hs in range(4):
                        for mtl in range(2):
                            apx = ppc.tile([C, 512], f32, tag="cv01", bufs=1)
                            gpx = ppc.tile([C, 512], f32, tag="cv10", bufs=1)
                            hcol = hs * 256 + mtl * C
                            for dh in range(2):
                                nc.tensor.matmul(
                                    apx[:], w1t[:, dh, hcol:hcol + C],
                                    h_st[:, dh, 2 * ch:2 * ch + 2, :],
                                    start=(dh == 0), stop=(dh == 1))
                            for dh in range(2):
                                nc.tensor.matmul(
                                    gpx[:], vvt[:, dh, hcol:hcol + C],
                                    h_st[:, dh, 2 * ch:2 * ch + 2, :],
                                    start=(dh == 0), stop=(dh == 1))
                            sil = pd.tile([C, 512], f32, tag="sil", bufs=2)
                            nc.scalar.activation(out=sil[:], in_=apx[:], func=AF.Sigmoid)
                            nc.vector.tensor_mul(out=sil[:], in0=sil[:], in1=apx[:])
                            nc.vector.tensor_mul(
                                out=g_st[:, hs * 2 + mtl, ch, :],
                                in0=sil[:], in1=gpx[:])

                    tmps = []
                    for ot in range(2):
                        ops = ppt.tile([C, 512], f32, tag="ct", bufs=2)
                        for hh in range(8):
                            nc.tensor.matmul(ops[:], w2t[:, hh, ot * C:(ot + 1) * C],
                                             g_st[:, hh, ch, :],
                                             start=(hh == 0), stop=(hh == 7))
                        tmp = pd.tile([C, 512], bf16, tag=f"tmp{ot}", bufs=1)
                        nc.vector.tensor_add(
                            out=tmp[:], in0=ops[:],
                            in1=h_st[:, ot, 2 * ch:2 * ch + 2, :])
                        tmps.append(tmp)
                    for bb_ in range(2):
                        b = 2 * ch + bb_
                        for tt in range(2):
                            osb = pd.tile([C, D], f32, tag="osb", bufs=3)
                            for ot in range(2):
                                tps = ppm.tile([C, C], bf16, tag="tr", bufs=2)
                                nc.tensor.transpose(
                                    tps[:],
                                    tmps[ot][:, bb_ * 256 + tt * C:bb_ * 256 + (tt + 1) * C],
                                    eyeb[:])
                                nc.vector.tensor_add(
                                    out=osb[:, ot * C:(ot + 1) * C], in0=tps[:],
                                    in1=xr[:, tt, b, ot * C:(ot + 1) * C])
                            nc.sync.dma_start(
                                out=out_ap[b, tt * C:(tt + 1) * C, :], in_=osb[:])

    nc.compile()
    return nc


def _to_bf16(x):
    import ml_dtypes
    u = np.ascontiguousarray(np.asarray(x, np.float32)).view(np.uint32)
    r = (u + 0x7FFF + ((u >> 16) & 1)) & 0xFFFF0000
    return np.ascontiguousarray((r >> 16).astype(np.uint16)).view(ml_dtypes.bfloat16)


def _host_prep(inputs):
    x = np.ascontiguousarray(np.asarray(inputs["x"], np.float32))
    sigma = np.asarray(inputs["sigma"], np.float64)
    phi = np.asarray(inputs["phi"], np.float64)
    rms_w = np.asarray(inputs["rms_w"], np.float64)
    M_u = np.asarray(inputs["M_u"], np.float64)
    Mp = np.asarray(inputs["M_phi_plus"], np.float64)
    Mm = np.asarray(inputs["M_phi_minus"], np.float64)
    m_y = np.asarray(inputs["m_y"], np.float64)
    w1 = np.ascontiguousarray(np.asarray(inputs["w1"], np.float32))
    v = np.ascontiguousarray(np.asarray(inputs["v"], np.float32))
    w2 = np.ascontiguousarray(np.asarray(inputs["w2"], np.float32))

    sr = np.clip(sigma, 1e-12, None) ** 0.25
    g_plus = (phi * sr[None, :]).astype(np.float32)

    # Toeplitz filter bank (plus only; minus shares it via v = alt*u),
    # partition-first: bb[tau_p, ((g,m), kl*C + tau)]
    bb = np.zeros((C, NBT, GS * C), np.float32)
    tau = np.arange(C)
    idx = tau[None, :] - tau[:, None]           # tau - tau_p
    for gi, grp in enumerate(_GROUPS):
        for kl, k in enumerate(grp):
            for m in range(min(NB[k] + 1, _GNB[gi])):
                sidx = m * C + idx
                valid = (sidx >= 0) & (sidx < NB[k] * C)
                si = np.clip(sidx, 0, T - 1)
                bb[:, _GOFF[gi] + m, kl * C:(kl + 1) * C] = np.where(
                    valid, g_plus[si, k], 0.0)
    bb = _to_bf16(bb.reshape(C, NBT * GS * C))

    # projection matrices, transposed to (d, o), rms_w folded into d rows;
    # partition-first [p, g, ks, dh, o]
    mm = np.zeros((NG, C, GS * 2, 2, D), np.float64)
    for gi, grp in enumerate(_GROUPS):
        for kl, k in enumerate(grp):
            for dh in range(2):
                wrow = rms_w[dh * C:(dh + 1) * C, None]
                mm[gi, :, kl * 2 + 0, dh, :] = Mp[k].T[dh * C:(dh + 1) * C, :] * wrow
                mm[gi, :, kl * 2 + 1, dh, :] = Mm[k].T[dh * C:(dh + 1) * C, :] * wrow
    mm = _to_bf16(mm.transpose(1, 0, 2, 3, 4).reshape(C, NG * GS * 2 * 2 * D))

    mu = np.zeros((C, KU, 2, D), np.float64)
    for j in range(KU):
        for dh in range(2):
            mu[:, j, dh, :] = M_u[j].T[dh * C:(dh + 1) * C, :] * rms_w[dh * C:(dh + 1) * C, None]
    mu = _to_bf16(mu.reshape(C, KU * 2 * D))

    # scan taps P_j (transposed), fp64 recurrence on host
    A1, A2 = m_y[0], m_y[1]
    P = [np.eye(D), A1.copy()]
    for j in range(2, J):
        P.append(A1 @ P[-1] + A2 @ P[-2])
    tp = np.zeros((C, J, 2, D), np.float64)
    for j in range(J):
        pjt = P[j].T
        tp[:, j, 0, :] = pjt[:C, :]
        tp[:, j, 1, :] = pjt[C:, :]
    tp = _to_bf16(tp.reshape(C, J * 2 * D))
    w1b = _to_bf16(w1.reshape(2, C, H).transpose(1, 0, 2).reshape(C, 2 * H))
    vb = _to_bf16(v.reshape(2, C, H).transpose(1, 0, 2).reshape(C, 2 * H))
    w2b = _to_bf16(w2.reshape(8, C, D).transpose(1, 0, 2).reshape(C, 8 * D))

    al = np.where(np.arange(C) % 2 == 0, 1.0, -1.0).astype(np.float32)
    alh = _to_bf16(np.tile(al[96:], GS))
    ey = np.eye(C, dtype=np.float32)

    # s-packed halo bank: bh[tau_p, (g,m), s, kl, t32] = g_s[m*C + 96 + t32 - tau_p]
    alt_t = np.where(np.arange(T) % 2 == 0, 1.0, -1.0)
    g_minus = (phi * alt_t[:, None] * sr[None, :]).astype(np.float32)
    bh = np.zeros((C, NBT, 2, GS, HW), np.float32)
    th = np.arange(96, C)
    idxh = th[None, :] - tau[:, None]
    for gi, grp in enumerate(_GROUPS):
        for kl, k in enumerate(grp):
            for m in range(min(NB[k] + 1, _GNB[gi])):
                sidx = m * C + idxh
                valid = (sidx >= 0) & (sidx < NB[k] * C)
                si = np.clip(sidx, 0, T - 1)
                bh[:, _GOFF[gi] + m, 0, kl, :] = np.where(valid, g_plus[si, k], 0.0)
                bh[:, _GOFF[gi] + m, 1, kl, :] = np.where(valid, g_minus[si, k], 0.0)
    bh = _to_bf16(bh.reshape(C, NBT * 2 * GS * HW))

    common = dict(bb=bb, mm=mm, mu=mu, tp=tp, w1=w1b, vv=vb, w2=w2b,
                  al=al, alh=alh, bh=bh, ey=ey)
    in_maps = []
    for c in range(NCORES):
        t0 = c * TB - PRE * C
        xwin = np.zeros((B, NXB * C, D), np.float32)
        lo = max(t0, 0)
        hi = min(t0 + NXB * C, T)
        if hi > lo:
            xwin[:, lo - t0:hi - t0, :] = x[:, lo:hi, :]
        m = dict(common)
        m["xb"] = _to_bf16(xwin)
        m["xw"] = np.ascontiguousarray(x[:, c * TB:(c + 1) * TB, :])
        in_maps.append(m)
    return in_maps


def kernel(**inputs):
    from concourse.bass_utils import run_bass_kernel_spmd
    if "nc" not in _BUILT:
        _BUILT["nc"] = _build_program()
    nc = _BUILT["nc"]
    in_maps = _host_prep(inputs)
    res = run_bass_kernel_spmd(nc, in_maps, core_ids=list(range(NCORES)))
    out = np.concatenate([res.results[c]["out"] for c in range(NCORES)], axis=1)
    return np.ascontiguousarray(out.astype(np.float32))


# revision 20
# speedup vs baseline: 1.1598x; 1.0698x over previous
"""Trainium2 Bass kernel for the STU (spectral transform unit) block. v2.

Strategy
--------
Time-shard the sequence across 8 cores (256 output steps each, halos for
causal history). Each core runs an identical SPMD program:

  rmsnorm -> causal filter-bank convolution as block-Toeplitz matmuls
  (per-filter lag truncation, filters sr-weighted) -> (k,d)->o contraction
  + AR-on-inputs taps -> output AR scan as a truncated matrix-tap
  convolution -> SwiGLU MLP -> residuals.

v2: all matmuls in bf16 (enables the compiler's fast-weight-load path;
fp32r disables it), filter truncation capped at 4 lag blocks, scan taps
truncated at J=12, the scan-halo block narrowed from 128 to 32 steps
(its filter columns are a strided slice of the main Toeplitz bank), a
single +-bank shared by conv(u) and conv(alt*u), and rms_w folded into
the contraction weights. PSUM->SBUF casts round-robin across the
vector/scalar/gpsimd engines.
"""

import contextlib
import numpy as np

# ---------------- problem constants (hardcoded shapes) ----------------
B, T, D, K, KU, KY, H = 4, 2048, 256, 24, 3, 2, 1024
NCORES = 8
TB = T // NCORES          # 256 output timesteps per core
C = 128                   # conv / tile block

# per-filter truncation: number of 128-lag blocks kept for each k (0..23)
NB = [1, 1, 1, 1, 1, 1, 1, 1, 1, 2, 3, 3, 3, 4, 4, 4, 4, 3, 2, 2, 2, 1, 1, 1]
J = 12                    # scan taps
GS = 4                    # filters per conv group
HW = 32                   # scan-halo width (needs >= J-1)
PRE = 4                   # history blocks before the core's 2-block window
NXB = PRE + 2             # u/x window blocks per core
NIB = 3                   # output regions per core: halo(32) + 2 full blocks
YW = HW + 2 * C           # y window width (288)

_ORDER = sorted(range(K), key=lambda k: -NB[k])
_GROUPS = [_ORDER[i * GS:(i + 1) * GS] for i in range(K // GS)]
# remainder lag-block only for nb=1 filters (concentrated); long filters'
# parallelogram truncation error matches their tail plateau anyway
_GNB = [max((NB[k] + 1 if NB[k] == 1 else NB[k]) for k in g) for g in _GROUPS]
NG = len(_GROUPS)
NBT = sum(_GNB)           # total (g, m) lag blocks in the bank (15)
_GOFF = [sum(_GNB[:i]) for i in range(NG)]

_BUILT = {}


def _build_program():
    import concourse.bacc as bacc
    import concourse.tile as tile
    import concourse.mybir as mybir

    f32 = mybir.dt.float32
    bf16 = mybir.dt.bfloat16
    AF = mybir.ActivationFunctionType
    ALU = mybir.AluOpType

    nc = bacc.Bacc("TRN2", target_bir_lowering=False, debug=False,
                   num_devices=NCORES)

    # ---------------- DRAM tensors ----------------
    xw_ap = nc.dram_tensor("xw", [B, 2 * C, D], f32, kind="ExternalInput").ap()
    xb_ap = nc.dram_tensor("xb", [B, NXB * C, D], bf16, kind="ExternalInput").ap()
    bb_ap = nc.dram_tensor("bb", [C, NBT * GS * C], bf16, kind="ExternalInput").ap()
    mm_ap = nc.dram_tensor("mm", [C, NG * GS * 2 * 2 * D], bf16, kind="ExternalInput").ap()
    mu_ap = nc.dram_tensor("mu", [C, KU * 2 * D], bf16, kind="ExternalInput").ap()
    tp_ap = nc.dram_tensor("tp", [C, J * 2 * D], bf16, kind="ExternalInput").ap()
    w1_ap = nc.dram_tensor("w1", [C, 2 * H], bf16, kind="ExternalInput").ap()
    vv_ap = nc.dram_tensor("vv", [C, 2 * H], bf16, kind="ExternalInput").ap()
    w2_ap = nc.dram_tensor("w2", [C, 8 * D], bf16, kind="ExternalInput").ap()
    al_ap = nc.dram_tensor("al", [C], f32, kind="ExternalInput").ap()
    alh_ap = nc.dram_tensor("alh", [GS * HW], bf16, kind="ExternalInput").ap()
    bh_ap = nc.dram_tensor("bh", [C, NBT * 2 * GS * HW], bf16, kind="ExternalInput").ap()
    ey_ap = nc.dram_tensor("ey", [C, C], f32, kind="ExternalInput").ap()
    out_ap = nc.dram_tensor("out", [B, TB, D], f32, kind="ExternalOutput").ap()

    import concourse.bass as bass

    with tile.TileContext(nc) as tc:
        ctx = contextlib.ExitStack()
        with ctx:
            p0 = ctx.enter_context(tc.tile_pool(name="p0", bufs=1))
            pc = ctx.enter_context(tc.tile_pool(name="pc", bufs=1))
            small = ctx.enter_context(tc.tile_pool(name="small", bufs=4))
            ppc = ctx.enter_context(tc.tile_pool(name="ppc", bufs=1, space="PSUM"))
            ppt = ctx.enter_context(tc.tile_pool(name="ppt", bufs=2, space="PSUM"))
            ppm = ctx.enter_context(tc.tile_pool(name="ppm", bufs=1, space="PSUM"))

            # ---------------- input window first (phase A gates everything) ----
            xta = p0.tile([C, NXB, B, D], bf16)
            for b in range(B):
                for blk in range(NXB):
                    nc.sync.dma_start(out=xta[:, blk, b, :],
                                      in_=xb_ap[b, blk * C:(blk + 1) * C, :])

            # ---------------- constants ----------------
            eye = p0.tile([C, C], f32)
            nc.sync.dma_start(out=eye[:], in_=ey_ap)
            eyeb = p0.tile([C, C], bf16)
            nc.vector.tensor_copy(out=eyeb[:], in_=eye[:])
            altc = p0.tile([C, 1], f32)
            nc.sync.dma_start(
                out=altc[:],
                in_=bass.AP(tensor=al_ap.tensor, offset=al_ap.offset,
                            ap=[[1, C], [0, 1]]))
            epst = p0.tile([C, 1], f32)
            nc.vector.memset(epst[:], 1e-6)
            ones = p0.tile([C, D], f32)
            nc.vector.memset(ones[:], 1.0)
            altB = p0.tile([C, D], bf16)
            nc.scalar.activation(out=altB[:], in_=ones[:], func=AF.Copy,
                                 scale=altc[:])
            alth = p0.tile([C, GS * HW], bf16)
            nc.sync.dma_start(
                out=alth[:],
                in_=bass.AP(tensor=alh_ap.tensor, offset=alh_ap.offset,
                            ap=[[0, C], [1, GS * HW]]))

            # ---------------- weights (DMA'd in order of first use) ----------------
            bbt = p0.tile([C, NBT, GS, C], bf16)
            bht = p0.tile([C, NBT, 2, GS, HW], bf16)
            mmt = p0.tile([C, NG, GS * 2, 2, D], bf16)
            mut = p0.tile([C, KU, 2, D], bf16)
            tpt = p0.tile([C, J, 2, D], bf16)

            def dma_bb(g):
                nc.sync.dma_start(
                    out=bbt[:, _GOFF[g]:_GOFF[g] + _GNB[g], :, :].rearrange(
                        "p a b c -> p (a b c)"),
                    in_=bb_ap[:, _GOFF[g] * GS * C:(_GOFF[g] + _GNB[g]) * GS * C])

            def dma_mm(g):
                nc.sync.dma_start(
                    out=mmt[:, g].rearrange("p a b c -> p (a b c)"),
                    in_=mm_ap[:, g * GS * 2 * 2 * D:(g + 1) * GS * 2 * 2 * D])

            nc.sync.dma_start(
                out=bht[:].rearrange("p a b c d -> p (a b c d)"), in_=bh_ap)
            dma_bb(0)
            nc.sync.dma_start(out=mut[:].rearrange("p a b c -> p (a b c)"), in_=mu_ap)
            dma_mm(0)
            for g in range(1, NG):
                dma_bb(g)
                dma_mm(g)
            nc.sync.dma_start(out=tpt[:].rearrange("p a b c -> p (a b c)"), in_=tp_ap)
            w1t = p0.tile([C, 2, H], bf16)
            nc.sync.dma_start(out=w1t[:].rearrange("p a b -> p (a b)"), in_=w1_ap)
            vvt = p0.tile([C, 2, H], bf16)
            nc.sync.dma_start(out=vvt[:].rearrange("p a b -> p (a b)"), in_=vv_ap)
            w2t = p0.tile([C, 8, D], bf16)
            nc.sync.dma_start(out=w2t[:].rearrange("p a b -> p (a b)"), in_=w2_ap)

            # persistent activation stores
            y_st = pc.tile([C, 2, B, YW], bf16)   # spectral+ar accum (o x (b,t))
            h_st = pc.tile([C, 2, B, TB], bf16)
            xr = pc.tile([C, 2, B, D], f32)
            for w in range(2):
                for b in range(B):
                    nc.gpsimd.dma_start(out=xr[:, w, b, :],
                                        in_=xw_ap[b, w * C:(w + 1) * C, :])

            with tc.tile_pool(name="pa", bufs=1) as pa, \
                 tc.tile_pool(name="pb", bufs=1) as pb:
                # ---------------- phase A: rmsnorm (+ alt copy) ----------------
                u_all = pa.tile([C, NXB, B, D], bf16)
                v_all = pa.tile([C, NXB, B, D], bf16)
                tix = 0
                for b in range(B):
                    for blk in range(NXB):
                        xt = xta[:, blk, b, :]
                        sq = pb.tile([C, D], bf16, tag="sq", bufs=2)
                        ssum = small.tile([C, 1], f32, tag="ssum")
                        if tix % 2 == 0:
                            nc.scalar.activation(out=sq[:], in_=xt, func=AF.Square,
                                                 accum_out=ssum[:])
                        else:
                            nc.vector.tensor_mul(out=sq[:], in0=xt, in1=xt)
                            nc.vector.tensor_reduce(out=ssum[:], in_=sq[:],
                                                    axis=mybir.AxisListType.X,
                                                    op=ALU.add)
                        tix += 1
                        nc.scalar.activation(out=ssum[:], in_=ssum[:], func=AF.Sqrt,
                                             bias=epst[:], scale=1.0 / D)
                        nc.vector.reciprocal(out=ssum[:], in_=ssum[:])
                        nc.scalar.activation(out=u_all[:, blk, b, :], in_=xt,
                                             func=AF.Copy, scale=ssum[:])
                        nc.vector.tensor_mul(out=v_all[:, blk, b, :],
                                             in0=u_all[:, blk, b, :], in1=altB[:])

                # u^T for AR-on-inputs taps: blocks PRE-2 .. PRE+1
                uT = pa.tile([C, 2, B, 4 * C], bf16)

                def do_uT():
                    cnt = 0
                    for w in range(4):
                        blk = PRE - 2 + w
                        for b in range(B):
                            for dh in range(2):
                                tps = ppm.tile([C, C], bf16, tag="tr", bufs=2)
                                nc.tensor.transpose(
                                    tps[:], u_all[:, blk, b, dh * C:(dh + 1) * C], eyeb[:])
                                if cnt % 2 == 0:
                                    nc.scalar.activation(
                                        out=uT[:, dh, b, w * C:(w + 1) * C], in_=tps[:],
                                        func=AF.Copy)
                                else:
                                    nc.vector.tensor_copy(
                                        out=uT[:, dh, b, w * C:(w + 1) * C], in_=tps[:])
                                cnt += 1

                # ---------------- phase B: conv + contraction ----------------
                # order: conv(g0,i0) fills the PE while phase A drains, then
                # uT/AR, then the remaining conv blocks.
                cast_rr = 0

                def do_ar(i):
                    wdt = HW if i == 0 else C
                    col0 = 0 if i == 0 else HW + (i - 1) * C
                    base = (2 * C - HW) if i == 0 else ((1 + i) * C)
                    for ot in range(2):
                        ctp = ppt.tile([C, 512], f32, tag="ct", bufs=2)
                        step, last = 0, KU * 2 - 1
                        for j in range(KU):
                            off = base - j
                            for dh in range(2):
                                nc.tensor.matmul(
                                    ctp[:, :B * wdt],
                                    mut[:, j, dh, ot * C:(ot + 1) * C],
                                    uT[:, dh, :, off:off + wdt],
                                    start=(step == 0), stop=(step == last))
                                step += 1
                        dst = y_st[:, ot, :, col0:col0 + wdt]
                        srcv = ctp[:, :B * wdt].rearrange("p (b c) -> p b c", b=B)
                        nc.vector.tensor_add(out=dst, in0=dst, in1=srcv)

                def do_conv(g, first):
                    nonlocal cast_rr
                    nbg = _GNB[g]
                    goff = _GOFF[g]
                    ups = [pb.tile([C, 2, 2, GS, B, HW if i == 0 else C], bf16,
                                    tag=f"up{i}", name=f"up{i}", bufs=1)
                           for i in range(NIB)]
                    for b in range(B):
                        for i in range(NIB):
                            wdt = HW if i == 0 else C
                            gsw = GS * wdt
                            up = ups[i]
                            if i == 0:
                                cps = [ppc.tile([C, 512], f32, tag=f"cv0{dh}",
                                                name=f"cv0{dh}", bufs=1)
                                       for dh in range(2)]
                                for m in range(nbg):
                                    blk = PRE - 1 - m
                                    for dh in range(2):
                                        nc.tensor.matmul(
                                            cps[dh][:, :2 * gsw],
                                            u_all[:, blk, b, dh * C:(dh + 1) * C],
                                            bht[:, goff + m],
                                            start=(m == 0), stop=(m == nbg - 1))
                                for dh in range(2):
                                    srcv = cps[dh][:, :2 * gsw].rearrange(
                                        "p (s k c) -> p s k c", s=2, k=GS)
                                    nc.scalar.activation(
                                        out=up[:, 0, dh, :, b, :wdt],
                                        in_=srcv[:, 0], func=AF.Copy)
                                    nc.vector.tensor_mul(
                                        out=up[:, 1, dh, :, b, :wdt],
                                        in0=srcv[:, 1],
                                        in1=alth[:].rearrange("p (k c) -> p k c", k=GS))
                            else:
                                cps = [[ppc.tile([C, 512], f32, tag=f"cv{s}{dh}",
                                                 name=f"cv{s}{dh}", bufs=1)
                                        for dh in range(2)] for s in range(2)]
                                for m in range(nbg):
                                    blk = PRE - 1 + i - m
                                    mov = bbt[:, goff + m, :, :]
                                    for s in range(2):
                                        src = u_all if s == 0 else v_all
                                        for dh in range(2):
                                            nc.tensor.matmul(
                                                cps[s][dh][:, :gsw],
                                                src[:, blk, b, dh * C:(dh + 1) * C],
                                                mov,
                                                start=(m == 0), stop=(m == nbg - 1))
                                for s in range(2):
                                    for dh in range(2):
                                        dst = up[:, s, dh, :, b, :wdt]
                                        srcv = cps[s][dh][:, :gsw].rearrange(
                                            "p (k c) -> p k c", k=GS)
                                        if cast_rr % 2 == 0:
                                            nc.vector.tensor_copy(out=dst, in_=srcv)
                                        else:
                                            nc.scalar.activation(out=dst, in_=srcv,
                                                                 func=AF.Copy)
                                        cast_rr += 1
                    for i in range(NIB):
                        wdt = HW if i == 0 else C
                        col0 = 0 if i == 0 else HW + (i - 1) * C
                        for ot in range(2):
                            ctp = ppt.tile([C, 512], f32, tag="ct", bufs=2)
                            step, last = 0, GS * 2 * 2 - 1
                            for kl in range(GS):
                                for s in range(2):
                                    for dh in range(2):
                                        nc.tensor.matmul(
                                            ctp[:, :B * wdt],
                                            mmt[:, g, kl * 2 + s, dh, ot * C:(ot + 1) * C],
                                            ups[i][:, s, dh, kl, :, :wdt],
                                            start=(step == 0), stop=(step == last))
                                        step += 1
                            dst = y_st[:, ot, :, col0:col0 + wdt]
                            srcv = ctp[:, :B * wdt].rearrange("p (b c) -> p b c", b=B)
                            if first:
                                nc.vector.tensor_copy(out=dst, in_=srcv)
                            else:
                                nc.vector.tensor_add(out=dst, in0=dst, in1=srcv)

                do_conv(0, True)
                do_uT()
                for i in range(NIB):
                    do_ar(i)
                for g in range(1, NG):
                    do_conv(g, False)

            # ---------------- phase C: AR-scan as tap conv ----------------
            with tc.tile_pool(name="pd", bufs=1) as pd:
                for ch in range(2):
                    for ot in range(2):
                        tg = ("cv00", "cv01")[(2 * ch + ot) % 2]
                        yps = ppc.tile([C, 512], f32, tag=tg, bufs=1)
                        step, last = 0, J * 2 - 1
                        for j in range(J):
                            for dh in range(2):
                                rhs = y_st[:, dh, 2 * ch:2 * ch + 2, HW - j:HW - j + TB]
                                nc.tensor.matmul(
                                    yps[:], tpt[:, j, dh, ot * C:(ot + 1) * C], rhs,
                                    start=(step == 0), stop=(step == last))
                                step += 1
                        nc.scalar.activation(
                            out=h_st[:, ot, 2 * ch:2 * ch + 2, :],
                            in_=yps[:].rearrange("p (b c) -> p b c", b=2),
                            func=AF.Copy)

                # ---------------- phase D: SwiGLU MLP + residuals ----------------
                g_st = pd.tile([C, 8, 2, 512], bf16)
                for ch in range(2):
                    for hs in range(4):
                        for mtl in range(2):
                            apx = ppc.tile([C, 512], f32, tag="cv01", bufs=1)
                            gpx = ppc.tile([C, 512], f32, tag="cv10", bufs=1)
                            hcol = hs * 256 + mtl * C
                            for dh in range(2):
                                nc.tensor.matmul(
                                    apx[:], w1t[:, dh, hcol:hcol + C],
                                    h_st[:, dh, 2 * ch:2 * ch + 2, :],
                                    start=(dh == 0), stop=(dh == 1))
                            for dh in range(2):
                                nc.tensor.matmul(
                                    gpx[:], vvt[:, dh, hcol:hcol + C],
                                    h_st[:, dh, 2 * ch:2 * ch + 2, :],
                                    start=(dh == 0), stop=(dh == 1))
                            sil = pd.tile([C, 512], f32, tag="sil", bufs=2)
                            nc.scalar.activation(out=sil[:], in_=apx[:], func=AF.Sigmoid)
                            nc.vector.tensor_mul(out=sil[:], in0=sil[:], in1=apx[:])
                            nc.vector.tensor_mul(
                                out=g_st[:, hs * 2 + mtl, ch, :],
                                in0=sil[:], in1=gpx[:])

                    tmps = []
                    for ot in range(2):
                        ops = ppt.tile([C, 512], f32, tag="ct", bufs=2)
                        for hh in range(8):
                            nc.tensor.matmul(ops[:], w2t[:, hh, ot * C:(ot + 1) * C],
                                             g_st[:, hh, ch, :],
                                             start=(hh == 0), stop=(hh == 7))
                        tmp = pd.tile([C, 512], bf16, tag=f"tmp{ot}", bufs=1)
                        nc.vector.tensor_add(
                            out=tmp[:], in0=ops[:],
                            in1=h_st[:, ot, 2 * ch:2 * ch + 2, :])
                        tmps.append(tmp)
                    for bb_ in range(2):
                        b = 2 * ch + bb_
                        for tt in range(2):
                            osb = pd.tile([C, D], f32, tag="osb", bufs=3)
                            for ot in range(2):
                                tps = ppm.tile([C, C], bf16, tag="tr", bufs=2)
                                nc.tensor.transpose(
                                    tps[:],
                                    tmps[ot][:, bb_ * 256 + tt * C:bb_ * 256 + (tt + 1) * C],
                                    eyeb[:])
                                nc.vector.tensor_add(
                                    out=osb[:, ot * C:(ot + 1) * C], in0=tps[:],
                                    in1=xr[:, tt, b, ot * C:(ot + 1) * C])
                            nc.sync.dma_start(
                                out=out_ap[b, tt * C:(tt + 1) * C, :], in_=osb[:])

    nc.compile()
    return nc


def _to_bf16(x):
    import ml_dtypes
    u = np.ascontiguousarray(np.asarray(x, np.float32)).view(np.uint32)
    r = (u + 0x7FFF + ((u >> 16) & 1)) & 0xFFFF0000
    return np.ascontiguousarray((r >> 16).astype(np.uint16)).view(ml_dtypes.bfloat16)


def _host_prep(inputs):
    x = np.ascontiguousarray(np.asarray(inputs["x"], np.float32))
    sigma = np.asarray(inputs["sigma"], np.float64)
    phi = np.asarray(inputs["phi"], np.float64)
    rms_w = np.asarray(inputs["rms_w"], np.float64)
    M_u = np.asarray(inputs["M_u"], np.float64)
    Mp = np.asarray(inputs["M_phi_plus"], np.float64)
    Mm = np.asarray(inputs["M_phi_minus"], np.float64)
    m_y = np.asarray(inputs["m_y"], np.float64)
    w1 = np.ascontiguousarray(np.asarray(inputs["w1"], np.float32))
    v = np.ascontiguousarray(np.asarray(inputs["v"], np.float32))
    w2 = np.ascontiguousarray(np.asarray(inputs["w2"], np.float32))

    sr = np.clip(sigma, 1e-12, None) ** 0.25
    g_plus = (phi * sr[None, :]).astype(np.float32)

    # Toeplitz filter bank (plus only; minus shares it via v = alt*u),
    # partition-first: bb[tau_p, ((g,m), kl*C + tau)]
    bb = np.zeros((C, NBT, GS * C), np.float32)
    tau = np.arange(C)
    idx = tau[None, :] - tau[:, None]           # tau - tau_p
    for gi, grp in enumerate(_GROUPS):
        for kl, k in enumerate(grp):
            for m in range(min(NB[k] + 1, _GNB[gi])):
                sidx = m * C + idx
                valid = (sidx >= 0) & (sidx < NB[k] * C)
                si = np.clip(sidx, 0, T - 1)
                bb[:, _GOFF[gi] + m, kl * C:(kl + 1) * C] = np.where(
                    valid, g_plus[si, k], 0.0)
    bb = _to_bf16(bb.reshape(C, NBT * GS * C))

    # projection matrices, transposed to (d, o), rms_w folded into d rows;
    # partition-first [p, g, ks, dh, o]
    mm = np.zeros((NG, C, GS * 2, 2, D), np.float64)
    for gi, grp in enumerate(_GROUPS):
        for kl, k in enumerate(grp):
            for dh in range(2):
                wrow = rms_w[dh * C:(dh + 1) * C, None]
                mm[gi, :, kl * 2 + 0, dh, :] = Mp[k].T[dh * C:(dh + 1) * C, :] * wrow
                mm[gi, :, kl * 2 + 1, dh, :] = Mm[k].T[dh * C:(dh + 1) * C, :] * wrow
    mm = _to_bf16(mm.transpose(1, 0, 2, 3, 4).reshape(C, NG * GS * 2 * 2 * D))

    mu = np.zeros((C, KU, 2, D), np.float64)
    for j in range(KU):
        for dh in range(2):
            mu[:, j, dh, :] = M_u[j].T[dh * C:(dh + 1) * C, :] * rms_w[dh * C:(dh + 1) * C, None]
    mu = _to_bf16(mu.reshape(C, KU * 2 * D))

    # scan taps P_j (transposed), fp64 recurrence on host
    A1, A2 = m_y[0], m_y[1]
    P = [np.eye(D), A1.copy()]
    for j in range(2, J):
        P.append(A1 @ P[-1] + A2 @ P[-2])
    tp = np.zeros((C, J, 2, D), np.float64)
    for j in range(J):
        pjt = P[j].T
        tp[:, j, 0, :] = pjt[:C, :]
        tp[:, j, 1, :] = pjt[C:, :]
    tp = _to_bf16(tp.reshape(C, J * 2 * D))
    w1b = _to_bf16(w1.reshape(2, C, H).transpose(1, 0, 2).reshape(C, 2 * H))
    vb = _to_bf16(v.reshape(2, C, H).transpose(1, 0, 2).reshape(C, 2 * H))
    w2b = _to_bf16(w2.reshape(8, C, D).transpose(1, 0, 2).reshape(C, 8 * D))

    al = np.where(np.arange(C) % 2 == 0, 1.0, -1.0).astype(np.float32)
    alh = _to_bf16(np.tile(al[96:], GS))
    ey = np.eye(C, dtype=np.float32)

    # s-packed halo bank: bh[tau_p, (g,m), s, kl, t32] = g_s[m*C + 96 + t32 - tau_p]
    alt_t = np.where(np.arange(T) % 2 == 0, 1.0, -1.0)
    g_minus = (phi * alt_t[:, None] * sr[None, :]).astype(np.float32)
    bh = np.zeros((C, NBT, 2, GS, HW), np.float32)
    th = np.arange(96, C)
    idxh = th[None, :] - tau[:, None]
    for gi, grp in enumerate(_GROUPS):
        for kl, k in enumerate(grp):
            for m in range(min(NB[k] + 1, _GNB[gi])):
                sidx = m * C + idxh
                valid = (sidx >= 0) & (sidx < NB[k] * C)
                si = np.clip(sidx, 0, T - 1)
                bh[:, _GOFF[gi] + m, 0, kl, :] = np.where(valid, g_plus[si, k], 0.0)
                bh[:, _GOFF[gi] + m, 1, kl, :] = np.where(valid, g_minus[si, k], 0.0)
    bh = _to_bf16(bh.reshape(C, NBT * 2 * GS * HW))

    common = dict(bb=bb, mm=mm, mu=mu, tp=tp, w1=w1b, vv=vb, w2=w2b,
                  al=al, alh=alh, bh=bh, ey=ey)
    in_maps = []
    for c in range(NCORES):
        t0 = c * TB - PRE * C
        xwin = np.zeros((B, NXB * C, D), np.float32)
        lo = max(t0, 0)
        hi = min(t0 + NXB * C, T)
        if hi > lo:
            xwin[:, lo - t0:hi - t0, :] = x[:, lo:hi, :]
        m = dict(common)
        m["xb"] = _to_bf16(xwin)
        m["xw"] = np.ascontiguousarray(x[:, c * TB:(c + 1) * TB, :])
        in_maps.append(m)
    return in_maps


def kernel(**inputs):
    from concourse.bass_utils import run_bass_kernel_spmd
    if "nc" not in _BUILT:
        _BUILT["nc"] = _build_program()
    nc = _BUILT["nc"]
    in_maps = _host_prep(inputs)
    res = run_bass_kernel_spmd(nc, in_maps, core_ids=list(range(NCORES)))
    out = np.concatenate([res.results[c]["out"] for c in range(NCORES)], axis=1)
    return np.ascontiguousarray(out.astype(np.float32))


# revision 21
# speedup vs baseline: 1.2048x; 1.0388x over previous
"""Trainium2 Bass kernel for the STU (spectral transform unit) block. v2.

Strategy
--------
Time-shard the sequence across 8 cores (256 output steps each, halos for
causal history). Each core runs an identical SPMD program:

  rmsnorm -> causal filter-bank convolution as block-Toeplitz matmuls
  (per-filter lag truncation, filters sr-weighted) -> (k,d)->o contraction
  + AR-on-inputs taps -> output AR scan as a truncated matrix-tap
  convolution -> SwiGLU MLP -> residuals.

v2: all matmuls in bf16 (enables the compiler's fast-weight-load path;
fp32r disables it), filter truncation capped at 4 lag blocks, scan taps
truncated at J=12, the scan-halo block narrowed from 128 to 32 steps
(its filter columns are a strided slice of the main Toeplitz bank), a
single +-bank shared by conv(u) and conv(alt*u), and rms_w folded into
the contraction weights. PSUM->SBUF casts round-robin across the
vector/scalar/gpsimd engines.
"""

import contextlib
import numpy as np

# ---------------- problem constants (hardcoded shapes) ----------------
B, T, D, K, KU, KY, H = 4, 2048, 256, 24, 3, 2, 1024
NCORES = 8
TB = T // NCORES          # 256 output timesteps per core
C = 128                   # conv / tile block

# per-filter truncation: number of 128-lag blocks kept for each k (0..23)
NB = [1, 1, 1, 1, 1, 1, 1, 1, 1, 2, 3, 3, 3, 4, 4, 4, 4, 3, 2, 2, 2, 1, 1, 1]
J = 12                    # scan taps
GS = 4                    # filters per conv group
HW = 32                   # scan-halo width (needs >= J-1)
PRE = 4                   # history blocks before the core's 2-block window
NXB = PRE + 2             # u/x window blocks per core
NIB = 3                   # output regions per core: halo(32) + 2 full blocks
YW = HW + 2 * C           # y window width (288)

_ORDER = sorted(range(K), key=lambda k: -NB[k])
_GROUPS = [_ORDER[i * GS:(i + 1) * GS] for i in range(K // GS)]
# remainder lag-block only for nb=1 filters (concentrated); long filters'
# parallelogram truncation error matches their tail plateau anyway
_GNB = [max((NB[k] + 1 if NB[k] == 1 else NB[k]) for k in g) for g in _GROUPS]
NG = len(_GROUPS)
NBT = sum(_GNB)           # total (g, m) lag blocks in the bank (15)
_GOFF = [sum(_GNB[:i]) for i in range(NG)]

_BUILT = {}


def _build_program():
    import concourse.bacc as bacc
    import concourse.tile as tile
    import concourse.mybir as mybir

    f32 = mybir.dt.float32
    bf16 = mybir.dt.bfloat16
    AF = mybir.ActivationFunctionType
    ALU = mybir.AluOpType

    nc = bacc.Bacc("TRN2", target_bir_lowering=False, debug=False,
                   num_devices=NCORES)

    # ---------------- DRAM tensors ----------------
    xw_ap = nc.dram_tensor("xw", [B, 2 * C, D], f32, kind="ExternalInput").ap()
    xb_ap = nc.dram_tensor("xb", [B, NXB * C, D], bf16, kind="ExternalInput").ap()
    bb_ap = nc.dram_tensor("bb", [C, NBT * GS * C], bf16, kind="ExternalInput").ap()
    mm_ap = nc.dram_tensor("mm", [C, NG * GS * 2 * 2 * D], bf16, kind="ExternalInput").ap()
    mu_ap = nc.dram_tensor("mu", [C, KU * 2 * D], bf16, kind="ExternalInput").ap()
    tp_ap = nc.dram_tensor("tp", [C, J * 2 * D], bf16, kind="ExternalInput").ap()
    w1_ap = nc.dram_tensor("w1", [C, 2 * H], bf16, kind="ExternalInput").ap()
    vv_ap = nc.dram_tensor("vv", [C, 2 * H], bf16, kind="ExternalInput").ap()
    w2_ap = nc.dram_tensor("w2", [C, 8 * D], bf16, kind="ExternalInput").ap()
    al_ap = nc.dram_tensor("al", [C], f32, kind="ExternalInput").ap()
    alh_ap = nc.dram_tensor("alh", [GS * HW], bf16, kind="ExternalInput").ap()
    bh_ap = nc.dram_tensor("bh", [C, NBT * 2 * GS * HW], bf16, kind="ExternalInput").ap()
    ey_ap = nc.dram_tensor("ey", [C, C], f32, kind="ExternalInput").ap()
    out_ap = nc.dram_tensor("out", [B, TB, D], f32, kind="ExternalOutput").ap()

    import concourse.bass as bass

    with tile.TileContext(nc) as tc:
        ctx = contextlib.ExitStack()
        with ctx:
            p0 = ctx.enter_context(tc.tile_pool(name="p0", bufs=1))
            pc = ctx.enter_context(tc.tile_pool(name="pc", bufs=1))
            small = ctx.enter_context(tc.tile_pool(name="small", bufs=4))
            ppc = ctx.enter_context(tc.tile_pool(name="ppc", bufs=1, space="PSUM"))
            ppt = ctx.enter_context(tc.tile_pool(name="ppt", bufs=2, space="PSUM"))
            ppm = ctx.enter_context(tc.tile_pool(name="ppm", bufs=1, space="PSUM"))

            # ---------------- input window first (phase A gates everything) ----
            xta = p0.tile([C, NXB, B, D], bf16)

            def dma_xta(b):
                for blk in range(NXB):
                    nc.sync.dma_start(out=xta[:, blk, b, :],
                                      in_=xb_ap[b, blk * C:(blk + 1) * C, :])
            dma_xta(0)

            # ---------------- constants ----------------
            eye = p0.tile([C, C], f32)
            nc.sync.dma_start(out=eye[:], in_=ey_ap)
            eyeb = p0.tile([C, C], bf16)
            nc.vector.tensor_copy(out=eyeb[:], in_=eye[:])
            altc = p0.tile([C, 1], f32)
            nc.sync.dma_start(
                out=altc[:],
                in_=bass.AP(tensor=al_ap.tensor, offset=al_ap.offset,
                            ap=[[1, C], [0, 1]]))
            epst = p0.tile([C, 1], f32)
            nc.vector.memset(epst[:], 1e-6)
            ones = p0.tile([C, D], f32)
            nc.vector.memset(ones[:], 1.0)
            altB = p0.tile([C, D], bf16)
            nc.scalar.activation(out=altB[:], in_=ones[:], func=AF.Copy,
                                 scale=altc[:])
            alth = p0.tile([C, GS * HW], bf16)
            nc.sync.dma_start(
                out=alth[:],
                in_=bass.AP(tensor=alh_ap.tensor, offset=alh_ap.offset,
                            ap=[[0, C], [1, GS * HW]]))

            # ---------------- weights (DMA'd in order of first use) ----------------
            bbt = p0.tile([C, NBT, GS, C], bf16)
            bht = p0.tile([C, NBT, 2, GS, HW], bf16)
            mmt = p0.tile([C, NG, GS * 2, 2, D], bf16)
            mut = p0.tile([C, KU, 2, D], bf16)
            tpt = p0.tile([C, J, 2, D], bf16)

            def dma_bb(g):
                nc.sync.dma_start(
                    out=bbt[:, _GOFF[g]:_GOFF[g] + _GNB[g], :, :].rearrange(
                        "p a b c -> p (a b c)"),
                    in_=bb_ap[:, _GOFF[g] * GS * C:(_GOFF[g] + _GNB[g]) * GS * C])

            def dma_mm(g):
                nc.sync.dma_start(
                    out=mmt[:, g].rearrange("p a b c -> p (a b c)"),
                    in_=mm_ap[:, g * GS * 2 * 2 * D:(g + 1) * GS * 2 * 2 * D])

            nc.sync.dma_start(
                out=bht[:].rearrange("p a b c d -> p (a b c d)"), in_=bh_ap)
            dma_bb(0)
            for b in range(1, B):
                dma_xta(b)
            nc.sync.dma_start(out=mut[:].rearrange("p a b c -> p (a b c)"), in_=mu_ap)
            dma_mm(0)
            for g in range(1, NG):
                dma_bb(g)
                dma_mm(g)
            nc.sync.dma_start(out=tpt[:].rearrange("p a b c -> p (a b c)"), in_=tp_ap)
            w1t = p0.tile([C, 2, H], bf16)
            nc.sync.dma_start(out=w1t[:].rearrange("p a b -> p (a b)"), in_=w1_ap)
            vvt = p0.tile([C, 2, H], bf16)
            nc.sync.dma_start(out=vvt[:].rearrange("p a b -> p (a b)"), in_=vv_ap)
            w2t = p0.tile([C, 8, D], bf16)
            nc.sync.dma_start(out=w2t[:].rearrange("p a b -> p (a b)"), in_=w2_ap)

            # persistent activation stores
            y_st = pc.tile([C, 2, B, YW], bf16)   # spectral+ar accum (o x (b,t))
            h_st = pc.tile([C, 2, B, TB], bf16)
            xr = pc.tile([C, 2, B, D], f32)
            for w in range(2):
                for b in range(B):
                    nc.gpsimd.dma_start(out=xr[:, w, b, :],
                                        in_=xw_ap[b, w * C:(w + 1) * C, :])

            with tc.tile_pool(name="pa", bufs=1) as pa, \
                 tc.tile_pool(name="pb", bufs=1) as pb:
                # ---------------- phase A: rmsnorm (+ alt copy) ----------------
                u_all = pa.tile([C, NXB, B, D], bf16)
                v_all = pa.tile([C, NXB, B, D], bf16)
                tix = 0
                for b in range(B):
                    for blk in range(NXB):
                        xt = xta[:, blk, b, :]
                        sq = pb.tile([C, D], bf16, tag="sq", bufs=2)
                        ssum = small.tile([C, 1], f32, tag="ssum")
                        if tix % 2 == 0:
                            nc.scalar.activation(out=sq[:], in_=xt, func=AF.Square,
                                                 accum_out=ssum[:])
                        else:
                            nc.vector.tensor_mul(out=sq[:], in0=xt, in1=xt)
                            nc.vector.tensor_reduce(out=ssum[:], in_=sq[:],
                                                    axis=mybir.AxisListType.X,
                                                    op=ALU.add)
                        tix += 1
                        nc.scalar.activation(out=ssum[:], in_=ssum[:], func=AF.Sqrt,
                                             bias=epst[:], scale=1.0 / D)
                        nc.vector.reciprocal(out=ssum[:], in_=ssum[:])
                        nc.scalar.activation(out=u_all[:, blk, b, :], in_=xt,
                                             func=AF.Copy, scale=ssum[:])
                        nc.vector.tensor_mul(out=v_all[:, blk, b, :],
                                             in0=u_all[:, blk, b, :], in1=altB[:])

                # u^T for AR-on-inputs taps: blocks PRE-2 .. PRE+1
                uT = pa.tile([C, 2, B, 4 * C], bf16)

                def do_uT():
                    cnt = 0
                    for w in range(4):
                        blk = PRE - 2 + w
                        for b in range(B):
                            for dh in range(2):
                                tps = ppm.tile([C, C], bf16, tag="tr", bufs=2)
                                nc.tensor.transpose(
                                    tps[:], u_all[:, blk, b, dh * C:(dh + 1) * C], eyeb[:])
                                if cnt % 2 == 0:
                                    nc.scalar.activation(
                                        out=uT[:, dh, b, w * C:(w + 1) * C], in_=tps[:],
                                        func=AF.Copy)
                                else:
                                    nc.vector.tensor_copy(
                                        out=uT[:, dh, b, w * C:(w + 1) * C], in_=tps[:])
                                cnt += 1

                # ---------------- phase B: conv + contraction ----------------
                # order: conv(g0,i0) fills the PE while phase A drains, then
                # uT/AR, then the remaining conv blocks.
                cast_rr = 0

                def do_ar(i):
                    wdt = HW if i == 0 else C
                    col0 = 0 if i == 0 else HW + (i - 1) * C
                    base = (2 * C - HW) if i == 0 else ((1 + i) * C)
                    for ot in range(2):
                        ctp = ppt.tile([C, 512], f32, tag="ct", bufs=2)
                        step, last = 0, KU * 2 - 1
                        for j in range(KU):
                            off = base - j
                            for dh in range(2):
                                nc.tensor.matmul(
                                    ctp[:, :B * wdt],
                                    mut[:, j, dh, ot * C:(ot + 1) * C],
                                    uT[:, dh, :, off:off + wdt],
                                    start=(step == 0), stop=(step == last))
                                step += 1
                        dst = y_st[:, ot, :, col0:col0 + wdt]
                        srcv = ctp[:, :B * wdt].rearrange("p (b c) -> p b c", b=B)
                        nc.vector.tensor_add(out=dst, in0=dst, in1=srcv)

                def do_conv(g, first):
                    nonlocal cast_rr
                    nbg = _GNB[g]
                    goff = _GOFF[g]
                    ups = [pb.tile([C, 2, 2, GS, B, HW if i == 0 else C], bf16,
                                    tag=f"up{i}", name=f"up{i}", bufs=1)
                           for i in range(NIB)]
                    for b in range(B):
                        for i in range(NIB):
                            wdt = HW if i == 0 else C
                            gsw = GS * wdt
                            up = ups[i]
                            if i == 0:
                                cps = [ppc.tile([C, 512], f32, tag=f"cv0{dh}",
                                                name=f"cv0{dh}", bufs=1)
                                       for dh in range(2)]
                                for m in range(nbg):
                                    blk = PRE - 1 - m
                                    for dh in range(2):
                                        nc.tensor.matmul(
                                            cps[dh][:, :2 * gsw],
                                            u_all[:, blk, b, dh * C:(dh + 1) * C],
                                            bht[:, goff + m],
                                            start=(m == 0), stop=(m == nbg - 1))
                                for dh in range(2):
                                    srcv = cps[dh][:, :2 * gsw].rearrange(
                                        "p (s k c) -> p s k c", s=2, k=GS)
                                    nc.scalar.activation(
                                        out=up[:, 0, dh, :, b, :wdt],
                                        in_=srcv[:, 0], func=AF.Copy)
                                    nc.vector.tensor_mul(
                                        out=up[:, 1, dh, :, b, :wdt],
                                        in0=srcv[:, 1],
                                        in1=alth[:].rearrange("p (k c) -> p k c", k=GS))
                            else:
                                cps = [[ppc.tile([C, 512], f32, tag=f"cv{s}{dh}",
                                                 name=f"cv{s}{dh}", bufs=1)
                                        for dh in range(2)] for s in range(2)]
                                for m in range(nbg):
                                    blk = PRE - 1 + i - m
                                    mov = bbt[:, goff + m, :, :]
                                    for s in range(2):
                                        src = u_all if s == 0 else v_all
                                        for dh in range(2):
                                            nc.tensor.matmul(
                                                cps[s][dh][:, :gsw],
                                                src[:, blk, b, dh * C:(dh + 1) * C],
                                                mov,
                                                start=(m == 0), stop=(m == nbg - 1))
                                for s in range(2):
                                    for dh in range(2):
                                        dst = up[:, s, dh, :, b, :wdt]
                                        srcv = cps[s][dh][:, :gsw].rearrange(
                                            "p (k c) -> p k c", k=GS)
                                        if cast_rr % 2 == 0:
                                            nc.vector.tensor_copy(out=dst, in_=srcv)
                                        else:
                                            nc.scalar.activation(out=dst, in_=srcv,
                                                                 func=AF.Copy)
                                        cast_rr += 1
                    for i in range(NIB):
                        wdt = HW if i == 0 else C
                        col0 = 0 if i == 0 else HW + (i - 1) * C
                        for ot in range(2):
                            ctp = ppt.tile([C, 512], f32, tag="ct", bufs=2)
                            step, last = 0, GS * 2 * 2 - 1
                            for kl in range(GS):
                                for s in range(2):
                                    for dh in range(2):
                                        nc.tensor.matmul(
                                            ctp[:, :B * wdt],
                                            mmt[:, g, kl * 2 + s, dh, ot * C:(ot + 1) * C],
                                            ups[i][:, s, dh, kl, :, :wdt],
                                            start=(step == 0), stop=(step == last))
                                        step += 1
                            dst = y_st[:, ot, :, col0:col0 + wdt]
                            srcv = ctp[:, :B * wdt].rearrange("p (b c) -> p b c", b=B)
                            if first:
                                nc.vector.tensor_copy(out=dst, in_=srcv)
                            else:
                                nc.vector.tensor_add(out=dst, in0=dst, in1=srcv)

                do_conv(0, True)
                do_uT()
                for i in range(NIB):
                    do_ar(i)
                for g in range(1, NG):
                    do_conv(g, False)

            # ---------------- phase C: AR-scan as tap conv ----------------
            with tc.tile_pool(name="pd", bufs=1) as pd:
                for ch in range(2):
                    for ot in range(2):
                        tg = ("cv10", "cv11")[(2 * ch + ot) % 2]
                        yps = ppc.tile([C, 512], f32, tag=tg, bufs=1)
                        step, last = 0, J * 2 - 1
                        for j in range(J):
                            for dh in range(2):
                                rhs = y_st[:, dh, 2 * ch:2 * ch + 2, HW - j:HW - j + TB]
                                nc.tensor.matmul(
                                    yps[:], tpt[:, j, dh, ot * C:(ot + 1) * C], rhs,
                                    start=(step == 0), stop=(step == last))
                                step += 1
                        nc.scalar.activation(
                            out=h_st[:, ot, 2 * ch:2 * ch + 2, :],
                            in_=yps[:].rearrange("p (b c) -> p b c", b=2),
                            func=AF.Copy)

                # ---------------- phase D: SwiGLU MLP + residuals ----------------
                g_st = pd.tile([C, 8, 2, 512], bf16)
                mlp_i = 0
                for ch in range(2):
                    for hs in range(4):
                        for mtl in range(2):
                            ta, tg_ = (("cv00", "cv01"), ("cv10", "cv11"))[mlp_i % 2]
                            mlp_i += 1
                            apx = ppc.tile([C, 512], f32, tag=ta, name="apx", bufs=1)
                            gpx = ppc.tile([C, 512], f32, tag=tg_, name="gpx", bufs=1)
                            hcol = hs * 256 + mtl * C
                            for dh in range(2):
                                nc.tensor.matmul(
                                    apx[:], w1t[:, dh, hcol:hcol + C],
                                    h_st[:, dh, 2 * ch:2 * ch + 2, :],
                                    start=(dh == 0), stop=(dh == 1))
                            for dh in range(2):
                                nc.tensor.matmul(
                                    gpx[:], vvt[:, dh, hcol:hcol + C],
                                    h_st[:, dh, 2 * ch:2 * ch + 2, :],
                                    start=(dh == 0), stop=(dh == 1))
                            sil = pd.tile([C, 512], f32, tag="sil", bufs=2)
                            nc.scalar.activation(out=sil[:], in_=apx[:], func=AF.Sigmoid)
                            nc.vector.tensor_mul(out=sil[:], in0=sil[:], in1=apx[:])
                            nc.vector.tensor_mul(
                                out=g_st[:, hs * 2 + mtl, ch, :],
                                in0=sil[:], in1=gpx[:])

                    tmps = []
                    for ot in range(2):
                        ops = ppt.tile([C, 512], f32, tag="ct", bufs=2)
                        for hh in range(8):
                            nc.tensor.matmul(ops[:], w2t[:, hh, ot * C:(ot + 1) * C],
                                             g_st[:, hh, ch, :],
                                             start=(hh == 0), stop=(hh == 7))
                        tmp = pd.tile([C, 512], bf16, tag=f"tmp{ot}", bufs=1)
                        nc.vector.tensor_add(
                            out=tmp[:], in0=ops[:],
                            in1=h_st[:, ot, 2 * ch:2 * ch + 2, :])
                        tmps.append(tmp)
                    for bb_ in range(2):
                        b = 2 * ch + bb_
                        for tt in range(2):
                            osb = pd.tile([C, D], f32, tag="osb", bufs=3)
                            for ot in range(2):
                                tps = ppm.tile([C, C], bf16, tag="tr", bufs=2)
                                nc.tensor.transpose(
                                    tps[:],
                                    tmps[ot][:, bb_ * 256 + tt * C:bb_ * 256 + (tt + 1) * C],
                                    eyeb[:])
                                nc.vector.tensor_add(
                                    out=osb[:, ot * C:(ot + 1) * C], in0=tps[:],
                                    in1=xr[:, tt, b, ot * C:(ot + 1) * C])
                            nc.sync.dma_start(
                                out=out_ap[b, tt * C:(tt + 1) * C, :], in_=osb[:])

    nc.compile()
    return nc


def _to_bf16(x):
    import ml_dtypes
    u = np.ascontiguousarray(np.asarray(x, np.float32)).view(np.uint32)
    r = (u + 0x7FFF + ((u >> 16) & 1)) & 0xFFFF0000
    return np.ascontiguousarray((r >> 16).astype(np.uint16)).view(ml_dtypes.bfloat16)


def _host_prep(inputs):
    x = np.ascontiguousarray(np.asarray(inputs["x"], np.float32))
    sigma = np.asarray(inputs["sigma"], np.float64)
    phi = np.asarray(inputs["phi"], np.float64)
    rms_w = np.asarray(inputs["rms_w"], np.float64)
    M_u = np.asarray(inputs["M_u"], np.float64)
    Mp = np.asarray(inputs["M_phi_plus"], np.float64)
    Mm = np.asarray(inputs["M_phi_minus"], np.float64)
    m_y = np.asarray(inputs["m_y"], np.float64)
    w1 = np.ascontiguousarray(np.asarray(inputs["w1"], np.float32))
    v = np.ascontiguousarray(np.asarray(inputs["v"], np.float32))
    w2 = np.ascontiguousarray(np.asarray(inputs["w2"], np.float32))

    sr = np.clip(sigma, 1e-12, None) ** 0.25
    g_plus = (phi * sr[None, :]).astype(np.float32)

    # Toeplitz filter bank (plus only; minus shares it via v = alt*u),
    # partition-first: bb[tau_p, ((g,m), kl*C + tau)]
    bb = np.zeros((C, NBT, GS * C), np.float32)
    tau = np.arange(C)
    idx = tau[None, :] - tau[:, None]           # tau - tau_p
    for gi, grp in enumerate(_GROUPS):
        for kl, k in enumerate(grp):
            for m in range(min(NB[k] + 1, _GNB[gi])):
                sidx = m * C + idx
                valid = (sidx >= 0) & (sidx < NB[k] * C)
                si = np.clip(sidx, 0, T - 1)
                bb[:, _GOFF[gi] + m, kl * C:(kl + 1) * C] = np.where(
                    valid, g_plus[si, k], 0.0)
    bb = _to_bf16(bb.reshape(C, NBT * GS * C))

    # projection matrices, transposed to (d, o), rms_w folded into d rows;
    # partition-first [p, g, ks, dh, o]
    mm = np.zeros((NG, C, GS * 2, 2, D), np.float64)
    for gi, grp in enumerate(_GROUPS):
        for kl, k in enumerate(grp):
            for dh in range(2):
                wrow = rms_w[dh * C:(dh + 1) * C, None]
                mm[gi, :, kl * 2 + 0, dh, :] = Mp[k].T[dh * C:(dh + 1) * C, :] * wrow
                mm[gi, :, kl * 2 + 1, dh, :] = Mm[k].T[dh * C:(dh + 1) * C, :] * wrow
    mm = _to_bf16(mm.transpose(1, 0, 2, 3, 4).reshape(C, NG * GS * 2 * 2 * D))

    mu = np.zeros((C, KU, 2, D), np.float64)
    for j in range(KU):
        for dh in range(2):
            mu[:, j, dh, :] = M_u[j].T[dh * C:(dh + 1) * C, :] * rms_w[dh * C:(dh + 1) * C, None]
    mu = _to_bf16(mu.reshape(C, KU * 2 * D))

    # scan taps P_j (transposed), fp64 recurrence on host
    A1, A2 = m_y[0], m_y[1]
    P = [np.eye(D), A1.copy()]
    for j in range(2, J):
        P.append(A1 @ P[-1] + A2 @ P[-2])
    tp = np.zeros((C, J, 2, D), np.float64)
    for j in range(J):
        pjt = P[j].T
        tp[:, j, 0, :] = pjt[:C, :]
        tp[:, j, 1, :] = pjt[C:, :]
    tp = _to_bf16(tp.reshape(C, J * 2 * D))
    w1b = _to_bf16(w1.reshape(2, C, H).transpose(1, 0, 2).reshape(C, 2 * H))
    vb = _to_bf16(v.reshape(2, C, H).transpose(1, 0, 2).reshape(C, 2 * H))
    w2b = _to_bf16(w2.reshape(8, C, D).transpose(1, 0, 2).reshape(C, 8 * D))

    al = np.where(np.arange(C) % 2 == 0, 1.0, -1.0).astype(np.float32)
    alh = _to_bf16(np.tile(al[96:], GS))
    ey = np.eye(C, dtype=np.float32)

    # s-packed halo bank: bh[tau_p, (g,m), s, kl, t32] = g_s[m*C + 96 + t32 - tau_p]
    alt_t = np.where(np.arange(T) % 2 == 0, 1.0, -1.0)
    g_minus = (phi * alt_t[:, None] * sr[None, :]).astype(np.float32)
    bh = np.zeros((C, NBT, 2, GS, HW), np.float32)
    th = np.arange(96, C)
    idxh = th[None, :] - tau[:, None]
    for gi, grp in enumerate(_GROUPS):
        for kl, k in enumerate(grp):
            for m in range(min(NB[k] + 1, _GNB[gi])):
                sidx = m * C + idxh
                valid = (sidx >= 0) & (sidx < NB[k] * C)
                si = np.clip(sidx, 0, T - 1)
                bh[:, _GOFF[gi] + m, 0, kl, :] = np.where(valid, g_plus[si, k], 0.0)
                bh[:, _GOFF[gi] + m, 1, kl, :] = np.where(valid, g_minus[si, k], 0.0)
    bh = _to_bf16(bh.reshape(C, NBT * 2 * GS * HW))

    common = dict(bb=bb, mm=mm, mu=mu, tp=tp, w1=w1b, vv=vb, w2=w2b,
                  al=al, alh=alh, bh=bh, ey=ey)
    in_maps = []
    for c in range(NCORES):
        t0 = c * TB - PRE * C
        xwin = np.zeros((B, NXB * C, D), np.float32)
        lo = max(t0, 0)
        hi = min(t0 + NXB * C, T)
        if hi > lo:
            xwin[:, lo - t0:hi - t0, :] = x[:, lo:hi, :]
        m = dict(common)
        m["xb"] = _to_bf16(xwin)
        m["xw"] = np.ascontiguousarray(x[:, c * TB:(c + 1) * TB, :])
        in_maps.append(m)
    return in_maps


def kernel(**inputs):
    from concourse.bass_utils import run_bass_kernel_spmd
    if "nc" not in _BUILT:
        _BUILT["nc"] = _build_program()
    nc = _BUILT["nc"]
    in_maps = _host_prep(inputs)
    res = run_bass_kernel_spmd(nc, in_maps, core_ids=list(range(NCORES)))
    out = np.concatenate([res.results[c]["out"] for c in range(NCORES)], axis=1)
    return np.ascontiguousarray(out.astype(np.float32))


# revision 22
# speedup vs baseline: 1.2507x; 1.0381x over previous
"""Trainium2 Bass kernel for the STU (spectral transform unit) block. v2.

Strategy
--------
Time-shard the sequence across 8 cores (256 output steps each, halos for
causal history). Each core runs an identical SPMD program:

  rmsnorm -> causal filter-bank convolution as block-Toeplitz matmuls
  (per-filter lag truncation, filters sr-weighted) -> (k,d)->o contraction
  + AR-on-inputs taps -> output AR scan as a truncated matrix-tap
  convolution -> SwiGLU MLP -> residuals.

v2: all matmuls in bf16 (enables the compiler's fast-weight-load path;
fp32r disables it), filter truncation capped at 4 lag blocks, scan taps
truncated at J=12, the scan-halo block narrowed from 128 to 32 steps
(its filter columns are a strided slice of the main Toeplitz bank), a
single +-bank shared by conv(u) and conv(alt*u), and rms_w folded into
the contraction weights. PSUM->SBUF casts round-robin across the
vector/scalar/gpsimd engines.
"""

import contextlib
import numpy as np

# ---------------- problem constants (hardcoded shapes) ----------------
B, T, D, K, KU, KY, H = 4, 2048, 256, 24, 3, 2, 1024
NCORES = 8
TB = T // NCORES          # 256 output timesteps per core
C = 128                   # conv / tile block

# per-filter truncation: number of 128-lag blocks kept for each k (0..23)
NB = [1, 1, 1, 1, 1, 1, 1, 1, 1, 1, 3, 2, 2, 3, 3, 3, 3, 2, 2, 1, 1, 1, 1, 1]
J = 12                    # scan taps
GS = 4                    # filters per conv group
HW = 32                   # scan-halo width (needs >= J-1)
PRE = 3                   # history blocks before the core's 2-block window
NXB = PRE + 2             # u/x window blocks per core
NIB = 3                   # output regions per core: halo(32) + 2 full blocks
YW = HW + 2 * C           # y window width (288)

_ORDER = sorted(range(K), key=lambda k: -NB[k])
_GROUPS = [_ORDER[i * GS:(i + 1) * GS] for i in range(K // GS)]
# remainder lag-block only for nb=1 filters (concentrated); long filters'
# parallelogram truncation error matches their tail plateau anyway
_GNB = [max((NB[k] + 1 if NB[k] == 1 else NB[k]) for k in g) for g in _GROUPS]
NG = len(_GROUPS)
NBT = sum(_GNB)           # total (g, m) lag blocks in the bank (15)
_GOFF = [sum(_GNB[:i]) for i in range(NG)]

_BUILT = {}


def _build_program():
    import concourse.bacc as bacc
    import concourse.tile as tile
    import concourse.mybir as mybir

    f32 = mybir.dt.float32
    bf16 = mybir.dt.bfloat16
    AF = mybir.ActivationFunctionType
    ALU = mybir.AluOpType

    nc = bacc.Bacc("TRN2", target_bir_lowering=False, debug=False,
                   num_devices=NCORES)

    # ---------------- DRAM tensors ----------------
    xw_ap = nc.dram_tensor("xw", [B, 2 * C, D], f32, kind="ExternalInput").ap()
    xb_ap = nc.dram_tensor("xb", [B, NXB * C, D], bf16, kind="ExternalInput").ap()
    bb_ap = nc.dram_tensor("bb", [C, NBT * GS * C], bf16, kind="ExternalInput").ap()
    mm_ap = nc.dram_tensor("mm", [C, NG * GS * 2 * 2 * D], bf16, kind="ExternalInput").ap()
    mu_ap = nc.dram_tensor("mu", [C, KU * 2 * D], bf16, kind="ExternalInput").ap()
    tp_ap = nc.dram_tensor("tp", [C, J * 2 * D], bf16, kind="ExternalInput").ap()
    w1_ap = nc.dram_tensor("w1", [C, 2 * H], bf16, kind="ExternalInput").ap()
    vv_ap = nc.dram_tensor("vv", [C, 2 * H], bf16, kind="ExternalInput").ap()
    w2_ap = nc.dram_tensor("w2", [C, 8 * D], bf16, kind="ExternalInput").ap()
    al_ap = nc.dram_tensor("al", [C], f32, kind="ExternalInput").ap()
    alh_ap = nc.dram_tensor("alh", [GS * HW], bf16, kind="ExternalInput").ap()
    bh_ap = nc.dram_tensor("bh", [C, NBT * 2 * GS * HW], bf16, kind="ExternalInput").ap()
    ey_ap = nc.dram_tensor("ey", [C, C], f32, kind="ExternalInput").ap()
    out_ap = nc.dram_tensor("out", [B, TB, D], f32, kind="ExternalOutput").ap()

    import concourse.bass as bass

    with tile.TileContext(nc) as tc:
        ctx = contextlib.ExitStack()
        with ctx:
            p0 = ctx.enter_context(tc.tile_pool(name="p0", bufs=1))
            pc = ctx.enter_context(tc.tile_pool(name="pc", bufs=1))
            small = ctx.enter_context(tc.tile_pool(name="small", bufs=4))
            ppc = ctx.enter_context(tc.tile_pool(name="ppc", bufs=1, space="PSUM"))
            ppt = ctx.enter_context(tc.tile_pool(name="ppt", bufs=2, space="PSUM"))
            ppm = ctx.enter_context(tc.tile_pool(name="ppm", bufs=1, space="PSUM"))

            # ---------------- input window first (phase A gates everything) ----
            xta = p0.tile([C, NXB, B, D], bf16)

            def dma_xta(b):
                for blk in range(NXB):
                    nc.sync.dma_start(out=xta[:, blk, b, :],
                                      in_=xb_ap[b, blk * C:(blk + 1) * C, :])
            dma_xta(0)

            # ---------------- constants ----------------
            eye = p0.tile([C, C], f32)
            nc.sync.dma_start(out=eye[:], in_=ey_ap)
            eyeb = p0.tile([C, C], bf16)
            nc.vector.tensor_copy(out=eyeb[:], in_=eye[:])
            altc = p0.tile([C, 1], f32)
            nc.sync.dma_start(
                out=altc[:],
                in_=bass.AP(tensor=al_ap.tensor, offset=al_ap.offset,
                            ap=[[1, C], [0, 1]]))
            epst = p0.tile([C, 1], f32)
            nc.vector.memset(epst[:], 1e-6)
            ones = p0.tile([C, D], f32)
            nc.vector.memset(ones[:], 1.0)
            altB = p0.tile([C, D], bf16)
            nc.scalar.activation(out=altB[:], in_=ones[:], func=AF.Copy,
                                 scale=altc[:])
            alth = p0.tile([C, GS * HW], bf16)
            nc.sync.dma_start(
                out=alth[:],
                in_=bass.AP(tensor=alh_ap.tensor, offset=alh_ap.offset,
                            ap=[[0, C], [1, GS * HW]]))

            # ---------------- weights (DMA'd in order of first use) ----------------
            bbt = p0.tile([C, NBT, GS, C], bf16)
            bht = p0.tile([C, NBT, 2, GS, HW], bf16)
            mmt = p0.tile([C, NG, GS * 2, 2, D], bf16)
            mut = p0.tile([C, KU, 2, D], bf16)
            tpt = p0.tile([C, J, 2, D], bf16)

            def dma_bb(g):
                nc.sync.dma_start(
                    out=bbt[:, _GOFF[g]:_GOFF[g] + _GNB[g], :, :].rearrange(
                        "p a b c -> p (a b c)"),
                    in_=bb_ap[:, _GOFF[g] * GS * C:(_GOFF[g] + _GNB[g]) * GS * C])

            def dma_mm(g):
                nc.sync.dma_start(
                    out=mmt[:, g].rearrange("p a b c -> p (a b c)"),
                    in_=mm_ap[:, g * GS * 2 * 2 * D:(g + 1) * GS * 2 * 2 * D])

            nc.sync.dma_start(
                out=bht[:].rearrange("p a b c d -> p (a b c d)"), in_=bh_ap)
            dma_bb(0)
            for b in range(1, B):
                dma_xta(b)
            nc.sync.dma_start(out=mut[:].rearrange("p a b c -> p (a b c)"), in_=mu_ap)
            dma_mm(0)
            for g in range(1, NG):
                dma_bb(g)
                dma_mm(g)
            nc.sync.dma_start(out=tpt[:].rearrange("p a b c -> p (a b c)"), in_=tp_ap)
            w1t = p0.tile([C, 2, H], bf16)
            nc.sync.dma_start(out=w1t[:].rearrange("p a b -> p (a b)"), in_=w1_ap)
            vvt = p0.tile([C, 2, H], bf16)
            nc.sync.dma_start(out=vvt[:].rearrange("p a b -> p (a b)"), in_=vv_ap)
            w2t = p0.tile([C, 8, D], bf16)
            nc.sync.dma_start(out=w2t[:].rearrange("p a b -> p (a b)"), in_=w2_ap)

            # persistent activation stores
            y_st = pc.tile([C, 2, B, YW], bf16)   # spectral+ar accum (o x (b,t))
            h_st = pc.tile([C, 2, B, TB], bf16)
            xr = pc.tile([C, 2, B, D], f32)
            for w in range(2):
                for b in range(B):
                    nc.gpsimd.dma_start(out=xr[:, w, b, :],
                                        in_=xw_ap[b, w * C:(w + 1) * C, :])

            with tc.tile_pool(name="pa", bufs=1) as pa, \
                 tc.tile_pool(name="pb", bufs=1) as pb:
                # ---------------- phase A: rmsnorm (+ alt copy) ----------------
                u_all = pa.tile([C, NXB, B, D], bf16)
                v_all = pa.tile([C, NXB, B, D], bf16)
                tix = 0
                for b in range(B):
                    for blk in range(NXB):
                        xt = xta[:, blk, b, :]
                        sq = pb.tile([C, D], bf16, tag="sq", bufs=2)
                        ssum = small.tile([C, 1], f32, tag="ssum")
                        if tix % 2 == 0:
                            nc.scalar.activation(out=sq[:], in_=xt, func=AF.Square,
                                                 accum_out=ssum[:])
                        else:
                            nc.vector.tensor_mul(out=sq[:], in0=xt, in1=xt)
                            nc.vector.tensor_reduce(out=ssum[:], in_=sq[:],
                                                    axis=mybir.AxisListType.X,
                                                    op=ALU.add)
                        tix += 1
                        nc.scalar.activation(out=ssum[:], in_=ssum[:], func=AF.Sqrt,
                                             bias=epst[:], scale=1.0 / D)
                        nc.vector.reciprocal(out=ssum[:], in_=ssum[:])
                        nc.scalar.activation(out=u_all[:, blk, b, :], in_=xt,
                                             func=AF.Copy, scale=ssum[:])
                        nc.vector.tensor_mul(out=v_all[:, blk, b, :],
                                             in0=u_all[:, blk, b, :], in1=altB[:])

                # u^T for AR-on-inputs taps: blocks PRE-2 .. PRE+1
                uT = pa.tile([C, 2, B, 4 * C], bf16)

                def do_uT():
                    cnt = 0
                    for w in range(4):
                        blk = PRE - 2 + w
                        for b in range(B):
                            for dh in range(2):
                                tps = ppm.tile([C, C], bf16, tag="tr", bufs=2)
                                nc.tensor.transpose(
                                    tps[:], u_all[:, blk, b, dh * C:(dh + 1) * C], eyeb[:])
                                if cnt % 2 == 0:
                                    nc.scalar.activation(
                                        out=uT[:, dh, b, w * C:(w + 1) * C], in_=tps[:],
                                        func=AF.Copy)
                                else:
                                    nc.vector.tensor_copy(
                                        out=uT[:, dh, b, w * C:(w + 1) * C], in_=tps[:])
                                cnt += 1

                # ---------------- phase B: conv + contraction ----------------
                # order: conv(g0,i0) fills the PE while phase A drains, then
                # uT/AR, then the remaining conv blocks.
                cast_rr = 0

                def do_ar(i):
                    wdt = HW if i == 0 else C
                    col0 = 0 if i == 0 else HW + (i - 1) * C
                    base = (2 * C - HW) if i == 0 else ((1 + i) * C)
                    for ot in range(2):
                        ctp = ppt.tile([C, 512], f32, tag="ct", bufs=2)
                        step, last = 0, KU * 2 - 1
                        for j in range(KU):
                            off = base - j
                            for dh in range(2):
                                nc.tensor.matmul(
                                    ctp[:, :B * wdt],
                                    mut[:, j, dh, ot * C:(ot + 1) * C],
                                    uT[:, dh, :, off:off + wdt],
                                    start=(step == 0), stop=(step == last))
                                step += 1
                        dst = y_st[:, ot, :, col0:col0 + wdt]
                        srcv = ctp[:, :B * wdt].rearrange("p (b c) -> p b c", b=B)
                        nc.vector.tensor_add(out=dst, in0=dst, in1=srcv)

                def do_conv(g, first):
                    nonlocal cast_rr
                    nbg = _GNB[g]
                    goff = _GOFF[g]
                    ups = [pb.tile([C, 2, 2, GS, B, HW if i == 0 else C], bf16,
                                    tag=f"up{i}", name=f"up{i}", bufs=1)
                           for i in range(NIB)]
                    for b in range(B):
                        for i in range(NIB):
                            wdt = HW if i == 0 else C
                            gsw = GS * wdt
                            up = ups[i]
                            if i == 0:
                                cps = [ppc.tile([C, 512], f32, tag=f"cv0{dh}",
                                                name=f"cv0{dh}", bufs=1)
                                       for dh in range(2)]
                                for m in range(nbg):
                                    blk = PRE - 1 - m
                                    for dh in range(2):
                                        nc.tensor.matmul(
                                            cps[dh][:, :2 * gsw],
                                            u_all[:, blk, b, dh * C:(dh + 1) * C],
                                            bht[:, goff + m],
                                            start=(m == 0), stop=(m == nbg - 1))
                                for dh in range(2):
                                    srcv = cps[dh][:, :2 * gsw].rearrange(
                                        "p (s k c) -> p s k c", s=2, k=GS)
                                    nc.scalar.activation(
                                        out=up[:, 0, dh, :, b, :wdt],
                                        in_=srcv[:, 0], func=AF.Copy)
                                    nc.vector.tensor_mul(
                                        out=up[:, 1, dh, :, b, :wdt],
                                        in0=srcv[:, 1],
                                        in1=alth[:].rearrange("p (k c) -> p k c", k=GS))
                            else:
                                cps = [[ppc.tile([C, 512], f32, tag=f"cv{s}{dh}",
                                                 name=f"cv{s}{dh}", bufs=1)
                                        for dh in range(2)] for s in range(2)]
                                for m in range(nbg):
                                    blk = PRE - 1 + i - m
                                    mov = bbt[:, goff + m, :, :]
                                    for s in range(2):
                                        src = u_all if s == 0 else v_all
                                        for dh in range(2):
                                            nc.tensor.matmul(
                                                cps[s][dh][:, :gsw],
                                                src[:, blk, b, dh * C:(dh + 1) * C],
                                                mov,
                                                start=(m == 0), stop=(m == nbg - 1))
                                for s in range(2):
                                    for dh in range(2):
                                        dst = up[:, s, dh, :, b, :wdt]
                                        srcv = cps[s][dh][:, :gsw].rearrange(
                                            "p (k c) -> p k c", k=GS)
                                        if cast_rr % 2 == 0:
                                            nc.vector.tensor_copy(out=dst, in_=srcv)
                                        else:
                                            nc.scalar.activation(out=dst, in_=srcv,
                                                                 func=AF.Copy)
                                        cast_rr += 1
                    for i in range(NIB):
                        wdt = HW if i == 0 else C
                        col0 = 0 if i == 0 else HW + (i - 1) * C
                        for ot in range(2):
                            ctp = ppt.tile([C, 512], f32, tag="ct", bufs=2)
                            step, last = 0, GS * 2 * 2 - 1
                            for kl in range(GS):
                                for s in range(2):
                                    for dh in range(2):
                                        nc.tensor.matmul(
                                            ctp[:, :B * wdt],
                                            mmt[:, g, kl * 2 + s, dh, ot * C:(ot + 1) * C],
                                            ups[i][:, s, dh, kl, :, :wdt],
                                            start=(step == 0), stop=(step == last))
                                        step += 1
                            dst = y_st[:, ot, :, col0:col0 + wdt]
                            srcv = ctp[:, :B * wdt].rearrange("p (b c) -> p b c", b=B)
                            if first:
                                nc.vector.tensor_copy(out=dst, in_=srcv)
                            else:
                                nc.vector.tensor_add(out=dst, in0=dst, in1=srcv)

                do_conv(0, True)
                do_uT()
                for i in range(NIB):
                    do_ar(i)
                for g in range(1, NG):
                    do_conv(g, False)

            # ---------------- phase C: AR-scan as tap conv ----------------
            with tc.tile_pool(name="pd", bufs=1) as pd:
                for ch in range(2):
                    for ot in range(2):
                        tg = ("cv10", "cv11")[(2 * ch + ot) % 2]
                        yps = ppc.tile([C, 512], f32, tag=tg, bufs=1)
                        step, last = 0, J * 2 - 1
                        for j in range(J):
                            for dh in range(2):
                                rhs = y_st[:, dh, 2 * ch:2 * ch + 2, HW - j:HW - j + TB]
                                nc.tensor.matmul(
                                    yps[:], tpt[:, j, dh, ot * C:(ot + 1) * C], rhs,
                                    start=(step == 0), stop=(step == last))
                                step += 1
                        nc.scalar.activation(
                            out=h_st[:, ot, 2 * ch:2 * ch + 2, :],
                            in_=yps[:].rearrange("p (b c) -> p b c", b=2),
                            func=AF.Copy)

                # ---------------- phase D: SwiGLU MLP + residuals ----------------
                g_st = pd.tile([C, 8, 2, 512], bf16)
                mlp_i = 0
                for ch in range(2):
                    for hs in range(4):
                        for mtl in range(2):
                            ta, tg_ = (("cv00", "cv01"), ("cv10", "cv11"))[mlp_i % 2]
                            mlp_i += 1
                            apx = ppc.tile([C, 512], f32, tag=ta, name="apx", bufs=1)
                            gpx = ppc.tile([C, 512], f32, tag=tg_, name="gpx", bufs=1)
                            hcol = hs * 256 + mtl * C
                            for dh in range(2):
                                nc.tensor.matmul(
                                    apx[:], w1t[:, dh, hcol:hcol + C],
                                    h_st[:, dh, 2 * ch:2 * ch + 2, :],
                                    start=(dh == 0), stop=(dh == 1))
                            for dh in range(2):
                                nc.tensor.matmul(
                                    gpx[:], vvt[:, dh, hcol:hcol + C],
                                    h_st[:, dh, 2 * ch:2 * ch + 2, :],
                                    start=(dh == 0), stop=(dh == 1))
                            sil = pd.tile([C, 512], f32, tag="sil", bufs=2)
                            nc.scalar.activation(out=sil[:], in_=apx[:], func=AF.Sigmoid)
                            nc.vector.tensor_mul(out=sil[:], in0=sil[:], in1=apx[:])
                            nc.vector.tensor_mul(
                                out=g_st[:, hs * 2 + mtl, ch, :],
                                in0=sil[:], in1=gpx[:])

                    tmps = []
                    for ot in range(2):
                        ops = ppt.tile([C, 512], f32, tag="ct", bufs=2)
                        for hh in range(8):
                            nc.tensor.matmul(ops[:], w2t[:, hh, ot * C:(ot + 1) * C],
                                             g_st[:, hh, ch, :],
                                             start=(hh == 0), stop=(hh == 7))
                        tmp = pd.tile([C, 512], bf16, tag=f"tmp{ot}", bufs=1)
                        nc.vector.tensor_add(
                            out=tmp[:], in0=ops[:],
                            in1=h_st[:, ot, 2 * ch:2 * ch + 2, :])
                        tmps.append(tmp)
                    for bb_ in range(2):
                        b = 2 * ch + bb_
                        for tt in range(2):
                            osb = pd.tile([C, D], f32, tag="osb", bufs=3)
                            for ot in range(2):
                                tps = ppm.tile([C, C], bf16, tag="tr", bufs=2)
                                nc.tensor.transpose(
                                    tps[:],
                                    tmps[ot][:, bb_ * 256 + tt * C:bb_ * 256 + (tt + 1) * C],
                                    eyeb[:])
                                nc.vector.tensor_add(
                                    out=osb[:, ot * C:(ot + 1) * C], in0=tps[:],
                                    in1=xr[:, tt, b, ot * C:(ot + 1) * C])
                            nc.sync.dma_start(
                                out=out_ap[b, tt * C:(tt + 1) * C, :], in_=osb[:])

    nc.compile()
    return nc


def _to_bf16(x):
    import ml_dtypes
    u = np.ascontiguousarray(np.asarray(x, np.float32)).view(np.uint32)
    r = (u + 0x7FFF + ((u >> 16) & 1)) & 0xFFFF0000
    return np.ascontiguousarray((r >> 16).astype(np.uint16)).view(ml_dtypes.bfloat16)


def _host_prep(inputs):
    x = np.ascontiguousarray(np.asarray(inputs["x"], np.float32))
    sigma = np.asarray(inputs["sigma"], np.float64)
    phi = np.asarray(inputs["phi"], np.float64)
    rms_w = np.asarray(inputs["rms_w"], np.float64)
    M_u = np.asarray(inputs["M_u"], np.float64)
    Mp = np.asarray(inputs["M_phi_plus"], np.float64)
    Mm = np.asarray(inputs["M_phi_minus"], np.float64)
    m_y = np.asarray(inputs["m_y"], np.float64)
    w1 = np.ascontiguousarray(np.asarray(inputs["w1"], np.float32))
    v = np.ascontiguousarray(np.asarray(inputs["v"], np.float32))
    w2 = np.ascontiguousarray(np.asarray(inputs["w2"], np.float32))

    sr = np.clip(sigma, 1e-12, None) ** 0.25
    g_plus = (phi * sr[None, :]).astype(np.float32)

    # Toeplitz filter bank (plus only; minus shares it via v = alt*u),
    # partition-first: bb[tau_p, ((g,m), kl*C + tau)]
    bb = np.zeros((C, NBT, GS * C), np.float32)
    tau = np.arange(C)
    idx = tau[None, :] - tau[:, None]           # tau - tau_p
    for gi, grp in enumerate(_GROUPS):
        for kl, k in enumerate(grp):
            for m in range(min(NB[k] + 1, _GNB[gi])):
                sidx = m * C + idx
                valid = (sidx >= 0) & (sidx < NB[k] * C)
                si = np.clip(sidx, 0, T - 1)
                bb[:, _GOFF[gi] + m, kl * C:(kl + 1) * C] = np.where(
                    valid, g_plus[si, k], 0.0)
    bb = _to_bf16(bb.reshape(C, NBT * GS * C))

    # projection matrices, transposed to (d, o), rms_w folded into d rows;
    # partition-first [p, g, ks, dh, o]
    mm = np.zeros((NG, C, GS * 2, 2, D), np.float64)
    for gi, grp in enumerate(_GROUPS):
        for kl, k in enumerate(grp):
            for dh in range(2):
                wrow = rms_w[dh * C:(dh + 1) * C, None]
                mm[gi, :, kl * 2 + 0, dh, :] = Mp[k].T[dh * C:(dh + 1) * C, :] * wrow
                mm[gi, :, kl * 2 + 1, dh, :] = Mm[k].T[dh * C:(dh + 1) * C, :] * wrow
    mm = _to_bf16(mm.transpose(1, 0, 2, 3, 4).reshape(C, NG * GS * 2 * 2 * D))

    mu = np.zeros((C, KU, 2, D), np.float64)
    for j in range(KU):
        for dh in range(2):
            mu[:, j, dh, :] = M_u[j].T[dh * C:(dh + 1) * C, :] * rms_w[dh * C:(dh + 1) * C, None]
    mu = _to_bf16(mu.reshape(C, KU * 2 * D))

    # scan taps P_j (transposed), fp64 recurrence on host
    A1, A2 = m_y[0], m_y[1]
    P = [np.eye(D), A1.copy()]
    for j in range(2, J):
        P.append(A1 @ P[-1] + A2 @ P[-2])
    tp = np.zeros((C, J, 2, D), np.float64)
    for j in range(J):
        pjt = P[j].T
        tp[:, j, 0, :] = pjt[:C, :]
        tp[:, j, 1, :] = pjt[C:, :]
    tp = _to_bf16(tp.reshape(C, J * 2 * D))
    w1b = _to_bf16(w1.reshape(2, C, H).transpose(1, 0, 2).reshape(C, 2 * H))
    vb = _to_bf16(v.reshape(2, C, H).transpose(1, 0, 2).reshape(C, 2 * H))
    w2b = _to_bf16(w2.reshape(8, C, D).transpose(1, 0, 2).reshape(C, 8 * D))

    al = np.where(np.arange(C) % 2 == 0, 1.0, -1.0).astype(np.float32)
    alh = _to_bf16(np.tile(al[96:], GS))
    ey = np.eye(C, dtype=np.float32)

    # s-packed halo bank: bh[tau_p, (g,m), s, kl, t32] = g_s[m*C + 96 + t32 - tau_p]
    alt_t = np.where(np.arange(T) % 2 == 0, 1.0, -1.0)
    g_minus = (phi * alt_t[:, None] * sr[None, :]).astype(np.float32)
    bh = np.zeros((C, NBT, 2, GS, HW), np.float32)
    th = np.arange(96, C)
    idxh = th[None, :] - tau[:, None]
    for gi, grp in enumerate(_GROUPS):
        for kl, k in enumerate(grp):
            for m in range(min(NB[k] + 1, _GNB[gi])):
                sidx = m * C + idxh
                valid = (sidx >= 0) & (sidx < NB[k] * C)
                si = np.clip(sidx, 0, T - 1)
                bh[:, _GOFF[gi] + m, 0, kl, :] = np.where(valid, g_plus[si, k], 0.0)
                bh[:, _GOFF[gi] + m, 1, kl, :] = np.where(valid, g_minus[si, k], 0.0)
    bh = _to_bf16(bh.reshape(C, NBT * 2 * GS * HW))

    common = dict(bb=bb, mm=mm, mu=mu, tp=tp, w1=w1b, vv=vb, w2=w2b,
                  al=al, alh=alh, bh=bh, ey=ey)
    in_maps = []
    for c in range(NCORES):
        t0 = c * TB - PRE * C
        xwin = np.zeros((B, NXB * C, D), np.float32)
        lo = max(t0, 0)
        hi = min(t0 + NXB * C, T)
        if hi > lo:
            xwin[:, lo - t0:hi - t0, :] = x[:, lo:hi, :]
        m = dict(common)
        m["xb"] = _to_bf16(xwin)
        m["xw"] = np.ascontiguousarray(x[:, c * TB:(c + 1) * TB, :])
        in_maps.append(m)
    return in_maps


def kernel(**inputs):
    from concourse.bass_utils import run_bass_kernel_spmd
    if "nc" not in _BUILT:
        _BUILT["nc"] = _build_program()
    nc = _BUILT["nc"]
    in_maps = _host_prep(inputs)
    res = run_bass_kernel_spmd(nc, in_maps, core_ids=list(range(NCORES)))
    out = np.concatenate([res.results[c]["out"] for c in range(NCORES)], axis=1)
    return np.ascontiguousarray(out.astype(np.float32))


# revision 23
# speedup vs baseline: 1.2968x; 1.0368x over previous
"""Trainium2 Bass kernel for the STU (spectral transform unit) block. v2.

Strategy
--------
Time-shard the sequence across 8 cores (256 output steps each, halos for
causal history). Each core runs an identical SPMD program:

  rmsnorm -> causal filter-bank convolution as block-Toeplitz matmuls
  (per-filter lag truncation, filters sr-weighted) -> (k,d)->o contraction
  + AR-on-inputs taps -> output AR scan as a truncated matrix-tap
  convolution -> SwiGLU MLP -> residuals.

v2: all matmuls in bf16 (enables the compiler's fast-weight-load path;
fp32r disables it), filter truncation capped at 4 lag blocks, scan taps
truncated at J=12, the scan-halo block narrowed from 128 to 32 steps
(its filter columns are a strided slice of the main Toeplitz bank), a
single +-bank shared by conv(u) and conv(alt*u), and rms_w folded into
the contraction weights. PSUM->SBUF casts round-robin across the
vector/scalar/gpsimd engines.
"""

import contextlib
import numpy as np

# ---------------- problem constants (hardcoded shapes) ----------------
B, T, D, K, KU, KY, H = 4, 2048, 256, 24, 3, 2, 1024
NCORES = 8
TB = T // NCORES          # 256 output timesteps per core
C = 128                   # conv / tile block

# per-filter truncation: number of 128-lag blocks kept for each k (0..23)
NB = [1, 1, 1, 1, 1, 1, 1, 1, 1, 1, 3, 2, 2, 3, 3, 3, 3, 2, 2, 1, 1, 1, 1, 1]
J = 12                    # scan taps
GS = 4                    # filters per conv group
HW = 16                   # scan-halo width (needs >= J-1)
PRE = 3                   # history blocks before the core's 2-block window
NXB = PRE + 2             # u/x window blocks per core
NIB = 3                   # output regions per core: halo(32) + 2 full blocks
YW = HW + 2 * C           # y window width (288)

_ORDER = sorted(range(K), key=lambda k: -NB[k])
_GROUPS = [_ORDER[i * GS:(i + 1) * GS] for i in range(K // GS)]
# remainder lag-block only for nb=1 filters (concentrated); long filters'
# parallelogram truncation error matches their tail plateau anyway
_GNB = [max((NB[k] + 1 if NB[k] == 1 else NB[k]) for k in g) for g in _GROUPS]
NG = len(_GROUPS)
NBT = sum(_GNB)           # total (g, m) lag blocks in the bank (15)
_GOFF = [sum(_GNB[:i]) for i in range(NG)]

_BUILT = {}


def _build_program():
    import concourse.bacc as bacc
    import concourse.tile as tile
    import concourse.mybir as mybir

    f32 = mybir.dt.float32
    bf16 = mybir.dt.bfloat16
    AF = mybir.ActivationFunctionType
    ALU = mybir.AluOpType

    nc = bacc.Bacc("TRN2", target_bir_lowering=False, debug=False,
                   num_devices=NCORES)

    # ---------------- DRAM tensors ----------------
    xw_ap = nc.dram_tensor("xw", [B, 2 * C, D], f32, kind="ExternalInput").ap()
    xb_ap = nc.dram_tensor("xb", [B, NXB * C, D], bf16, kind="ExternalInput").ap()
    bb_ap = nc.dram_tensor("bb", [C, NBT * GS * C], bf16, kind="ExternalInput").ap()
    mm_ap = nc.dram_tensor("mm", [C, NG * GS * 2 * 2 * D], bf16, kind="ExternalInput").ap()
    mu_ap = nc.dram_tensor("mu", [C, KU * 2 * D], bf16, kind="ExternalInput").ap()
    tp_ap = nc.dram_tensor("tp", [C, J * 2 * D], bf16, kind="ExternalInput").ap()
    w1_ap = nc.dram_tensor("w1", [C, 2 * H], bf16, kind="ExternalInput").ap()
    vv_ap = nc.dram_tensor("vv", [C, 2 * H], bf16, kind="ExternalInput").ap()
    w2_ap = nc.dram_tensor("w2", [C, 8 * D], bf16, kind="ExternalInput").ap()
    al_ap = nc.dram_tensor("al", [C], f32, kind="ExternalInput").ap()
    alh_ap = nc.dram_tensor("alh", [GS * HW], bf16, kind="ExternalInput").ap()
    bh_ap = nc.dram_tensor("bh", [C, NBT * 2 * GS * HW], bf16, kind="ExternalInput").ap()
    ey_ap = nc.dram_tensor("ey", [C, C], f32, kind="ExternalInput").ap()
    out_ap = nc.dram_tensor("out", [B, TB, D], f32, kind="ExternalOutput").ap()

    import concourse.bass as bass

    with tile.TileContext(nc) as tc:
        ctx = contextlib.ExitStack()
        with ctx:
            p0 = ctx.enter_context(tc.tile_pool(name="p0", bufs=1))
            pc = ctx.enter_context(tc.tile_pool(name="pc", bufs=1))
            small = ctx.enter_context(tc.tile_pool(name="small", bufs=4))
            ppc = ctx.enter_context(tc.tile_pool(name="ppc", bufs=1, space="PSUM"))
            ppt = ctx.enter_context(tc.tile_pool(name="ppt", bufs=2, space="PSUM"))
            ppm = ctx.enter_context(tc.tile_pool(name="ppm", bufs=1, space="PSUM"))

            # ---------------- input window first (phase A gates everything) ----
            xta = p0.tile([C, NXB, B, D], bf16)

            def dma_xta(b):
                for blk in range(NXB):
                    nc.sync.dma_start(out=xta[:, blk, b, :],
                                      in_=xb_ap[b, blk * C:(blk + 1) * C, :])
            dma_xta(0)

            # ---------------- constants ----------------
            eye = p0.tile([C, C], f32)
            nc.sync.dma_start(out=eye[:], in_=ey_ap)
            eyeb = p0.tile([C, C], bf16)
            nc.vector.tensor_copy(out=eyeb[:], in_=eye[:])
            altc = p0.tile([C, 1], f32)
            nc.sync.dma_start(
                out=altc[:],
                in_=bass.AP(tensor=al_ap.tensor, offset=al_ap.offset,
                            ap=[[1, C], [0, 1]]))
            epst = p0.tile([C, 1], f32)
            nc.vector.memset(epst[:], 1e-6)
            ones = p0.tile([C, D], f32)
            nc.vector.memset(ones[:], 1.0)
            altB = p0.tile([C, D], bf16)
            nc.scalar.activation(out=altB[:], in_=ones[:], func=AF.Copy,
                                 scale=altc[:])
            alth = p0.tile([C, GS * HW], bf16)
            nc.sync.dma_start(
                out=alth[:],
                in_=bass.AP(tensor=alh_ap.tensor, offset=alh_ap.offset,
                            ap=[[0, C], [1, GS * HW]]))

            # ---------------- weights (DMA'd in order of first use) ----------------
            bbt = p0.tile([C, NBT, GS, C], bf16)
            bht = p0.tile([C, NBT, 2, GS, HW], bf16)
            mmt = p0.tile([C, NG, GS * 2, 2, D], bf16)
            mut = p0.tile([C, KU, 2, D], bf16)
            tpt = p0.tile([C, J, 2, D], bf16)

            def dma_bb(g):
                nc.sync.dma_start(
                    out=bbt[:, _GOFF[g]:_GOFF[g] + _GNB[g], :, :].rearrange(
                        "p a b c -> p (a b c)"),
                    in_=bb_ap[:, _GOFF[g] * GS * C:(_GOFF[g] + _GNB[g]) * GS * C])

            def dma_mm(g):
                nc.sync.dma_start(
                    out=mmt[:, g].rearrange("p a b c -> p (a b c)"),
                    in_=mm_ap[:, g * GS * 2 * 2 * D:(g + 1) * GS * 2 * 2 * D])

            nc.sync.dma_start(
                out=bht[:].rearrange("p a b c d -> p (a b c d)"), in_=bh_ap)
            dma_bb(0)
            for b in range(1, B):
                dma_xta(b)
            nc.sync.dma_start(out=mut[:].rearrange("p a b c -> p (a b c)"), in_=mu_ap)
            dma_mm(0)
            for g in range(1, NG):
                dma_bb(g)
                dma_mm(g)
            nc.sync.dma_start(out=tpt[:].rearrange("p a b c -> p (a b c)"), in_=tp_ap)
            w1t = p0.tile([C, 2, H], bf16)
            nc.sync.dma_start(out=w1t[:].rearrange("p a b -> p (a b)"), in_=w1_ap)
            vvt = p0.tile([C, 2, H], bf16)
            nc.sync.dma_start(out=vvt[:].rearrange("p a b -> p (a b)"), in_=vv_ap)
            w2t = p0.tile([C, 8, D], bf16)
            nc.sync.dma_start(out=w2t[:].rearrange("p a b -> p (a b)"), in_=w2_ap)

            # persistent activation stores
            y_st = pc.tile([C, 2, B, YW], bf16)   # spectral+ar accum (o x (b,t))
            h_st = pc.tile([C, 2, B, TB], bf16)
            xr = pc.tile([C, 2, B, D], f32)
            for w in range(2):
                for b in range(B):
                    nc.gpsimd.dma_start(out=xr[:, w, b, :],
                                        in_=xw_ap[b, w * C:(w + 1) * C, :])

            with tc.tile_pool(name="pa", bufs=1) as pa, \
                 tc.tile_pool(name="pb", bufs=1) as pb:
                # ---------------- phase A: rmsnorm (+ alt copy) ----------------
                u_all = pa.tile([C, NXB, B, D], bf16)
                v_all = pa.tile([C, NXB, B, D], bf16)
                tix = 0
                for b in range(B):
                    for blk in range(NXB):
                        xt = xta[:, blk, b, :]
                        sq = pb.tile([C, D], bf16, tag="sq", bufs=2)
                        ssum = small.tile([C, 1], f32, tag="ssum")
                        if tix % 2 == 0:
                            nc.scalar.activation(out=sq[:], in_=xt, func=AF.Square,
                                                 accum_out=ssum[:])
                        else:
                            nc.vector.tensor_mul(out=sq[:], in0=xt, in1=xt)
                            nc.vector.tensor_reduce(out=ssum[:], in_=sq[:],
                                                    axis=mybir.AxisListType.X,
                                                    op=ALU.add)
                        tix += 1
                        nc.scalar.activation(out=ssum[:], in_=ssum[:], func=AF.Sqrt,
                                             bias=epst[:], scale=1.0 / D)
                        nc.vector.reciprocal(out=ssum[:], in_=ssum[:])
                        nc.scalar.activation(out=u_all[:, blk, b, :], in_=xt,
                                             func=AF.Copy, scale=ssum[:])
                        nc.vector.tensor_mul(out=v_all[:, blk, b, :],
                                             in0=u_all[:, blk, b, :], in1=altB[:])

                # u^T for AR-on-inputs taps: blocks PRE-2 .. PRE+1
                uT = pa.tile([C, 2, B, 4 * C], bf16)

                def do_uT():
                    cnt = 0
                    for w in range(4):
                        blk = PRE - 2 + w
                        for b in range(B):
                            for dh in range(2):
                                tps = ppm.tile([C, C], bf16, tag="tr", bufs=2)
                                nc.tensor.transpose(
                                    tps[:], u_all[:, blk, b, dh * C:(dh + 1) * C], eyeb[:])
                                if cnt % 2 == 0:
                                    nc.scalar.activation(
                                        out=uT[:, dh, b, w * C:(w + 1) * C], in_=tps[:],
                                        func=AF.Copy)
                                else:
                                    nc.vector.tensor_copy(
                                        out=uT[:, dh, b, w * C:(w + 1) * C], in_=tps[:])
                                cnt += 1

                # ---------------- phase B: conv + contraction ----------------
                # order: conv(g0,i0) fills the PE while phase A drains, then
                # uT/AR, then the remaining conv blocks.
                cast_rr = 0

                def do_ar(i):
                    wdt = HW if i == 0 else C
                    col0 = 0 if i == 0 else HW + (i - 1) * C
                    base = (2 * C - HW) if i == 0 else ((1 + i) * C)
                    for ot in range(2):
                        ctp = ppt.tile([C, 512], f32, tag="ct", bufs=2)
                        step, last = 0, KU * 2 - 1
                        for j in range(KU):
                            off = base - j
                            for dh in range(2):
                                nc.tensor.matmul(
                                    ctp[:, :B * wdt],
                                    mut[:, j, dh, ot * C:(ot + 1) * C],
                                    uT[:, dh, :, off:off + wdt],
                                    start=(step == 0), stop=(step == last))
                                step += 1
                        dst = y_st[:, ot, :, col0:col0 + wdt]
                        srcv = ctp[:, :B * wdt].rearrange("p (b c) -> p b c", b=B)
                        nc.vector.tensor_add(out=dst, in0=dst, in1=srcv)

                def do_conv(g, first):
                    nonlocal cast_rr
                    nbg = _GNB[g]
                    goff = _GOFF[g]
                    ups = [pb.tile([C, 2, 2, GS, B, HW if i == 0 else C], bf16,
                                    tag=f"up{i}", name=f"up{i}", bufs=1)
                           for i in range(NIB)]
                    for b in range(B):
                        for i in range(NIB):
                            wdt = HW if i == 0 else C
                            gsw = GS * wdt
                            up = ups[i]
                            if i == 0:
                                cps = [ppc.tile([C, 512], f32, tag=f"cv0{dh}",
                                                name=f"cv0{dh}", bufs=1)
                                       for dh in range(2)]
                                for m in range(nbg):
                                    blk = PRE - 1 - m
                                    for dh in range(2):
                                        nc.tensor.matmul(
                                            cps[dh][:, :2 * gsw],
                                            u_all[:, blk, b, dh * C:(dh + 1) * C],
                                            bht[:, goff + m],
                                            start=(m == 0), stop=(m == nbg - 1))
                                for dh in range(2):
                                    srcv = cps[dh][:, :2 * gsw].rearrange(
                                        "p (s k c) -> p s k c", s=2, k=GS)
                                    nc.scalar.activation(
                                        out=up[:, 0, dh, :, b, :wdt],
                                        in_=srcv[:, 0], func=AF.Copy)
                                    nc.vector.tensor_mul(
                                        out=up[:, 1, dh, :, b, :wdt],
                                        in0=srcv[:, 1],
                                        in1=alth[:].rearrange("p (k c) -> p k c", k=GS))
                            else:
                                cps = [[ppc.tile([C, 512], f32, tag=f"cv{s}{dh}",
                                                 name=f"cv{s}{dh}", bufs=1)
                                        for dh in range(2)] for s in range(2)]
                                for m in range(nbg):
                                    blk = PRE - 1 + i - m
                                    mov = bbt[:, goff + m, :, :]
                                    for s in range(2):
                                        src = u_all if s == 0 else v_all
                                        for dh in range(2):
                                            nc.tensor.matmul(
                                                cps[s][dh][:, :gsw],
                                                src[:, blk, b, dh * C:(dh + 1) * C],
                                                mov,
                                                start=(m == 0), stop=(m == nbg - 1))
                                for s in range(2):
                                    for dh in range(2):
                                        dst = up[:, s, dh, :, b, :wdt]
                                        srcv = cps[s][dh][:, :gsw].rearrange(
                                            "p (k c) -> p k c", k=GS)
                                        if cast_rr % 2 == 0:
                                            nc.vector.tensor_copy(out=dst, in_=srcv)
                                        else:
                                            nc.scalar.activation(out=dst, in_=srcv,
                                                                 func=AF.Copy)
                                        cast_rr += 1
                    for i in range(NIB):
                        wdt = HW if i == 0 else C
                        col0 = 0 if i == 0 else HW + (i - 1) * C
                        for ot in range(2):
                            ctp = ppt.tile([C, 512], f32, tag="ct", bufs=2)
                            step, last = 0, GS * 2 * 2 - 1
                            for kl in range(GS):
                                for s in range(2):
                                    for dh in range(2):
                                        nc.tensor.matmul(
                                            ctp[:, :B * wdt],
                                            mmt[:, g, kl * 2 + s, dh, ot * C:(ot + 1) * C],
                                            ups[i][:, s, dh, kl, :, :wdt],
                                            start=(step == 0), stop=(step == last))
                                        step += 1
                            dst = y_st[:, ot, :, col0:col0 + wdt]
                            srcv = ctp[:, :B * wdt].rearrange("p (b c) -> p b c", b=B)
                            if first:
                                nc.vector.tensor_copy(out=dst, in_=srcv)
                            else:
                                nc.vector.tensor_add(out=dst, in0=dst, in1=srcv)

                do_conv(0, True)
                do_uT()
                for i in range(NIB):
                    do_ar(i)
                for g in range(1, NG):
                    do_conv(g, False)

            # ---------------- phase C: AR-scan as tap conv ----------------
            with tc.tile_pool(name="pd", bufs=1) as pd:
                for ch in range(2):
                    for ot in range(2):
                        tg = ("cv10", "cv11")[(2 * ch + ot) % 2]
                        yps = ppc.tile([C, 512], f32, tag=tg, bufs=1)
                        step, last = 0, J * 2 - 1
                        for j in range(J):
                            for dh in range(2):
                                rhs = y_st[:, dh, 2 * ch:2 * ch + 2, HW - j:HW - j + TB]
                                nc.tensor.matmul(
                                    yps[:], tpt[:, j, dh, ot * C:(ot + 1) * C], rhs,
                                    start=(step == 0), stop=(step == last))
                                step += 1
                        nc.scalar.activation(
                            out=h_st[:, ot, 2 * ch:2 * ch + 2, :],
                            in_=yps[:].rearrange("p (b c) -> p b c", b=2),
                            func=AF.Copy)

                # ---------------- phase D: SwiGLU MLP + residuals ----------------
                g_st = pd.tile([C, 8, 2, 512], bf16)
                mlp_i = 0
                for ch in range(2):
                    for hs in range(4):
                        for mtl in range(2):
                            ta, tg_ = (("cv00", "cv01"), ("cv10", "cv11"))[mlp_i % 2]
                            mlp_i += 1
                            apx = ppc.tile([C, 512], f32, tag=ta, name="apx", bufs=1)
                            gpx = ppc.tile([C, 512], f32, tag=tg_, name="gpx", bufs=1)
                            hcol = hs * 256 + mtl * C
                            for dh in range(2):
                                nc.tensor.matmul(
                                    apx[:], w1t[:, dh, hcol:hcol + C],
                                    h_st[:, dh, 2 * ch:2 * ch + 2, :],
                                    start=(dh == 0), stop=(dh == 1))
                            for dh in range(2):
                                nc.tensor.matmul(
                                    gpx[:], vvt[:, dh, hcol:hcol + C],
                                    h_st[:, dh, 2 * ch:2 * ch + 2, :],
                                    start=(dh == 0), stop=(dh == 1))
                            sil = pd.tile([C, 512], f32, tag="sil", bufs=2)
                            nc.scalar.activation(out=sil[:], in_=apx[:], func=AF.Sigmoid)
                            nc.vector.tensor_mul(out=sil[:], in0=sil[:], in1=apx[:])
                            nc.vector.tensor_mul(
                                out=g_st[:, hs * 2 + mtl, ch, :],
                                in0=sil[:], in1=gpx[:])

                    tmps = []
                    for ot in range(2):
                        ops = ppt.tile([C, 512], f32, tag="ct", bufs=2)
                        for hh in range(8):
                            nc.tensor.matmul(ops[:], w2t[:, hh, ot * C:(ot + 1) * C],
                                             g_st[:, hh, ch, :],
                                             start=(hh == 0), stop=(hh == 7))
                        tmp = pd.tile([C, 512], bf16, tag=f"tmp{ot}", bufs=1)
                        nc.vector.tensor_add(
                            out=tmp[:], in0=ops[:],
                            in1=h_st[:, ot, 2 * ch:2 * ch + 2, :])
                        tmps.append(tmp)
                    for bb_ in range(2):
                        b = 2 * ch + bb_
                        for tt in range(2):
                            osb = pd.tile([C, D], f32, tag="osb", bufs=3)
                            for ot in range(2):
                                tps = ppm.tile([C, C], bf16, tag="tr", bufs=2)
                                nc.tensor.transpose(
                                    tps[:],
                                    tmps[ot][:, bb_ * 256 + tt * C:bb_ * 256 + (tt + 1) * C],
                                    eyeb[:])
                                nc.vector.tensor_add(
                                    out=osb[:, ot * C:(ot + 1) * C], in0=tps[:],
                                    in1=xr[:, tt, b, ot * C:(ot + 1) * C])
                            nc.sync.dma_start(
                                out=out_ap[b, tt * C:(tt + 1) * C, :], in_=osb[:])

    nc.compile()
    return nc


def _to_bf16(x):
    import ml_dtypes
    u = np.ascontiguousarray(np.asarray(x, np.float32)).view(np.uint32)
    r = (u + 0x7FFF + ((u >> 16) & 1)) & 0xFFFF0000
    return np.ascontiguousarray((r >> 16).astype(np.uint16)).view(ml_dtypes.bfloat16)


def _host_prep(inputs):
    x = np.ascontiguousarray(np.asarray(inputs["x"], np.float32))
    sigma = np.asarray(inputs["sigma"], np.float64)
    phi = np.asarray(inputs["phi"], np.float64)
    rms_w = np.asarray(inputs["rms_w"], np.float64)
    M_u = np.asarray(inputs["M_u"], np.float64)
    Mp = np.asarray(inputs["M_phi_plus"], np.float64)
    Mm = np.asarray(inputs["M_phi_minus"], np.float64)
    m_y = np.asarray(inputs["m_y"], np.float64)
    w1 = np.ascontiguousarray(np.asarray(inputs["w1"], np.float32))
    v = np.ascontiguousarray(np.asarray(inputs["v"], np.float32))
    w2 = np.ascontiguousarray(np.asarray(inputs["w2"], np.float32))

    sr = np.clip(sigma, 1e-12, None) ** 0.25
    g_plus = (phi * sr[None, :]).astype(np.float32)

    # Toeplitz filter bank (plus only; minus shares it via v = alt*u),
    # partition-first: bb[tau_p, ((g,m), kl*C + tau)]
    bb = np.zeros((C, NBT, GS * C), np.float32)
    tau = np.arange(C)
    idx = tau[None, :] - tau[:, None]           # tau - tau_p
    for gi, grp in enumerate(_GROUPS):
        for kl, k in enumerate(grp):
            for m in range(min(NB[k] + 1, _GNB[gi])):
                sidx = m * C + idx
                valid = (sidx >= 0) & (sidx < NB[k] * C)
                si = np.clip(sidx, 0, T - 1)
                bb[:, _GOFF[gi] + m, kl * C:(kl + 1) * C] = np.where(
                    valid, g_plus[si, k], 0.0)
    bb = _to_bf16(bb.reshape(C, NBT * GS * C))

    # projection matrices, transposed to (d, o), rms_w folded into d rows;
    # partition-first [p, g, ks, dh, o]
    mm = np.zeros((NG, C, GS * 2, 2, D), np.float64)
    for gi, grp in enumerate(_GROUPS):
        for kl, k in enumerate(grp):
            for dh in range(2):
                wrow = rms_w[dh * C:(dh + 1) * C, None]
                mm[gi, :, kl * 2 + 0, dh, :] = Mp[k].T[dh * C:(dh + 1) * C, :] * wrow
                mm[gi, :, kl * 2 + 1, dh, :] = Mm[k].T[dh * C:(dh + 1) * C, :] * wrow
    mm = _to_bf16(mm.transpose(1, 0, 2, 3, 4).reshape(C, NG * GS * 2 * 2 * D))

    mu = np.zeros((C, KU, 2, D), np.float64)
    for j in range(KU):
        for dh in range(2):
            mu[:, j, dh, :] = M_u[j].T[dh * C:(dh + 1) * C, :] * rms_w[dh * C:(dh + 1) * C, None]
    mu = _to_bf16(mu.reshape(C, KU * 2 * D))

    # scan taps P_j (transposed), fp64 recurrence on host
    A1, A2 = m_y[0], m_y[1]
    P = [np.eye(D), A1.copy()]
    for j in range(2, J):
        P.append(A1 @ P[-1] + A2 @ P[-2])
    tp = np.zeros((C, J, 2, D), np.float64)
    for j in range(J):
        pjt = P[j].T
        tp[:, j, 0, :] = pjt[:C, :]
        tp[:, j, 1, :] = pjt[C:, :]
    tp = _to_bf16(tp.reshape(C, J * 2 * D))
    w1b = _to_bf16(w1.reshape(2, C, H).transpose(1, 0, 2).reshape(C, 2 * H))
    vb = _to_bf16(v.reshape(2, C, H).transpose(1, 0, 2).reshape(C, 2 * H))
    w2b = _to_bf16(w2.reshape(8, C, D).transpose(1, 0, 2).reshape(C, 8 * D))

    al = np.where(np.arange(C) % 2 == 0, 1.0, -1.0).astype(np.float32)
    alh = _to_bf16(np.tile(al[C - HW:], GS))
    ey = np.eye(C, dtype=np.float32)

    # s-packed halo bank: bh[tau_p, (g,m), s, kl, t32] = g_s[m*C + 96 + t32 - tau_p]
    alt_t = np.where(np.arange(T) % 2 == 0, 1.0, -1.0)
    g_minus = (phi * alt_t[:, None] * sr[None, :]).astype(np.float32)
    bh = np.zeros((C, NBT, 2, GS, HW), np.float32)
    th = np.arange(C - HW, C)
    idxh = th[None, :] - tau[:, None]
    for gi, grp in enumerate(_GROUPS):
        for kl, k in enumerate(grp):
            for m in range(min(NB[k] + 1, _GNB[gi])):
                sidx = m * C + idxh
                valid = (sidx >= 0) & (sidx < NB[k] * C)
                si = np.clip(sidx, 0, T - 1)
                bh[:, _GOFF[gi] + m, 0, kl, :] = np.where(valid, g_plus[si, k], 0.0)
                bh[:, _GOFF[gi] + m, 1, kl, :] = np.where(valid, g_minus[si, k], 0.0)
    bh = _to_bf16(bh.reshape(C, NBT * 2 * GS * HW))

    common = dict(bb=bb, mm=mm, mu=mu, tp=tp, w1=w1b, vv=vb, w2=w2b,
                  al=al, alh=alh, bh=bh, ey=ey)
    in_maps = []
    for c in range(NCORES):
        t0 = c * TB - PRE * C
        xwin = np.zeros((B, NXB * C, D), np.float32)
        lo = max(t0, 0)
        hi = min(t0 + NXB * C, T)
        if hi > lo:
            xwin[:, lo - t0:hi - t0, :] = x[:, lo:hi, :]
        m = dict(common)
        m["xb"] = _to_bf16(xwin)
        m["xw"] = np.ascontiguousarray(x[:, c * TB:(c + 1) * TB, :])
        in_maps.append(m)
    return in_maps


def kernel(**inputs):
    from concourse.bass_utils import run_bass_kernel_spmd
    if "nc" not in _BUILT:
        _BUILT["nc"] = _build_program()
    nc = _BUILT["nc"]
    in_maps = _host_prep(inputs)
    res = run_bass_kernel_spmd(nc, in_maps, core_ids=list(range(NCORES)))
    out = np.concatenate([res.results[c]["out"] for c in range(NCORES)], axis=1)
    return np.ascontiguousarray(out.astype(np.float32))


# revision 24
# speedup vs baseline: 1.3402x; 1.0335x over previous
"""Trainium2 Bass kernel for the STU (spectral transform unit) block. v2.

Strategy
--------
Time-shard the sequence across 8 cores (256 output steps each, halos for
causal history). Each core runs an identical SPMD program:

  rmsnorm -> causal filter-bank convolution as block-Toeplitz matmuls
  (per-filter lag truncation, filters sr-weighted) -> (k,d)->o contraction
  + AR-on-inputs taps -> output AR scan as a truncated matrix-tap
  convolution -> SwiGLU MLP -> residuals.

v2: all matmuls in bf16 (enables the compiler's fast-weight-load path;
fp32r disables it), filter truncation capped at 4 lag blocks, scan taps
truncated at J=12, the scan-halo block narrowed from 128 to 32 steps
(its filter columns are a strided slice of the main Toeplitz bank), a
single +-bank shared by conv(u) and conv(alt*u), and rms_w folded into
the contraction weights. PSUM->SBUF casts round-robin across the
vector/scalar/gpsimd engines.
"""

import contextlib
import numpy as np

# ---------------- problem constants (hardcoded shapes) ----------------
B, T, D, K, KU, KY, H = 4, 2048, 256, 24, 3, 2, 1024
NCORES = 8
TB = T // NCORES          # 256 output timesteps per core
C = 128                   # conv / tile block

# per-filter truncation: number of 128-lag blocks kept for each k (0..23)
NB = [1, 1, 1, 1, 1, 1, 1, 1, 1, 1, 3, 2, 2, 3, 3, 3, 2, 2, 2, 1, 1, 1, 1, 1]
J = 11                    # scan taps
GS = 4                    # filters per conv group
HW = 16                   # scan-halo width (needs >= J-1)
PRE = 3                   # history blocks before the core's 2-block window
NXB = PRE + 2             # u/x window blocks per core
NIB = 3                   # output regions per core: halo(32) + 2 full blocks
YW = HW + 2 * C           # y window width (288)

_ORDER = sorted(range(K), key=lambda k: -NB[k])
_GROUPS = [_ORDER[i * GS:(i + 1) * GS] for i in range(K // GS)]
# remainder lag-block only for nb=1 filters (concentrated); long filters'
# parallelogram truncation error matches their tail plateau anyway
_GNB = [max((NB[k] + 1 if NB[k] == 1 else NB[k]) for k in g) for g in _GROUPS]
NG = len(_GROUPS)
NBT = sum(_GNB)           # total (g, m) lag blocks in the bank (15)
_GOFF = [sum(_GNB[:i]) for i in range(NG)]

_BUILT = {}


def _build_program():
    import concourse.bacc as bacc
    import concourse.tile as tile
    import concourse.mybir as mybir

    f32 = mybir.dt.float32
    bf16 = mybir.dt.bfloat16
    AF = mybir.ActivationFunctionType
    ALU = mybir.AluOpType

    nc = bacc.Bacc("TRN2", target_bir_lowering=False, debug=False,
                   num_devices=NCORES)

    # ---------------- DRAM tensors ----------------
    xw_ap = nc.dram_tensor("xw", [B, 2 * C, D], f32, kind="ExternalInput").ap()
    xb_ap = nc.dram_tensor("xb", [B, NXB * C, D], bf16, kind="ExternalInput").ap()
    bb_ap = nc.dram_tensor("bb", [C, NBT * GS * C], bf16, kind="ExternalInput").ap()
    mm_ap = nc.dram_tensor("mm", [C, NG * GS * 2 * 2 * D], bf16, kind="ExternalInput").ap()
    mu_ap = nc.dram_tensor("mu", [C, KU * 2 * D], bf16, kind="ExternalInput").ap()
    tp_ap = nc.dram_tensor("tp", [C, J * 2 * D], bf16, kind="ExternalInput").ap()
    w1_ap = nc.dram_tensor("w1", [C, 2 * H], bf16, kind="ExternalInput").ap()
    vv_ap = nc.dram_tensor("vv", [C, 2 * H], bf16, kind="ExternalInput").ap()
    w2_ap = nc.dram_tensor("w2", [C, 8 * D], bf16, kind="ExternalInput").ap()
    al_ap = nc.dram_tensor("al", [C], f32, kind="ExternalInput").ap()
    alh_ap = nc.dram_tensor("alh", [GS * HW], bf16, kind="ExternalInput").ap()
    bh_ap = nc.dram_tensor("bh", [C, NBT * 2 * GS * HW], bf16, kind="ExternalInput").ap()
    ey_ap = nc.dram_tensor("ey", [C, C], f32, kind="ExternalInput").ap()
    out_ap = nc.dram_tensor("out", [B, TB, D], f32, kind="ExternalOutput").ap()

    import concourse.bass as bass

    with tile.TileContext(nc) as tc:
        ctx = contextlib.ExitStack()
        with ctx:
            p0 = ctx.enter_context(tc.tile_pool(name="p0", bufs=1))
            pc = ctx.enter_context(tc.tile_pool(name="pc", bufs=1))
            small = ctx.enter_context(tc.tile_pool(name="small", bufs=4))
            ppc = ctx.enter_context(tc.tile_pool(name="ppc", bufs=1, space="PSUM"))
            ppt = ctx.enter_context(tc.tile_pool(name="ppt", bufs=2, space="PSUM"))
            ppm = ctx.enter_context(tc.tile_pool(name="ppm", bufs=1, space="PSUM"))

            # ---------------- input window first (phase A gates everything) ----
            xta = p0.tile([C, NXB, B, D], bf16)

            def dma_xta(b):
                for blk in range(NXB):
                    nc.sync.dma_start(out=xta[:, blk, b, :],
                                      in_=xb_ap[b, blk * C:(blk + 1) * C, :])
            dma_xta(0)

            # ---------------- constants ----------------
            eye = p0.tile([C, C], f32)
            nc.sync.dma_start(out=eye[:], in_=ey_ap)
            eyeb = p0.tile([C, C], bf16)
            nc.vector.tensor_copy(out=eyeb[:], in_=eye[:])
            altc = p0.tile([C, 1], f32)
            nc.sync.dma_start(
                out=altc[:],
                in_=bass.AP(tensor=al_ap.tensor, offset=al_ap.offset,
                            ap=[[1, C], [0, 1]]))
            epst = p0.tile([C, 1], f32)
            nc.vector.memset(epst[:], 1e-6)
            ones = p0.tile([C, D], f32)
            nc.vector.memset(ones[:], 1.0)
            altB = p0.tile([C, D], bf16)
            nc.scalar.activation(out=altB[:], in_=ones[:], func=AF.Copy,
                                 scale=altc[:])
            alth = p0.tile([C, GS * HW], bf16)
            nc.sync.dma_start(
                out=alth[:],
                in_=bass.AP(tensor=alh_ap.tensor, offset=alh_ap.offset,
                            ap=[[0, C], [1, GS * HW]]))

            # ---------------- weights (DMA'd in order of first use) ----------------
            bbt = p0.tile([C, NBT, GS, C], bf16)
            bht = p0.tile([C, NBT, 2, GS, HW], bf16)
            mmt = p0.tile([C, NG, GS * 2, 2, D], bf16)
            mut = p0.tile([C, KU, 2, D], bf16)
            tpt = p0.tile([C, J, 2, D], bf16)

            def dma_bb(g):
                nc.sync.dma_start(
                    out=bbt[:, _GOFF[g]:_GOFF[g] + _GNB[g], :, :].rearrange(
                        "p a b c -> p (a b c)"),
                    in_=bb_ap[:, _GOFF[g] * GS * C:(_GOFF[g] + _GNB[g]) * GS * C])

            def dma_mm(g):
                nc.sync.dma_start(
                    out=mmt[:, g].rearrange("p a b c -> p (a b c)"),
                    in_=mm_ap[:, g * GS * 2 * 2 * D:(g + 1) * GS * 2 * 2 * D])

            nc.sync.dma_start(
                out=bht[:].rearrange("p a b c d -> p (a b c d)"), in_=bh_ap)
            dma_bb(0)
            for b in range(1, B):
                dma_xta(b)
            nc.sync.dma_start(out=mut[:].rearrange("p a b c -> p (a b c)"), in_=mu_ap)
            dma_mm(0)
            for g in range(1, NG):
                dma_bb(g)
                dma_mm(g)
            nc.sync.dma_start(out=tpt[:].rearrange("p a b c -> p (a b c)"), in_=tp_ap)
            w1t = p0.tile([C, 2, H], bf16)
            nc.sync.dma_start(out=w1t[:].rearrange("p a b -> p (a b)"), in_=w1_ap)
            vvt = p0.tile([C, 2, H], bf16)
            nc.sync.dma_start(out=vvt[:].rearrange("p a b -> p (a b)"), in_=vv_ap)
            w2t = p0.tile([C, 8, D], bf16)
            nc.sync.dma_start(out=w2t[:].rearrange("p a b -> p (a b)"), in_=w2_ap)

            # persistent activation stores
            y_st = pc.tile([C, 2, B, YW], bf16)   # spectral+ar accum (o x (b,t))
            h_st = pc.tile([C, 2, B, TB], bf16)
            xr = pc.tile([C, 2, B, D], f32)
            for w in range(2):
                for b in range(B):
                    nc.gpsimd.dma_start(out=xr[:, w, b, :],
                                        in_=xw_ap[b, w * C:(w + 1) * C, :])

            with tc.tile_pool(name="pa", bufs=1) as pa, \
                 tc.tile_pool(name="pb", bufs=1) as pb:
                # ---------------- phase A: rmsnorm (+ alt copy) ----------------
                u_all = pa.tile([C, NXB, B, D], bf16)
                v_all = pa.tile([C, NXB, B, D], bf16)
                tix = 0
                for b in range(B):
                    for blk in range(NXB):
                        xt = xta[:, blk, b, :]
                        sq = pb.tile([C, D], bf16, tag="sq", bufs=2)
                        ssum = small.tile([C, 1], f32, tag="ssum")
                        if tix % 2 == 0:
                            nc.scalar.activation(out=sq[:], in_=xt, func=AF.Square,
                                                 accum_out=ssum[:])
                        else:
                            nc.vector.tensor_mul(out=sq[:], in0=xt, in1=xt)
                            nc.vector.tensor_reduce(out=ssum[:], in_=sq[:],
                                                    axis=mybir.AxisListType.X,
                                                    op=ALU.add)
                        tix += 1
                        nc.scalar.activation(out=ssum[:], in_=ssum[:], func=AF.Sqrt,
                                             bias=epst[:], scale=1.0 / D)
                        nc.vector.reciprocal(out=ssum[:], in_=ssum[:])
                        nc.scalar.activation(out=u_all[:, blk, b, :], in_=xt,
                                             func=AF.Copy, scale=ssum[:])
                        nc.vector.tensor_mul(out=v_all[:, blk, b, :],
                                             in0=u_all[:, blk, b, :], in1=altB[:])

                # u^T for AR-on-inputs taps: blocks PRE-2 .. PRE+1
                uT = pa.tile([C, 2, B, 4 * C], bf16)

                def do_uT():
                    cnt = 0
                    for w in range(4):
                        blk = PRE - 2 + w
                        for b in range(B):
                            for dh in range(2):
                                tps = ppm.tile([C, C], bf16, tag="tr", bufs=2)
                                nc.tensor.transpose(
                                    tps[:], u_all[:, blk, b, dh * C:(dh + 1) * C], eyeb[:])
                                if cnt % 2 == 0:
                                    nc.scalar.activation(
                                        out=uT[:, dh, b, w * C:(w + 1) * C], in_=tps[:],
                                        func=AF.Copy)
                                else:
                                    nc.vector.tensor_copy(
                                        out=uT[:, dh, b, w * C:(w + 1) * C], in_=tps[:])
                                cnt += 1

                # ---------------- phase B: conv + contraction ----------------
                # order: conv(g0,i0) fills the PE while phase A drains, then
                # uT/AR, then the remaining conv blocks.
                cast_rr = 0

                def do_ar(i):
                    wdt = HW if i == 0 else C
                    col0 = 0 if i == 0 else HW + (i - 1) * C
                    base = (2 * C - HW) if i == 0 else ((1 + i) * C)
                    for ot in range(2):
                        ctp = ppt.tile([C, 512], f32, tag="ct", bufs=2)
                        step, last = 0, KU * 2 - 1
                        for j in range(KU):
                            off = base - j
                            for dh in range(2):
                                nc.tensor.matmul(
                                    ctp[:, :B * wdt],
                                    mut[:, j, dh, ot * C:(ot + 1) * C],
                                    uT[:, dh, :, off:off + wdt],
                                    start=(step == 0), stop=(step == last))
                                step += 1
                        dst = y_st[:, ot, :, col0:col0 + wdt]
                        srcv = ctp[:, :B * wdt].rearrange("p (b c) -> p b c", b=B)
                        nc.vector.tensor_add(out=dst, in0=dst, in1=srcv)

                def do_conv(g, first):
                    nonlocal cast_rr
                    nbg = _GNB[g]
                    goff = _GOFF[g]
                    ups = [pb.tile([C, 2, 2, GS, B, HW if i == 0 else C], bf16,
                                    tag=f"up{i}", name=f"up{i}", bufs=1)
                           for i in range(NIB)]
                    for b in range(B):
                        for i in range(NIB):
                            wdt = HW if i == 0 else C
                            gsw = GS * wdt
                            up = ups[i]
                            if i == 0:
                                cps = [ppc.tile([C, 512], f32, tag=f"cv0{dh}",
                                                name=f"cv0{dh}", bufs=1)
                                       for dh in range(2)]
                                for m in range(nbg):
                                    blk = PRE - 1 - m
                                    for dh in range(2):
                                        nc.tensor.matmul(
                                            cps[dh][:, :2 * gsw],
                                            u_all[:, blk, b, dh * C:(dh + 1) * C],
                                            bht[:, goff + m],
                                            start=(m == 0), stop=(m == nbg - 1))
                                for dh in range(2):
                                    srcv = cps[dh][:, :2 * gsw].rearrange(
                                        "p (s k c) -> p s k c", s=2, k=GS)
                                    nc.scalar.activation(
                                        out=up[:, 0, dh, :, b, :wdt],
                                        in_=srcv[:, 0], func=AF.Copy)
                                    nc.vector.tensor_mul(
                                        out=up[:, 1, dh, :, b, :wdt],
                                        in0=srcv[:, 1],
                                        in1=alth[:].rearrange("p (k c) -> p k c", k=GS))
                            else:
                                cps = [[ppc.tile([C, 512], f32, tag=f"cv{s}{dh}",
                                                 name=f"cv{s}{dh}", bufs=1)
                                        for dh in range(2)] for s in range(2)]
                                for m in range(nbg):
                                    blk = PRE - 1 + i - m
                                    mov = bbt[:, goff + m, :, :]
                                    for s in range(2):
                                        src = u_all if s == 0 else v_all
                                        for dh in range(2):
                                            nc.tensor.matmul(
                                                cps[s][dh][:, :gsw],
                                                src[:, blk, b, dh * C:(dh + 1) * C],
                                                mov,
                                                start=(m == 0), stop=(m == nbg - 1))
                                for s in range(2):
                                    for dh in range(2):
                                        dst = up[:, s, dh, :, b, :wdt]
                                        srcv = cps[s][dh][:, :gsw].rearrange(
                                            "p (k c) -> p k c", k=GS)
                                        if cast_rr % 2 == 0:
                                            nc.vector.tensor_copy(out=dst, in_=srcv)
                                        else:
                                            nc.scalar.activation(out=dst, in_=srcv,
                                                                 func=AF.Copy)
                                        cast_rr += 1
                    for i in range(NIB):
                        wdt = HW if i == 0 else C
                        col0 = 0 if i == 0 else HW + (i - 1) * C
                        for ot in range(2):
                            ctp = ppt.tile([C, 512], f32, tag="ct", bufs=2)
                            step, last = 0, GS * 2 * 2 - 1
                            for kl in range(GS):
                                for s in range(2):
                                    for dh in range(2):
                                        nc.tensor.matmul(
                                            ctp[:, :B * wdt],
                                            mmt[:, g, kl * 2 + s, dh, ot * C:(ot + 1) * C],
                                            ups[i][:, s, dh, kl, :, :wdt],
                                            start=(step == 0), stop=(step == last))
                                        step += 1
                            dst = y_st[:, ot, :, col0:col0 + wdt]
                            srcv = ctp[:, :B * wdt].rearrange("p (b c) -> p b c", b=B)
                            if first:
                                nc.vector.tensor_copy(out=dst, in_=srcv)
                            else:
                                nc.vector.tensor_add(out=dst, in0=dst, in1=srcv)

                do_conv(0, True)
                do_uT()
                for i in range(NIB):
                    do_ar(i)
                for g in range(1, NG):
                    do_conv(g, False)

            # ---------------- phase C: AR-scan as tap conv ----------------
            with tc.tile_pool(name="pd", bufs=1) as pd:
                for ch in range(2):
                    for ot in range(2):
                        tg = ("cv10", "cv11")[(2 * ch + ot) % 2]
                        yps = ppc.tile([C, 512], f32, tag=tg, bufs=1)
                        step, last = 0, J * 2 - 1
                        for j in range(J):
                            for dh in range(2):
                                rhs = y_st[:, dh, 2 * ch:2 * ch + 2, HW - j:HW - j + TB]
                                nc.tensor.matmul(
                                    yps[:], tpt[:, j, dh, ot * C:(ot + 1) * C], rhs,
                                    start=(step == 0), stop=(step == last))
                                step += 1
                        nc.scalar.activation(
                            out=h_st[:, ot, 2 * ch:2 * ch + 2, :],
                            in_=yps[:].rearrange("p (b c) -> p b c", b=2),
                            func=AF.Copy)

                # ---------------- phase D: SwiGLU MLP + residuals ----------------
                g_st = pd.tile([C, 8, 2, 512], bf16)
                mlp_i = 0
                for ch in range(2):
                    for hs in range(4):
                        for mtl in range(2):
                            ta, tg_ = (("cv00", "cv01"), ("cv10", "cv11"))[mlp_i % 2]
                            mlp_i += 1
                            apx = ppc.tile([C, 512], f32, tag=ta, name="apx", bufs=1)
                            gpx = ppc.tile([C, 512], f32, tag=tg_, name="gpx", bufs=1)
                            hcol = hs * 256 + mtl * C
                            for dh in range(2):
                                nc.tensor.matmul(
                                    apx[:], w1t[:, dh, hcol:hcol + C],
                                    h_st[:, dh, 2 * ch:2 * ch + 2, :],
                                    start=(dh == 0), stop=(dh == 1))
                            for dh in range(2):
                                nc.tensor.matmul(
                                    gpx[:], vvt[:, dh, hcol:hcol + C],
                                    h_st[:, dh, 2 * ch:2 * ch + 2, :],
                                    start=(dh == 0), stop=(dh == 1))
                            sil = pd.tile([C, 512], f32, tag="sil", bufs=2)
                            nc.scalar.activation(out=sil[:], in_=apx[:], func=AF.Sigmoid)
                            nc.vector.tensor_mul(out=sil[:], in0=sil[:], in1=apx[:])
                            nc.vector.tensor_mul(
                                out=g_st[:, hs * 2 + mtl, ch, :],
                                in0=sil[:], in1=gpx[:])

                    tmps = []
                    for ot in range(2):
                        ops = ppt.tile([C, 512], f32, tag="ct", bufs=2)
                        for hh in range(8):
                            nc.tensor.matmul(ops[:], w2t[:, hh, ot * C:(ot + 1) * C],
                                             g_st[:, hh, ch, :],
                                             start=(hh == 0), stop=(hh == 7))
                        tmp = pd.tile([C, 512], bf16, tag=f"tmp{ot}", bufs=1)
                        nc.vector.tensor_add(
                            out=tmp[:], in0=ops[:],
                            in1=h_st[:, ot, 2 * ch:2 * ch + 2, :])
                        tmps.append(tmp)
                    for bb_ in range(2):
                        b = 2 * ch + bb_
                        for tt in range(2):
                            osb = pd.tile([C, D], f32, tag="osb", bufs=3)
                            for ot in range(2):
                                tps = ppm.tile([C, C], bf16, tag="tr", bufs=2)
                                nc.tensor.transpose(
                                    tps[:],
                                    tmps[ot][:, bb_ * 256 + tt * C:bb_ * 256 + (tt + 1) * C],
                                    eyeb[:])
                                nc.vector.tensor_add(
                                    out=osb[:, ot * C:(ot + 1) * C], in0=tps[:],
                                    in1=xr[:, tt, b, ot * C:(ot + 1) * C])
                            nc.sync.dma_start(
                                out=out_ap[b, tt * C:(tt + 1) * C, :], in_=osb[:])

    nc.compile()
    return nc


def _to_bf16(x):
    import ml_dtypes
    u = np.ascontiguousarray(np.asarray(x, np.float32)).view(np.uint32)
    r = (u + 0x7FFF + ((u >> 16) & 1)) & 0xFFFF0000
    return np.ascontiguousarray((r >> 16).astype(np.uint16)).view(ml_dtypes.bfloat16)


def _host_prep(inputs):
    x = np.ascontiguousarray(np.asarray(inputs["x"], np.float32))
    sigma = np.asarray(inputs["sigma"], np.float64)
    phi = np.asarray(inputs["phi"], np.float64)
    rms_w = np.asarray(inputs["rms_w"], np.float64)
    M_u = np.asarray(inputs["M_u"], np.float64)
    Mp = np.asarray(inputs["M_phi_plus"], np.float64)
    Mm = np.asarray(inputs["M_phi_minus"], np.float64)
    m_y = np.asarray(inputs["m_y"], np.float64)
    w1 = np.ascontiguousarray(np.asarray(inputs["w1"], np.float32))
    v = np.ascontiguousarray(np.asarray(inputs["v"], np.float32))
    w2 = np.ascontiguousarray(np.asarray(inputs["w2"], np.float32))

    sr = np.clip(sigma, 1e-12, None) ** 0.25
    g_plus = (phi * sr[None, :]).astype(np.float32)

    # Toeplitz filter bank (plus only; minus shares it via v = alt*u),
    # partition-first: bb[tau_p, ((g,m), kl*C + tau)]
    bb = np.zeros((C, NBT, GS * C), np.float32)
    tau = np.arange(C)
    idx = tau[None, :] - tau[:, None]           # tau - tau_p
    for gi, grp in enumerate(_GROUPS):
        for kl, k in enumerate(grp):
            for m in range(min(NB[k] + 1, _GNB[gi])):
                sidx = m * C + idx
                valid = (sidx >= 0) & (sidx < NB[k] * C)
                si = np.clip(sidx, 0, T - 1)
                bb[:, _GOFF[gi] + m, kl * C:(kl + 1) * C] = np.where(
                    valid, g_plus[si, k], 0.0)
    bb = _to_bf16(bb.reshape(C, NBT * GS * C))

    # projection matrices, transposed to (d, o), rms_w folded into d rows;
    # partition-first [p, g, ks, dh, o]
    mm = np.zeros((NG, C, GS * 2, 2, D), np.float64)
    for gi, grp in enumerate(_GROUPS):
        for kl, k in enumerate(grp):
            for dh in range(2):
                wrow = rms_w[dh * C:(dh + 1) * C, None]
                mm[gi, :, kl * 2 + 0, dh, :] = Mp[k].T[dh * C:(dh + 1) * C, :] * wrow
                mm[gi, :, kl * 2 + 1, dh, :] = Mm[k].T[dh * C:(dh + 1) * C, :] * wrow
    mm = _to_bf16(mm.transpose(1, 0, 2, 3, 4).reshape(C, NG * GS * 2 * 2 * D))

    mu = np.zeros((C, KU, 2, D), np.float64)
    for j in range(KU):
        for dh in range(2):
            mu[:, j, dh, :] = M_u[j].T[dh * C:(dh + 1) * C, :] * rms_w[dh * C:(dh + 1) * C, None]
    mu = _to_bf16(mu.reshape(C, KU * 2 * D))

    # scan taps P_j (transposed), fp64 recurrence on host
    A1, A2 = m_y[0], m_y[1]
    P = [np.eye(D), A1.copy()]
    for j in range(2, J):
        P.append(A1 @ P[-1] + A2 @ P[-2])
    tp = np.zeros((C, J, 2, D), np.float64)
    for j in range(J):
        pjt = P[j].T
        tp[:, j, 0, :] = pjt[:C, :]
        tp[:, j, 1, :] = pjt[C:, :]
    tp = _to_bf16(tp.reshape(C, J * 2 * D))
    w1b = _to_bf16(w1.reshape(2, C, H).transpose(1, 0, 2).reshape(C, 2 * H))
    vb = _to_bf16(v.reshape(2, C, H).transpose(1, 0, 2).reshape(C, 2 * H))
    w2b = _to_bf16(w2.reshape(8, C, D).transpose(1, 0, 2).reshape(C, 8 * D))

    al = np.where(np.arange(C) % 2 == 0, 1.0, -1.0).astype(np.float32)
    alh = _to_bf16(np.tile(al[C - HW:], GS))
    ey = np.eye(C, dtype=np.float32)

    # s-packed halo bank: bh[tau_p, (g,m), s, kl, t32] = g_s[m*C + 96 + t32 - tau_p]
    alt_t = np.where(np.arange(T) % 2 == 0, 1.0, -1.0)
    g_minus = (phi * alt_t[:, None] * sr[None, :]).astype(np.float32)
    bh = np.zeros((C, NBT, 2, GS, HW), np.float32)
    th = np.arange(C - HW, C)
    idxh = th[None, :] - tau[:, None]
    for gi, grp in enumerate(_GROUPS):
        for kl, k in enumerate(grp):
            for m in range(min(NB[k] + 1, _GNB[gi])):
                sidx = m * C + idxh
                valid = (sidx >= 0) & (sidx < NB[k] * C)
                si = np.clip(sidx, 0, T - 1)
                bh[:, _GOFF[gi] + m, 0, kl, :] = np.where(valid, g_plus[si, k], 0.0)
                bh[:, _GOFF[gi] + m, 1, kl, :] = np.where(valid, g_minus[si, k], 0.0)
    bh = _to_bf16(bh.reshape(C, NBT * 2 * GS * HW))

    common = dict(bb=bb, mm=mm, mu=mu, tp=tp, w1=w1b, vv=vb, w2=w2b,
                  al=al, alh=alh, bh=bh, ey=ey)
    in_maps = []
    for c in range(NCORES):
        t0 = c * TB - PRE * C
        xwin = np.zeros((B, NXB * C, D), np.float32)
        lo = max(t0, 0)
        hi = min(t0 + NXB * C, T)
        if hi > lo:
            xwin[:, lo - t0:hi - t0, :] = x[:, lo:hi, :]
        m = dict(common)
        m["xb"] = _to_bf16(xwin)
        m["xw"] = np.ascontiguousarray(x[:, c * TB:(c + 1) * TB, :])
        in_maps.append(m)
    return in_maps


def kernel(**inputs):
    from concourse.bass_utils import run_bass_kernel_spmd
    if "nc" not in _BUILT:
        _BUILT["nc"] = _build_program()
    nc = _BUILT["nc"]
    in_maps = _host_prep(inputs)
    res = run_bass_kernel_spmd(nc, in_maps, core_ids=list(range(NCORES)))
    out = np.concatenate([res.results[c]["out"] for c in range(NCORES)], axis=1)
    return np.ascontiguousarray(out.astype(np.float32))
